# revision 1
# baseline (speedup 1.0000x reference)
"""Multi-head attention (B=4, S=2048, E=768, H=12, D=64, causal) on 8 trn2
NeuronCores.

Sharding: core c -> batch b = c//2, head-half g = c%2 (6 heads each).
Each core computes its 6 heads' attention plus the partial output
projection; the host sums the two half-head partials per batch.

On-device strategy (per core):
  - Host pre-transposes x[b] and the weight slices so every matmul
    contraction dim (e / d / k / e_h) lands on SBUF partitions; no
    on-device transposes.
  - QK projection emits qk^T [f, s]; V projection emits V [s, f] --
    exactly the operand orientations the attention matmuls need.
  - Scores are computed TRANSPOSED (S^T[k, q] = K^T Q) so the exp'd
    tiles E^T[k, q] feed the ctx matmul (ctx^T = V_aug^T E^T) directly.
  - A ones-column packed into V_aug makes the PE compute the softmax
    row-sums as ctx^T row 64 for free.
  - Causal masking: fully-masked tiles skipped; diagonal tiles get a
    -1e9 strict-lower-triangle added via a bf16 matmul into the same
    PSUM accumulation group.
  - Softmax normalization: rinv = 1/rowsum (DVE), broadcast across
    partitions with a K=1 matmul, multiplied in while leaving PSUM.
  - All big matmuls run in float32r (fp32 storage, 8-bit-mantissa
    matmul) at 1 cycle/row: 4x faster than fp32 matmul on trn2.
  - ctx matmuls are software-pipelined one step behind the scores
    matmuls so the in-order PE never stalls waiting for ACT's exp.
  - One PSUM pool with fixed tags (16 KB/partition exactly) is shared
    by all phases so work pipelines through buffer rotation.
"""
import sys, json, os

for _p in ("/opt/trn_rl_repo",):
    if _p not in sys.path and os.path.isdir(_p):
        sys.path.insert(0, _p)

import numpy as np
import concourse.bass as bass
import concourse.mybir as mybir
import concourse.tile as tile
from concourse.bass_utils import run_bass_kernel_spmd

B, S, E, H, D = 4, 2048, 768, 12, 64
HPC = H // 2          # heads per core = 6
FPC = HPC * D         # features per core per q/k/v = 384
EC = E // 128         # 6 contraction chunks for projections
SC = S // 128         # 16 s-chunks
QW = S // 512         # 4 q-windows
KC = S // 128         # 16 k-chunks
F32 = mybir.dt.float32
F32R = mybir.dt.float32r
BF16 = mybir.dt.bfloat16
EXP = mybir.ActivationFunctionType.Exp
NEG = -1.0e9


def round_f32r(a: np.ndarray) -> np.ndarray:
    """Round fp32 -> fp32r (8 explicit mantissa bits), RNE, as fp32 bits."""
    a = np.ascontiguousarray(a, dtype=np.float32)
    u = a.view(np.uint32).astype(np.uint64)
    u2 = (u + 0x3FFF + ((u >> 15) & 1)) & np.uint64(0xFFFF8000)
    return u2.astype(np.uint32).view(np.float32)


def _patch_multiwait(nc, max_waits=1):
    """This container's walrus rejects instructions with more than one sync
    wait. Split excess waits onto same-engine NOPs emitted immediately
    before the instruction (same-engine streams are order-preserving)."""
    raw = nc.to_json_bytes()
    m = json.loads(raw)
    for f in m["functions"]:
        for b in f["blocks"]:
            out = []
            for inst in b["instructions"]:
                si = inst.get("sync_info") or {}
                ws = si.get("on_wait") or []
                if len(ws) > max_waits:
                    eng = inst["engine"]
                    for i, w in enumerate(ws[:-max_waits]):
                        out.append({
                            "debug": inst.get("debug", 0), "engine": eng,
                            "ins": [], "name": inst["name"] + f"-mw{i}",
                            "opcode": "NoOp", "outs": [],
                            "sync_info": {"on_update": [], "on_wait": [w]},
                        })
                    si["on_wait"] = ws[-max_waits:]
                out.append(inst)
            b["instructions"] = out
    patched = json.dumps(m).encode()
    nc.to_json_bytes = lambda: patched
    return nc


def build_nc(repeat=1, with_bias=True):
    nc = bass.Bass()
    xT = nc.dram_tensor("xT", [E, S], F32R, kind="ExternalInput")
    wqkT = nc.dram_tensor("wqkT", [E, 2 * FPC], F32R, kind="ExternalInput")
    wvT = nc.dram_tensor("wvT", [E, FPC], F32R, kind="ExternalInput")
    woT = nc.dram_tensor("woT", [FPC, E], F32R, kind="ExternalInput")
    bqk = nc.dram_tensor("bqk", [128, 2 * FPC // 128], F32, kind="ExternalInput")
    bv = nc.dram_tensor("bv", [1, FPC], F32R, kind="ExternalInput")
    bo = nc.dram_tensor("bo", [1, E], F32R, kind="ExternalInput")
    tri = nc.dram_tensor("tri", [128, 128], BF16, kind="ExternalInput")
    ident = nc.dram_tensor("ident", [128, 128], BF16, kind="ExternalInput")
    ones = nc.dram_tensor("ones", [1, 128], F32R, kind="ExternalInput")
    y = nc.dram_tensor("y", [S, E], F32, kind="ExternalOutput")

    with tile.TileContext(nc) as tc, \
         nc.allow_low_precision(reason="f32r matmul pipeline by design"):
        with tc.tile_pool(name="persist", bufs=1) as P, \
             tc.tile_pool(name="ps", bufs=1, space="PSUM") as PS:
            # --- persistent tiles (bottom-of-stack, live whole kernel)
            qkT_sb = [P.tile([128, S], F32R, name=f"qkT{i}") for i in range(6)]
            V_sb = [P.tile([128, 65 * HPC], F32R, name=f"V{i}") for i in range(KC)]
            ctxT_sb = [P.tile([128, S], F32R, name=f"ctxT{i}") for i in range(3)]
            woT_sb = [P.tile([128, E], F32R, name=f"woT{i}") for i in range(3)]
            bqk_sb = P.tile([128, 6], F32, name="bqk_sb")
            bv_sb = P.tile([1, FPC], F32R, name="bv_sb")
            bo_sb = P.tile([1, E], F32R, name="bo_sb")
            tri_sb = P.tile([128, 128], BF16, name="tri_sb")
            id_sb = P.tile([128, 128], BF16, name="id_sb")
            on_sb = P.tile([1, 128], F32R, name="on_sb")

            def ps_tile(shape, tag, bufs):
                return PS.tile(shape, F32, name=tag, tag=tag, bufs=bufs)

            # ================= phase 1: projections =================
            with tc.tile_pool(name="inp", bufs=1) as PI:
                xT_sb = [PI.tile([128, S], F32R, name=f"xT{i}") for i in range(EC)]
                wqkT_sb = [PI.tile([128, 2 * FPC], F32R, name=f"wqkT{i}")
                           for i in range(EC)]
                wvT_sb = [PI.tile([128, FPC], F32R, name=f"wvT{i}")
                          for i in range(EC)]
                # DMA order: the qk chains consume (xT[ec] all windows,
                # wqkT[ec] col fo=0) in e-chunk order -- ship exactly that.
                nc.sync.dma_start(bqk_sb[:], bqk.ap())
                for i in range(EC):
                    nc.sync.dma_start(xT_sb[i][:, 0:1024],
                                      xT.ap()[128 * i:128 * (i + 1), 0:1024])
                    nc.sync.dma_start(xT_sb[i][:, 1024:S],
                                      xT.ap()[128 * i:128 * (i + 1), 1024:S])
                    nc.sync.dma_start(wqkT_sb[i][:, 0:128],
                                      wqkT.ap()[128 * i:128 * (i + 1), 0:128])
                for i in range(EC):
                    nc.sync.dma_start(
                        wqkT_sb[i][:, 128:2 * FPC],
                        wqkT.ap()[128 * i:128 * (i + 1), 128:2 * FPC])
                for i in range(EC):
                    nc.sync.dma_start(wvT_sb[i][:],
                                      wvT.ap()[128 * i:128 * (i + 1), :])
                nc.sync.dma_start(bv_sb[:], bv.ap())
                nc.sync.dma_start(tri_sb[:], tri.ap())
                nc.sync.dma_start(id_sb[:], ident.ap())
                nc.sync.dma_start(on_sb[:], ones.ap())
                for i in range(3):
                    nc.sync.dma_start(woT_sb[i][:],
                                      woT.ap()[128 * i:128 * (i + 1), :])
                nc.sync.dma_start(bo_sb[:], bo.ap())

                # qk-proj: 4 concurrent s-window chains per f-chunk so the
                # stationary wqkT block is loaded once per e-chunk (the PE
                # elides repeated self-loads) and consecutive matmuls hit
                # alternating PSUM banks.
                for fo in range(6):
                    pairs = [ps_tile([128, 1024], "pss_t", 2) for _ in range(2)]
                    for ecc in range(EC):
                        for sw in range(QW):
                            nc.tensor.matmul(
                                pairs[sw // 2][:, 512 * (sw % 2):
                                               512 * (sw % 2 + 1)],
                                wqkT_sb[ecc][:, 128 * fo:128 * (fo + 1)],
                                xT_sb[ecc][:, 512 * sw:512 * (sw + 1)],
                                start=(ecc == 0), stop=(ecc == EC - 1),
                                skip_group_check=True)
                    for p in range(2):
                        if with_bias:
                            nc.vector.tensor_scalar_add(
                                qkT_sb[fo][:, 1024 * p:1024 * (p + 1)],
                                pairs[p][:], bqk_sb[:, fo:fo + 1])
                        else:
                            nc.vector.tensor_copy(
                                qkT_sb[fo][:, 1024 * p:1024 * (p + 1)],
                                pairs[p][:])
                # V-proj: interleave s-chunk pairs across two PSUM banks
                for scp in range(SC // 2):
                    psvs = [ps_tile([128, FPC], "psc_t", 2) for _ in range(2)]
                    for ecc in range(EC):
                        for p in range(2):
                            sc = 2 * scp + p
                            nc.tensor.matmul(
                                psvs[p][:],
                                xT_sb[ecc][:, 128 * sc:128 * (sc + 1)],
                                wvT_sb[ecc][:],
                                start=(ecc == 0),
                                stop=(not with_bias and ecc == EC - 1),
                                skip_group_check=True)
                    for p in range(2):
                        sc = 2 * scp + p
                        if with_bias:
                            nc.tensor.matmul(psvs[p][:], on_sb[:, 0:128],
                                             bv_sb[:], start=False, stop=True,
                                             skip_group_check=True)
                        vv = V_sb[sc][:].rearrange("p (h x) -> p h x", x=65)
                        nc.vector.tensor_copy(
                            vv[:, :, 0:64],
                            psvs[p][:].rearrange("p (h x) -> p h x", x=64))
                        nc.gpsimd.memset(vv[:, :, 64:65].bitcast(F32), 1.0)

            # ================= phase 2+3: attention + out-proj ==============
            with tc.tile_pool(name="esb", bufs=6) as EP, \
                 tc.tile_pool(name="nrm", bufs=6) as NP, \
                 tc.tile_pool(name="osb", bufs=3) as OP:

                def emit_scores(hp, qw, u):
                    """Scores (pair of k-chunks) for both heads + exp on the
                    [128,1024] pair tile. Returns {hd: (pss, E)}."""
                    qT, kT = qkT_sb[hp], qkT_sb[3 + hp]
                    Es = {}
                    for hd in range(2):
                        Es[hd] = (ps_tile([128, 1024], "pss_t", 2),
                                  EP.tile([128, 1024], F32R, name="E_t"))
                    # strict row-group alternation (base 0,64,0,64) so the
                    # K=64 score matmul pairs run concurrently on the PE
                    for half in range(2):
                        ki = 2 * u + half
                        j = ki - 4 * qw
                        diag = j >= 0
                        for hd in range(2):
                            base = 64 * hd
                            pss = Es[hd][0]
                            nc.tensor.matmul(
                                pss[:, 512 * half:512 * (half + 1)],
                                kT[base:base + 64, 128 * ki:128 * (ki + 1)],
                                qT[base:base + 64, 512 * qw:512 * (qw + 1)],
                                start=True, stop=not diag,
                                skip_group_check=True)
                        if diag:
                            for hd in range(2):
                                pss = Es[hd][0]
                                nc.tensor.matmul(
                                    pss[:, 512 * half + 128 * j:
                                        512 * half + 128 * (j + 1)],
                                    id_sb[:], tri_sb[:],
                                    start=False, stop=True,
                                    skip_group_check=True)
                    for hd in range(2):
                        pss, Et = Es[hd]
                        j0 = 2 * u - 4 * qw
                        c0 = 128 * j0 if j0 > 0 else 0
                        c1 = 128 * (j0 + 1) if j0 + 1 > 0 else 0
                        # one exp spans both halves; the gap cols
                        # [512, 512+c1) hold raw scores that no ctx matmul
                        # ever streams, so exp'ing them is harmless
                        nc.scalar.activation(Et[:, c0:1024], pss[:, c0:1024],
                                             EXP, scale=0.125)
                    return Es

                def emit_ctx(hp, qw, u, Es, psc):
                    nki = 4 * qw + 4
                    for half in range(2):
                        ki = 2 * u + half
                        j = ki - 4 * qw
                        c = 128 * j if j > 0 else 0
                        for hd in range(2):
                            _, Et = Es[hd]
                            h = 2 * hp + hd
                            nc.tensor.matmul(
                                psc[hd][:, c:512],
                                V_sb[ki][:, 65 * h:65 * h + 65],
                                Et[:, 512 * half + c:512 * (half + 1)],
                                start=(ki == 0), stop=(ki == nki - 1),
                                skip_group_check=True)

                def emit_norm(hp, qw, psc):
                    """Copy ctx'+rowsum out of PSUM (freeing it), then
                    normalize into ctxT."""
                    for hd in range(2):
                        craw = NP.tile([65, 512], F32, name="craw_t")
                        nc.vector.tensor_copy(craw[:], psc[hd][:])
                        rinv = NP.tile([1, 512], F32R, name="rinv_t")
                        nc.vector.reciprocal(rinv[:], craw[64:65, :])
                        pb = ps_tile([64, 512], "pb_t", 1)
                        nc.tensor.matmul(pb[:], on_sb[:, 0:64], rinv[:],
                                         start=True, stop=True)
                        bc = NP.tile([64, 512], F32, name="bc_t")
                        nc.vector.tensor_copy(bc[:], pb[:])
                        nc.vector.tensor_mul(
                            ctxT_sb[hp][64 * hd:64 * (hd + 1),
                                        512 * qw:512 * (qw + 1)],
                            craw[0:64, :], bc[:])

                def emit_outproj(qw):
                    for sc in range(4 * qw, 4 * qw + 4):
                        osb = OP.tile([128, E], F32, name="osb_t")
                        # two f-window chains on two PSUM slots, c-outer so
                        # the ctxT stationary is loaded once per c
                        pos = {0: ps_tile([128, 512], "po_t", 1),
                               512: ps_tile([128, 512], "pb_t", 1)}
                        for c in range(3):
                            for f0, fn in ((0, 512), (512, 256)):
                                nc.tensor.matmul(
                                    pos[f0][:, 0:fn],
                                    ctxT_sb[c][:, 128 * sc:128 * (sc + 1)],
                                    woT_sb[c][:, f0:f0 + fn],
                                    start=(c == 0),
                                    stop=(not with_bias and c == 2),
                                    skip_group_check=True)
                        for f0, fn in ((0, 512), (512, 256)):
                            if with_bias:
                                nc.tensor.matmul(pos[f0][:, 0:fn],
                                                 on_sb[:, 0:128],
                                                 bo_sb[:, f0:f0 + fn],
                                                 start=False, stop=True,
                                                 skip_group_check=True)
                            nc.vector.tensor_copy(osb[:, f0:f0 + fn],
                                                  pos[f0][:, 0:fn])
                        nc.sync.dma_start(y.ap()[128 * sc:128 * (sc + 1), :],
                                          osb[:])

                def emit_attention():
                    # software pipeline: ctx trails scores by one step
                    # (depth 2 measured slower on HW: the 2-slot ctx
                    # accumulator rotation serializes across head pairs)
                    DEPTH = 1
                    pending = []   # [(hp, qw, u, Es, psc, last_u), ...]

                    def flush_one():
                        php, pqw, pu, pEs, ppsc, plast = pending.pop(0)
                        emit_ctx(php, pqw, pu, pEs, ppsc)
                        if pu == plast:
                            emit_norm(php, pqw, ppsc)
                            if php == 2:
                                emit_outproj(pqw)

                    for qw in range(QW):
                        for hp in range(3):
                            nu = (4 * qw + 4) // 2
                            psc = {hd: ps_tile([65, 512], "psc_t", 2)
                                   for hd in range(2)}
                            for u in range(nu):
                                Es = emit_scores(hp, qw, u)
                                if len(pending) >= DEPTH:
                                    flush_one()
                                pending.append((hp, qw, u, Es, psc, nu - 1))
                    while pending:
                        flush_one()

                if repeat == 1:
                    emit_attention()
                else:
                    with tc.For_i(0, repeat, 1):
                        emit_attention()

    return _patch_multiwait(nc)


_NC = {}


def _get_nc(with_bias=True):
    if with_bias not in _NC:
        _NC[with_bias] = build_nc(with_bias=with_bias)
    return _NC[with_bias]


def _prep_core_inputs(x, in_proj_w, in_proj_b, out_w, out_b):
    """Build the 8 per-core input dicts (host-side shard + transpose)."""
    import ml_dtypes
    tri_np = np.where(np.arange(128)[None, :] < np.arange(128)[:, None],
                      np.float32(NEG), np.float32(0.0))
    tri_bf = tri_np.astype(ml_dtypes.bfloat16)
    id_bf = np.eye(128, dtype=np.float32).astype(ml_dtypes.bfloat16)
    ones_np = round_f32r(np.ones((1, 128), np.float32))

    xT_by_b = [round_f32r(np.asarray(x[b]).T) for b in range(B)]

    in_maps = []
    for c in range(8):
        b = c // 2
        g = c % 2
        f0 = FPC * g
        Wq = np.asarray(in_proj_w[f0:f0 + FPC])
        Wk = np.asarray(in_proj_w[E + f0:E + f0 + FPC])
        Wv = np.asarray(in_proj_w[2 * E + f0:2 * E + f0 + FPC])
        bq = np.asarray(in_proj_b[f0:f0 + FPC])
        bk = np.asarray(in_proj_b[E + f0:E + f0 + FPC])
        bvv = np.asarray(in_proj_b[2 * E + f0:2 * E + f0 + FPC])
        Wo = np.asarray(out_w[:, f0:f0 + FPC])
        bqk_np = np.concatenate([bq, bk]).astype(np.float32).reshape(6, 128).T
        in_maps.append({
            "xT": xT_by_b[b],
            "wqkT": round_f32r(np.concatenate([Wq, Wk], axis=0).T),
            "wvT": round_f32r(Wv.T),
            "woT": round_f32r(Wo.T),
            "bqk": np.ascontiguousarray(bqk_np),
            "bv": round_f32r(bvv.reshape(1, FPC)),
            # out bias only on even cores so the host-side pair-sum is exact
            "bo": round_f32r(np.asarray(out_b).reshape(1, E)) if g == 0
                  else np.zeros((1, E), np.float32),
            "tri": tri_bf,
            "ident": id_bf,
            "ones": ones_np,
        })
    return in_maps


def kernel(x, in_proj_w, in_proj_b, out_w, out_b):
    zero_bias = (not np.any(np.asarray(in_proj_b))) and \
                (not np.any(np.asarray(out_b)))
    nc = _get_nc(with_bias=not zero_bias)
    in_maps = _prep_core_inputs(x, in_proj_w, in_proj_b, out_w, out_b)
    res = run_bass_kernel_spmd(nc, in_maps, core_ids=list(range(8)))
    out = np.empty((B, S, E), np.float32)
    for b in range(B):
        out[b] = res.results[2 * b]["y"] + res.results[2 * b + 1]["y"]
    return out



# revision 48
# speedup vs baseline: 1.1677x; 1.1677x over previous
"""Multi-head attention (B=4, S=2048, E=768, H=12, D=64, causal) on 8 trn2
NeuronCores.

Sharding: core c -> batch b = c//2, head-half g = c%2 (6 heads each).
Each core computes its 6 heads' attention plus the partial output
projection; the host sums the two half-head partials per batch.

On-device strategy (per core):
  - Host pre-transposes x[b] and the weight slices so every matmul
    contraction dim (e / d / k / e_h) lands on SBUF partitions; no
    on-device transposes of inputs. Everything ships bf16 (halves DMA).
  - QK projection emits qk^T [f, s]; V projection emits V [s, f] with a
    ones column packed per head (V_aug) so the PE computes softmax
    row-sums for free.
  - Scores are computed TRANSPOSED (S^T[k, q] = K^T Q) in bf16; the
    diagonal tiles shrink their moving window to skip fully-masked
    columns. Causal masking inside the diagonal 128x128 block is a DVE
    multiply of the exp'd tile by a 0/1 lower-triangle (no PE matmul).
  - One merged exp per unit covers both heads' scores ([128, 2048]
    activation) to amortize the ACT access-latency init.
  - ctx is computed with E^T chunks STATIONARY and V_aug [k, 65] MOVING:
    65-cycle matmuls, and fully-masked (q-chunk, k-chunk) blocks are
    skipped entirely. The result lands [q, d+1] with q on partitions, so
    softmax normalization is a per-partition reciprocal + scalar
    multiply on DVE (no PE broadcast matmul), and a PE transpose brings
    ctx^T [d, q] back for the output projection.
  - All big matmuls run in bf16 at 1 cycle/row.
  - ctx matmuls are software-pipelined one unit behind the scores
    matmuls so the in-order PE never stalls waiting for ACT's exp.
"""
import sys, json, os

for _p in ("/opt/trn_rl_repo",):
    if _p not in sys.path and os.path.isdir(_p):
        sys.path.insert(0, _p)

import numpy as np
import concourse.bass as bass
import concourse.mybir as mybir
import concourse.tile as tile
from concourse.bass_utils import run_bass_kernel_spmd

B, S, E, H, D = 4, 2048, 768, 12, 64
HPC = H // 2          # heads per core = 6
FPC = HPC * D         # features per core per q/k/v = 384
EC = E // 128         # 6 contraction chunks for projections
SC = S // 128         # 16 s-chunks
QW = S // 512         # 4 q-windows
KC = S // 128         # 16 k-chunks
F32 = mybir.dt.float32
BF16 = mybir.dt.bfloat16
EXP = mybir.ActivationFunctionType.Exp


def _patch_multiwait(nc, max_waits=1):
    """This container's walrus rejects instructions with more than one sync
    wait. Split excess waits onto same-engine NOPs emitted immediately
    before the instruction (same-engine streams are order-preserving)."""
    raw = nc.to_json_bytes()
    m = json.loads(raw)
    for f in m["functions"]:
        for b in f["blocks"]:
            out = []
            for inst in b["instructions"]:
                si = inst.get("sync_info") or {}
                ws = si.get("on_wait") or []
                if len(ws) > max_waits:
                    eng = inst["engine"]
                    for i, w in enumerate(ws[:-max_waits]):
                        out.append({
                            "debug": inst.get("debug", 0), "engine": eng,
                            "ins": [], "name": inst["name"] + f"-mw{i}",
                            "opcode": "NoOp", "outs": [],
                            "sync_info": {"on_update": [], "on_wait": [w]},
                        })
                    si["on_wait"] = ws[-max_waits:]
                out.append(inst)
            b["instructions"] = out
    patched = json.dumps(m).encode()
    nc.to_json_bytes = lambda: patched
    return nc


def build_nc(with_bias=True):
    nc = bass.Bass()
    xT = nc.dram_tensor("xT", [E, S], BF16, kind="ExternalInput")
    wqkT = nc.dram_tensor("wqkT", [E, 2 * FPC], BF16, kind="ExternalInput")
    wvT = nc.dram_tensor("wvT", [E, FPC], BF16, kind="ExternalInput")
    woT = nc.dram_tensor("woT", [FPC, E], BF16, kind="ExternalInput")
    bqk = nc.dram_tensor("bqk", [128, 2 * FPC // 128], F32, kind="ExternalInput")
    bv = nc.dram_tensor("bv", [1, FPC], BF16, kind="ExternalInput")
    bo = nc.dram_tensor("bo", [1, E], BF16, kind="ExternalInput")
    tri = nc.dram_tensor("tri", [128, 128], BF16, kind="ExternalInput")
    ident = nc.dram_tensor("ident", [128, 128], BF16, kind="ExternalInput")
    ones = nc.dram_tensor("ones", [1, 128], BF16, kind="ExternalInput")
    y = nc.dram_tensor("y", [S, E], F32, kind="ExternalOutput")

    with tile.TileContext(nc) as tc, \
         nc.allow_low_precision(reason="bf16 matmul pipeline by design"):
        with tc.tile_pool(name="persist", bufs=1) as P, \
             tc.tile_pool(name="ps", bufs=1, space="PSUM") as PS:
            # --- persistent tiles (bottom-of-stack, live whole kernel)
            qkT_sb = [P.tile([128, S], BF16, name=f"qkT{i}") for i in range(6)]
            V_sb = [P.tile([128, 65 * HPC], BF16, name=f"V{i}") for i in range(KC)]
            ctxT_sb = [P.tile([128, S], BF16, name=f"ctxT{i}") for i in range(3)]
            woT_sb = [P.tile([128, E], BF16, name=f"woT{i}") for i in range(3)]
            bqk_sb = P.tile([128, 6], F32, name="bqk_sb")
            bv_sb = P.tile([1, FPC], BF16, name="bv_sb")
            bo_sb = P.tile([1, E], BF16, name="bo_sb")
            tri_sb = P.tile([128, 128], BF16, name="tri_sb")
            id_sb = P.tile([128, 128], BF16, name="id_sb")
            on_sb = P.tile([1, 128], BF16, name="on_sb")

            def ps_tile(shape, tag, bufs, dtype=F32):
                return PS.tile(shape, dtype, name=tag, tag=tag, bufs=bufs)

            # ============ phase 1 (projections) + attention, interleaved ====
            # The attention phase is ACT(exp)-throughput-bound, so the
            # projections are software-pipelined INTO the attention loop:
            # only the chains needed for the first scores run up front, and
            # the rest are emitted between attention units where the PE has
            # slack while ACT chews on exps.
            with tc.tile_pool(name="inp", bufs=1) as PI, \
                 tc.tile_pool(name="esb", bufs=14) as EP, \
                 tc.tile_pool(name="nrm", bufs=8) as NP, \
                 tc.tile_pool(name="osb", bufs=3) as OP:
                # consolidated phase-1 tiles: one DMA dispatch covers all six
                # e-chunks (the SP sequencer costs ~650ns per DMA, so fewer,
                # bigger strided DMAs win)
                xT_sb = PI.tile([128, EC * S], BF16, name="xT_all")
                wqkT_sb = PI.tile([128, EC * 2 * FPC], BF16, name="wqkT_all")
                wvT_sb = PI.tile([128, EC * FPC], BF16, name="wvT_all")
                xs = xT_sb[:].rearrange("p (e s) -> p e s", e=EC)
                xd = xT.ap().rearrange("(e p) s -> p e s", p=128)
                qs = wqkT_sb[:].rearrange("p (e f) -> p e f", e=EC)
                qd = wqkT.ap().rearrange("(e p) f -> p e f", p=128)
                # DMA order: first the tensors gating the two startup chains
                # (wqkT cols of fo=0/3, xT cols 0:1024), then wvT (V chunks),
                # tri (first diag mask), the rest of xT/wqkT, and the tail.
                # per-chunk pass-1 xT so the startup chains pipeline with the
                # DMA stream chunk by chunk
                nc.sync.dma_start(xs[:, 0, 0:1024], xd[:, 0, 0:1024])
                nc.sync.dma_start(qs[:, :, 0:128], qd[:, :, 0:128])
                nc.sync.dma_start(qs[:, :, 384:512], qd[:, :, 384:512])
                for i in range(1, EC):
                    nc.sync.dma_start(xs[:, i, 0:1024], xd[:, i, 0:1024])
                nc.sync.dma_start(
                    wvT_sb[:].rearrange("p (e f) -> p e f", e=EC),
                    wvT.ap().rearrange("(e p) f -> p e f", p=128))
                nc.sync.dma_start(tri_sb[:], tri.ap())
                nc.sync.dma_start(xs[:, :, 1024:S], xd[:, :, 1024:S])
                nc.sync.dma_start(qs[:, :, 128:384], qd[:, :, 128:384])
                nc.sync.dma_start(qs[:, :, 512:768], qd[:, :, 512:768])
                nc.sync.dma_start(id_sb[:], ident.ap())
                for i in range(3):
                    nc.sync.dma_start(woT_sb[i][:],
                                      woT.ap()[128 * i:128 * (i + 1), :])
                nc.sync.dma_start(bqk_sb[:], bqk.ap())
                nc.sync.dma_start(bv_sb[:], bv.ap())
                nc.sync.dma_start(on_sb[:], ones.ap())
                nc.sync.dma_start(bo_sb[:], bo.ap())

                def emit_qk_chain(fo, swp, windows=(0, 1)):
                    """qk-proj for f-chunk fo, s-windows 2*swp+windows.
                    Concurrent window chains in one pss slot."""
                    pair = ps_tile([128, 1024], "pss_t", 2)
                    for ecc in range(EC):
                        for swl in windows:
                            sw = 2 * swp + swl
                            nc.tensor.matmul(
                                pair[:, 512 * swl:512 * (swl + 1)],
                                wqkT_sb[:, 768 * ecc + 128 * fo:
                                        768 * ecc + 128 * (fo + 1)],
                                xT_sb[:, S * ecc + 512 * sw:
                                      S * ecc + 512 * (sw + 1)],
                                start=(ecc == 0), stop=(ecc == EC - 1),
                                skip_group_check=True)
                    # per-window copy-out so the first window's consumers
                    # don't wait for the second's
                    for swl in windows:
                        dst = qkT_sb[fo][:, 1024 * swp + 512 * swl:
                                         1024 * swp + 512 * (swl + 1)]
                        src = pair[:, 512 * swl:512 * (swl + 1)]
                        if with_bias:
                            nc.vector.tensor_scalar_add(
                                dst, src, bqk_sb[:, fo:fo + 1])
                        else:
                            nc.vector.tensor_copy(dst, src)

                def emit_v_chunk(sc):
                    """V-proj for s-chunk sc (one k-chunk of V_aug)."""
                    psv = ps_tile([128, FPC], "po_t", 1)
                    for ecc in range(EC):
                        nc.tensor.matmul(
                            psv[:],
                            xT_sb[:, S * ecc + 128 * sc:
                                  S * ecc + 128 * (sc + 1)],
                            wvT_sb[:, FPC * ecc:FPC * (ecc + 1)],
                            start=(ecc == 0),
                            stop=(not with_bias and ecc == EC - 1),
                            skip_group_check=True)
                    if with_bias:
                        nc.tensor.matmul(psv[:], on_sb[:, 0:128],
                                         bv_sb[:], start=False, stop=True,
                                         skip_group_check=True)
                    vv = V_sb[sc][:].rearrange("p (h x) -> p h x", x=65)
                    nc.vector.tensor_copy(
                        vv[:, :, 0:64],
                        psv[:].rearrange("p (h x) -> p h x", x=64))
                    nc.gpsimd.memset(vv[:, :, 64:65], 1.0)

                def emit_scores(hp, qw, u):
                    """Scores S^T[k, q] for a pair of k-chunks, both heads,
                    + exp, + DVE causal masks on diag blocks. Returns the
                    bf16 exp'd tile Et [128, 2048]
                    (cols 1024*hd + 512*half + qlocal)."""
                    qT, kT = qkT_sb[hp], qkT_sb[3 + hp]
                    pss = {hd: ps_tile([128, 1024], "pss_t", 2)
                           for hd in range(2)}
                    Et = EP.tile([128, 2048], BF16, name="E_t")
                    NOSHRINK = bool(int(os.environ.get("K_NOSHRINK", "0")))
                    for half in range(2):
                        ki = 2 * u + half
                        j = ki - 4 * qw
                        c = 128 * j if j > 0 and not NOSHRINK else 0
                        # strict row-group alternation (base 0,64,0,64) so the
                        # K=64 score matmul pairs run concurrently on the PE;
                        # diag tiles shrink the moving window to skip
                        # fully-masked columns
                        for hd in range(2):
                            base = 64 * hd
                            nc.tensor.matmul(
                                pss[hd][:, 512 * half + c:512 * (half + 1)],
                                kT[base:base + 64, 128 * ki:128 * (ki + 1)],
                                qT[base:base + 64,
                                   512 * qw + c:512 * (qw + 1)],
                                start=True, stop=True,
                                skip_group_check=True)
                    j0 = 2 * u - 4 * qw
                    j1 = j0 + 1
                    c0 = 128 * j0 if j0 > 0 else 0
                    # one exp per head spans both halves when contiguous;
                    # when the half-1 diag shrink leaves an unwritten PSUM
                    # gap, split the exp around it (reading the gap would
                    # race with the slot's previous occupant)
                    for hd in range(2):
                        if j1 > 0 and not NOSHRINK:
                            nc.scalar.activation(
                                Et[:, 1024 * hd + c0:1024 * hd + 512],
                                pss[hd][:, c0:512], EXP, scale=0.125)
                            c1 = 128 * j1
                            nc.scalar.activation(
                                Et[:, 1024 * hd + 512 + c1:1024 * (hd + 1)],
                                pss[hd][:, 512 + c1:1024], EXP, scale=0.125)
                        else:
                            nc.scalar.activation(
                                Et[:, 1024 * hd + c0:1024 * (hd + 1)],
                                pss[hd][:, c0:1024], EXP, scale=0.125)
                    # causal mask inside the diagonal 128x128 blocks:
                    # multiply by 0/1 upper-triangle (k<=q keeps)
                    for half in range(2):
                        j = 2 * u + half - 4 * qw
                        if j >= 0:
                            for hd in range(2):
                                off = 1024 * hd + 512 * half + 128 * j
                                nc.vector.tensor_mul(
                                    Et[:, off:off + 128],
                                    Et[:, off:off + 128], tri_sb[:])
                    return Et

                def emit_ctx_qc(hp, qw, qc, Ets, psc):
                    """ctx for one q-chunk, both heads: psc[hd][q, 65*qc+d]
                    = sum_ki E^T chunk (stationary) x V_aug chunk (moving).
                    qc-contiguous so each PSUM bank has exactly one open
                    accumulation group at a time; fully-masked (ki, qc)
                    blocks are skipped."""
                    for ki in range(0, 4 * qw + qc + 1):
                        u, half = divmod(ki, 2)
                        Et = Ets[u]
                        for hd in range(2):
                            h = 2 * hp + hd
                            nc.tensor.matmul(
                                psc[hd][:, 65 * qc:65 * qc + 65],
                                Et[:, 1024 * hd + 512 * half + 128 * qc:
                                    1024 * hd + 512 * half + 128 * qc + 128],
                                V_sb[ki][:, 65 * h:65 * h + 65],
                                start=(ki == 0), stop=(ki == 4 * qw + qc),
                                skip_group_check=True)

                def emit_norm(hp, qw, psc):
                    """Per-partition softmax normalization (q is on
                    partitions), then PE-transpose ctx back to [d, q]. The
                    raw ctx is copied out of PSUM first so the psc slots
                    free fast (the next step's first ctx write reuses them);
                    the rest of the norm runs off the critical path."""
                    craws = []
                    for hd in range(2):
                        craw = NP.tile([128, 260], F32, name="craw_t")
                        nc.vector.tensor_copy(craw[:], psc[hd][:])
                        craws.append(craw)
                    # pt shares the out-proj pb tag (both are allocated
                    # right before their writers, keeping the slot ring in
                    # emission order)
                    pt = ps_tile([128, 512], "pb_t", 1, dtype=BF16)
                    ctxns = []
                    for hd in range(2):
                        craw = craws[hd]
                        cv = craw[:].rearrange("p (q x) -> p q x", x=65)
                        rinv = NP.tile([128, 4], F32, name="rinv_t")
                        nc.vector.reciprocal(
                            rinv[:].rearrange("p (q x) -> p q x", x=1),
                            cv[:, :, 64:65])
                        ctxn = NP.tile([128, 256], BF16, name="ctxn_t")
                        ctxns.append((craw, rinv, ctxn))
                    # qc-outer, with per-qc ctxT copy-out: each out-proj
                    # s-chunk only waits its own 128-column block
                    for qc in range(4):
                        for hd in range(2):
                            craw, rinv, ctxn = ctxns[hd]
                            nc.vector.tensor_scalar_mul(
                                ctxn[:, 64 * qc:64 * (qc + 1)],
                                craw[:, 65 * qc:65 * qc + 64],
                                rinv[:, qc:qc + 1])
                            nc.tensor.transpose(
                                pt[64 * hd:64 * (hd + 1),
                                   128 * qc:128 * (qc + 1)],
                                ctxn[:, 64 * qc:64 * (qc + 1)], id_sb[:])
                        nc.vector.tensor_copy(
                            ctxT_sb[hp][:, 512 * qw + 128 * qc:
                                         512 * qw + 128 * (qc + 1)],
                            pt[:, 128 * qc:128 * (qc + 1)])

                def emit_outproj_sc(sc):
                    osb = OP.tile([128, E], F32, name="osb_t")
                    # two f-window chains on two PSUM slots, c-outer so the
                    # ctxT stationary is loaded once per c; the two slots'
                    # copy-out rotations hide each other's latency
                    pos = {0: ps_tile([128, 512], "po_t", 1),
                           512: ps_tile([128, 256], "pb_t", 1)}
                    for c in range(3):
                        for f0, fn in ((0, 512), (512, 256)):
                            nc.tensor.matmul(
                                pos[f0][:, 0:fn],
                                ctxT_sb[c][:, 128 * sc:128 * (sc + 1)],
                                woT_sb[c][:, f0:f0 + fn],
                                start=(c == 0),
                                stop=(not with_bias and c == 2),
                                skip_group_check=True)
                    for f0, fn in ((0, 512), (512, 256)):
                        if with_bias:
                            nc.tensor.matmul(pos[f0][:, 0:fn],
                                             on_sb[:, 0:128],
                                             bo_sb[:, f0:f0 + fn],
                                             start=False, stop=True,
                                             skip_group_check=True)
                        nc.vector.tensor_copy(osb[:, f0:f0 + fn],
                                              pos[f0][:, 0:fn])
                    nc.sync.dma_start(y.ap()[128 * sc:128 * (sc + 1), :],
                                      osb[:])

                def emit_attention():
                    # software pipeline: ctx runs as per-q-chunk tasks (each
                    # a full contiguous PSUM accumulation group) queued when
                    # a step's scores complete; one task is popped per unit
                    # so ctx/norm/out-proj spread between later units while
                    # ACT chews on exps.
                    work = []      # deferred ctx/norm/outproj thunks

                    def flush_one():
                        for _ in range(2):
                            if work:
                                work.pop(0)()

                    def finish_step(hp, qw, Ets):
                        # psc is allocated lazily at the first ctx task so
                        # the PSUM slot ring advances in emission order
                        holder = {}

                        def get_psc():
                            if not holder:
                                holder[0] = {
                                    hd: ps_tile([128, 260], "psc_t", 2)
                                    for hd in range(2)}
                            return holder[0]

                        for qc in range(4):
                            work.append(lambda qc=qc: emit_ctx_qc(
                                hp, qw, qc, Ets, get_psc()))
                        work.append(lambda: emit_norm(hp, qw, get_psc()))
                        if hp == 2:
                            for sc in range(4 * qw, 4 * qw + 4):
                                work.append(
                                    lambda sc=sc: emit_outproj_sc(sc))

                    # phase-1 chains interleaved between attention units:
                    # (hp, qw, u) -> thunks emitted right after that unit's
                    # scores+flush (so the next exp is never delayed by a
                    # projection chain). Deadlines: qkT window-pair swp of
                    # f-chunks (hp)/(3+hp) is read by (hp, qw>=2*swp) scores;
                    # V[k] is read by the ctx of unit u=k//2, which flushes
                    # DEPTH units later. qk chains (2.6us) avoid the last
                    # unit of a step; V chains (1us) fit anywhere.
                    intra = {}

                    def add(hp, qw, u, fn):
                        intra.setdefault((hp, qw, u), []).append(fn)

                    def addv(hp, qw, u, sc):
                        add(hp, qw, u, lambda: emit_v_chunk(sc))

                    def addqk(hp, qw, u, fo, swp):
                        add(hp, qw, u, lambda: emit_qk_chain(fo, swp))

                    addv(0, 0, 0, 0)
                    addv(0, 0, 1, 1)
                    addv(0, 1, 0, 2)
                    addv(0, 1, 1, 3)
                    addqk(0, 1, 2, 0, 1)
                    addqk(0, 1, 3, 3, 1)
                    addv(0, 2, 0, 4)
                    addv(0, 2, 0, 5)
                    addv(0, 2, 1, 6)
                    addv(0, 2, 1, 7)
                    addv(0, 2, 2, 8)
                    addv(0, 2, 3, 9)
                    addv(0, 2, 4, 10)
                    addv(0, 2, 5, 11)
                    for i, sc in enumerate(range(12, 16)):
                        addv(0, 3, i, sc)
                    addqk(0, 3, 4, 1, 0)
                    addqk(0, 3, 5, 4, 0)
                    addqk(0, 3, 6, 1, 1)
                    addqk(1, 1, 0, 4, 1)
                    addqk(1, 2, 0, 2, 0)
                    addqk(1, 2, 2, 5, 0)
                    addqk(1, 3, 0, 2, 1)
                    addqk(1, 3, 2, 5, 1)

                    for hp in range(3):
                        for qw in range(QW):
                            nu = 2 * qw + 2
                            Ets = []
                            for u in range(nu):
                                Ets.append(emit_scores(hp, qw, u))
                                for fn in intra.get((hp, qw, u), ()):
                                    fn()
                                flush_one()
                            finish_step(hp, qw, Ets)
                    while work:
                        flush_one()

                # start-up: only what the first scores need, window at a
                # time (the (0,0) step reads just s-window 0 of fo 0/3)
                emit_qk_chain(0, 0, windows=(0,))
                emit_qk_chain(3, 0, windows=(0,))
                emit_qk_chain(0, 0, windows=(1,))
                emit_qk_chain(3, 0, windows=(1,))
                emit_attention()

    return _patch_multiwait(nc)


_NC = {}


def _get_nc(with_bias=True):
    if with_bias not in _NC:
        _NC[with_bias] = build_nc(with_bias=with_bias)
    return _NC[with_bias]


def _prep_core_inputs(x, in_proj_w, in_proj_b, out_w, out_b):
    """Build the 8 per-core input dicts (host-side shard + transpose)."""
    import ml_dtypes
    bf16 = ml_dtypes.bfloat16
    # 0/1 keep-mask for S^T[k, q] diagonal blocks: keep where k <= q
    tri_np = (np.arange(128)[:, None] <= np.arange(128)[None, :])
    tri_bf = tri_np.astype(bf16)
    id_bf = np.eye(128, dtype=np.float32).astype(bf16)
    ones_np = np.ones((1, 128), np.float32).astype(bf16)

    xT_by_b = [np.asarray(x[b]).T.astype(bf16) for b in range(B)]

    in_maps = []
    for c in range(8):
        b = c // 2
        g = c % 2
        f0 = FPC * g
        Wq = np.asarray(in_proj_w[f0:f0 + FPC])
        Wk = np.asarray(in_proj_w[E + f0:E + f0 + FPC])
        Wv = np.asarray(in_proj_w[2 * E + f0:2 * E + f0 + FPC])
        bq = np.asarray(in_proj_b[f0:f0 + FPC])
        bk = np.asarray(in_proj_b[E + f0:E + f0 + FPC])
        bvv = np.asarray(in_proj_b[2 * E + f0:2 * E + f0 + FPC])
        Wo = np.asarray(out_w[:, f0:f0 + FPC])
        bqk_np = np.concatenate([bq, bk]).astype(np.float32).reshape(6, 128).T
        in_maps.append({
            "xT": xT_by_b[b],
            "wqkT": np.ascontiguousarray(
                np.concatenate([Wq, Wk], axis=0).T).astype(bf16),
            "wvT": np.ascontiguousarray(Wv.T).astype(bf16),
            "woT": np.ascontiguousarray(Wo.T).astype(bf16),
            "bqk": np.ascontiguousarray(bqk_np),
            "bv": bvv.reshape(1, FPC).astype(bf16),
            # out bias only on even cores so the host-side pair-sum is exact
            "bo": np.asarray(out_b).reshape(1, E).astype(bf16) if g == 0
                  else np.zeros((1, E), bf16),
            "tri": tri_bf,
            "ident": id_bf,
            "ones": ones_np,
        })
    return in_maps


def kernel(x, in_proj_w, in_proj_b, out_w, out_b):
    zero_bias = (not np.any(np.asarray(in_proj_b))) and \
                (not np.any(np.asarray(out_b)))
    nc = _get_nc(with_bias=not zero_bias)
    in_maps = _prep_core_inputs(x, in_proj_w, in_proj_b, out_w, out_b)
    res = run_bass_kernel_spmd(nc, in_maps, core_ids=list(range(8)))
    out = np.empty((B, S, E), np.float32)
    for b in range(B):
        out[b] = res.results[2 * b]["y"] + res.results[2 * b + 1]["y"]
    return out


# revision 52
# speedup vs baseline: 1.1691x; 1.0013x over previous
"""Multi-head attention (B=4, S=2048, E=768, H=12, D=64, causal) on 8 trn2
NeuronCores.

Sharding: core c -> batch b = c//2, head-half g = c%2 (6 heads each).
Each core computes its 6 heads' attention plus the partial output
projection; the host sums the two half-head partials per batch.

On-device strategy (per core):
  - Host pre-transposes x[b] and the weight slices so every matmul
    contraction dim (e / d / k / e_h) lands on SBUF partitions; no
    on-device transposes of inputs. Everything ships bf16 (halves DMA).
  - QK projection emits qk^T [f, s]; V projection emits V [s, f] with a
    ones column packed per head (V_aug) so the PE computes softmax
    row-sums for free.
  - Scores are computed TRANSPOSED (S^T[k, q] = K^T Q) in bf16; the
    diagonal tiles shrink their moving window to skip fully-masked
    columns. Causal masking inside the diagonal 128x128 block is a DVE
    multiply of the exp'd tile by a 0/1 lower-triangle (no PE matmul).
  - ctx is computed with E^T chunks STATIONARY and V_aug [k, 65] MOVING:
    65-cycle matmuls, and fully-masked (q-chunk, k-chunk) blocks are
    skipped entirely. Each 65-col psc region is accumulated as ONE
    contiguous PSUM group (qc-outer over all k-chunks of the step) --
    interleaving several open accumulation groups inside one PSUM bank
    corrupts the early-stopping groups. The result lands [q, d+1] with
    q on partitions, so softmax normalization is a per-partition
    reciprocal + scalar multiply on DVE (no PE broadcast matmul), and a
    PE transpose brings ctx^T [d, q] back for the output projection.
  - All big matmuls run in bf16 at 1 cycle/row.
  - The attention phase is ACT(exp)-throughput-bound, so scores/exps
    stream per unit while ctx/norm/out-proj run as deferred tasks popped
    between later units, and the projection chains of phase 1 are
    interleaved into the attention loop with just-in-time deadlines.
"""
import sys, json, os

for _p in ("/opt/trn_rl_repo",):
    if _p not in sys.path and os.path.isdir(_p):
        sys.path.insert(0, _p)

import numpy as np
import concourse.bass as bass
import concourse.mybir as mybir
import concourse.tile as tile
from concourse.bass_utils import run_bass_kernel_spmd

B, S, E, H, D = 4, 2048, 768, 12, 64
HPC = H // 2          # heads per core = 6
FPC = HPC * D         # features per core per q/k/v = 384
EC = E // 128         # 6 contraction chunks for projections
SC = S // 128         # 16 s-chunks
QW = S // 512         # 4 q-windows
KC = S // 128         # 16 k-chunks
F32 = mybir.dt.float32
BF16 = mybir.dt.bfloat16
EXP = mybir.ActivationFunctionType.Exp


def _patch_multiwait(nc, max_waits=1):
    """This container's walrus rejects instructions with more than one sync
    wait. Split excess waits onto same-engine NOPs emitted immediately
    before the instruction (same-engine streams are order-preserving)."""
    raw = nc.to_json_bytes()
    m = json.loads(raw)
    for f in m["functions"]:
        for b in f["blocks"]:
            out = []
            for inst in b["instructions"]:
                si = inst.get("sync_info") or {}
                ws = si.get("on_wait") or []
                if len(ws) > max_waits:
                    eng = inst["engine"]
                    for i, w in enumerate(ws[:-max_waits]):
                        out.append({
                            "debug": inst.get("debug", 0), "engine": eng,
                            "ins": [], "name": inst["name"] + f"-mw{i}",
                            "opcode": "NoOp", "outs": [],
                            "sync_info": {"on_update": [], "on_wait": [w]},
                        })
                    si["on_wait"] = ws[-max_waits:]
                out.append(inst)
            b["instructions"] = out
    patched = json.dumps(m).encode()
    nc.to_json_bytes = lambda: patched
    return nc


def build_nc(with_bias=True):
    nc = bass.Bass()
    xT = nc.dram_tensor("xT", [E, S], BF16, kind="ExternalInput")
    wqkT = nc.dram_tensor("wqkT", [E, 2 * FPC], BF16, kind="ExternalInput")
    wvT = nc.dram_tensor("wvT", [E, FPC], BF16, kind="ExternalInput")
    woT = nc.dram_tensor("woT", [FPC, E], BF16, kind="ExternalInput")
    bqk = nc.dram_tensor("bqk", [128, 2 * FPC // 128], F32, kind="ExternalInput")
    bv = nc.dram_tensor("bv", [1, FPC], BF16, kind="ExternalInput")
    bo = nc.dram_tensor("bo", [1, E], BF16, kind="ExternalInput")
    tri = nc.dram_tensor("tri", [128, 128], BF16, kind="ExternalInput")
    ident = nc.dram_tensor("ident", [128, 128], BF16, kind="ExternalInput")
    ones = nc.dram_tensor("ones", [1, 128], BF16, kind="ExternalInput")
    y = nc.dram_tensor("y", [S, E], F32, kind="ExternalOutput")

    with tile.TileContext(nc) as tc, \
         nc.allow_low_precision(reason="bf16 matmul pipeline by design"):
        with tc.tile_pool(name="persist", bufs=1) as P, \
             tc.tile_pool(name="ps", bufs=1, space="PSUM") as PS:
            # --- persistent tiles (bottom-of-stack, live whole kernel)
            qkT_sb = [P.tile([128, S], BF16, name=f"qkT{i}") for i in range(6)]
            V_sb = [P.tile([128, 65 * HPC], BF16, name=f"V{i}") for i in range(KC)]
            ctxT_sb = [P.tile([128, S], BF16, name=f"ctxT{i}") for i in range(3)]
            woT_sb = [P.tile([128, E], BF16, name=f"woT{i}") for i in range(3)]
            bqk_sb = P.tile([128, 6], F32, name="bqk_sb")
            bv_sb = P.tile([1, FPC], BF16, name="bv_sb")
            bo_sb = P.tile([1, E], BF16, name="bo_sb")
            tri_sb = P.tile([128, 128], BF16, name="tri_sb")
            id_sb = P.tile([128, 128], BF16, name="id_sb")
            on_sb = P.tile([1, 128], BF16, name="on_sb")

            def ps_tile(shape, tag, bufs, dtype=F32):
                return PS.tile(shape, dtype, name=tag, tag=tag, bufs=bufs)

            # ============ phase 1 (projections) + attention, interleaved ====
            # The attention phase is ACT(exp)-throughput-bound, so the
            # projections are software-pipelined INTO the attention loop:
            # only the chains needed for the first scores run up front, and
            # the rest are emitted between attention units where the PE has
            # slack while ACT chews on exps.
            with tc.tile_pool(name="inp", bufs=1) as PI, \
                 tc.tile_pool(name="esb", bufs=14) as EP, \
                 tc.tile_pool(name="nrm", bufs=8) as NP, \
                 tc.tile_pool(name="osb", bufs=3) as OP:
                # consolidated phase-1 tiles: one DMA dispatch covers all six
                # e-chunks (the SP sequencer costs ~650ns per DMA, so fewer,
                # bigger strided DMAs win)
                xT_sb = PI.tile([128, EC * S], BF16, name="xT_all")
                wqkT_sb = PI.tile([128, EC * 2 * FPC], BF16, name="wqkT_all")
                wvT_sb = PI.tile([128, EC * FPC], BF16, name="wvT_all")
                xs = xT_sb[:].rearrange("p (e s) -> p e s", e=EC)
                xd = xT.ap().rearrange("(e p) s -> p e s", p=128)
                qs = wqkT_sb[:].rearrange("p (e f) -> p e f", e=EC)
                qd = wqkT.ap().rearrange("(e p) f -> p e f", p=128)
                # DMA order: first the tensors gating the two startup chains
                # (wqkT cols of fo=0/3, xT cols 0:1024), then wvT (V chunks),
                # tri (first diag mask), the rest of xT/wqkT, and the tail.
                # per-chunk pass-1 xT so the startup chains pipeline with the
                # DMA stream chunk by chunk
                nc.sync.dma_start(xs[:, 0, 0:1024], xd[:, 0, 0:1024])
                nc.sync.dma_start(qs[:, :, 0:128], qd[:, :, 0:128])
                nc.sync.dma_start(qs[:, :, 384:512], qd[:, :, 384:512])
                for i in range(1, EC):
                    nc.sync.dma_start(xs[:, i, 0:1024], xd[:, i, 0:1024])
                nc.sync.dma_start(
                    wvT_sb[:].rearrange("p (e f) -> p e f", e=EC),
                    wvT.ap().rearrange("(e p) f -> p e f", p=128))
                nc.sync.dma_start(tri_sb[:], tri.ap())
                nc.sync.dma_start(xs[:, :, 1024:S], xd[:, :, 1024:S])
                nc.sync.dma_start(qs[:, :, 128:384], qd[:, :, 128:384])
                nc.sync.dma_start(qs[:, :, 512:768], qd[:, :, 512:768])
                nc.sync.dma_start(id_sb[:], ident.ap())
                for i in range(3):
                    nc.sync.dma_start(woT_sb[i][:],
                                      woT.ap()[128 * i:128 * (i + 1), :])
                nc.sync.dma_start(bqk_sb[:], bqk.ap())
                nc.sync.dma_start(bv_sb[:], bv.ap())
                nc.sync.dma_start(on_sb[:], ones.ap())
                nc.sync.dma_start(bo_sb[:], bo.ap())

                def emit_qk_chain(fo, swp, windows=(0, 1)):
                    """qk-proj for f-chunk fo, s-windows 2*swp+windows.
                    Concurrent window chains in one pss slot."""
                    pair = ps_tile([128, 1024], "pss_t", 2)
                    for ecc in range(EC):
                        for swl in windows:
                            sw = 2 * swp + swl
                            nc.tensor.matmul(
                                pair[:, 512 * swl:512 * (swl + 1)],
                                wqkT_sb[:, 768 * ecc + 128 * fo:
                                        768 * ecc + 128 * (fo + 1)],
                                xT_sb[:, S * ecc + 512 * sw:
                                      S * ecc + 512 * (sw + 1)],
                                start=(ecc == 0), stop=(ecc == EC - 1),
                                skip_group_check=True)
                    # per-window copy-out so the first window's consumers
                    # don't wait for the second's
                    for swl in windows:
                        dst = qkT_sb[fo][:, 1024 * swp + 512 * swl:
                                         1024 * swp + 512 * (swl + 1)]
                        src = pair[:, 512 * swl:512 * (swl + 1)]
                        if with_bias:
                            nc.vector.tensor_scalar_add(
                                dst, src, bqk_sb[:, fo:fo + 1])
                        else:
                            nc.vector.tensor_copy(dst, src)

                def emit_v_chunk(sc):
                    """V-proj for s-chunk sc (one k-chunk of V_aug)."""
                    psv = ps_tile([128, FPC], "po_t", 1)
                    for ecc in range(EC):
                        nc.tensor.matmul(
                            psv[:],
                            xT_sb[:, S * ecc + 128 * sc:
                                  S * ecc + 128 * (sc + 1)],
                            wvT_sb[:, FPC * ecc:FPC * (ecc + 1)],
                            start=(ecc == 0),
                            stop=(not with_bias and ecc == EC - 1),
                            skip_group_check=True)
                    if with_bias:
                        nc.tensor.matmul(psv[:], on_sb[:, 0:128],
                                         bv_sb[:], start=False, stop=True,
                                         skip_group_check=True)
                    vv = V_sb[sc][:].rearrange("p (h x) -> p h x", x=65)
                    nc.vector.tensor_copy(
                        vv[:, :, 0:64],
                        psv[:].rearrange("p (h x) -> p h x", x=64))
                    nc.gpsimd.memset(vv[:, :, 64:65], 1.0)

                def emit_scores(hp, qw, u):
                    """Scores S^T[k, q] for a pair of k-chunks, both heads,
                    + exp, + DVE causal masks on diag blocks. Returns the
                    bf16 exp'd tile Et [128, 2048]
                    (cols 1024*hd + 512*half + qlocal)."""
                    qT, kT = qkT_sb[hp], qkT_sb[3 + hp]
                    pss = {hd: ps_tile([128, 1024], "pss_t", 2)
                           for hd in range(2)}
                    Et = EP.tile([128, 2048], BF16, name="E_t")
                    NOSHRINK = bool(int(os.environ.get("K_NOSHRINK", "0")))
                    for half in range(2):
                        ki = 2 * u + half
                        j = ki - 4 * qw
                        c = 128 * j if j > 0 and not NOSHRINK else 0
                        # strict row-group alternation (base 0,64,0,64) so the
                        # K=64 score matmul pairs run concurrently on the PE;
                        # diag tiles shrink the moving window to skip
                        # fully-masked columns
                        for hd in range(2):
                            base = 64 * hd
                            nc.tensor.matmul(
                                pss[hd][:, 512 * half + c:512 * (half + 1)],
                                kT[base:base + 64, 128 * ki:128 * (ki + 1)],
                                qT[base:base + 64,
                                   512 * qw + c:512 * (qw + 1)],
                                start=True, stop=True,
                                skip_group_check=True)
                    j0 = 2 * u - 4 * qw
                    j1 = j0 + 1
                    c0 = 128 * j0 if j0 > 0 else 0
                    # one exp per head spans both halves when contiguous;
                    # when the half-1 diag shrink leaves an unwritten PSUM
                    # gap, split the exp around it (reading the gap would
                    # race with the slot's previous occupant)
                    for hd in range(2):
                        if j1 > 0 and not NOSHRINK:
                            nc.scalar.activation(
                                Et[:, 1024 * hd + c0:1024 * hd + 512],
                                pss[hd][:, c0:512], EXP, scale=0.125)
                            c1 = 128 * j1
                            nc.scalar.activation(
                                Et[:, 1024 * hd + 512 + c1:1024 * (hd + 1)],
                                pss[hd][:, 512 + c1:1024], EXP, scale=0.125)
                        else:
                            nc.scalar.activation(
                                Et[:, 1024 * hd + c0:1024 * (hd + 1)],
                                pss[hd][:, c0:1024], EXP, scale=0.125)
                    # causal mask inside the diagonal 128x128 blocks:
                    # multiply by 0/1 upper-triangle (k<=q keeps)
                    for half in range(2):
                        j = 2 * u + half - 4 * qw
                        if j >= 0:
                            for hd in range(2):
                                off = 1024 * hd + 512 * half + 128 * j
                                nc.vector.tensor_mul(
                                    Et[:, off:off + 128],
                                    Et[:, off:off + 128], tri_sb[:])
                    return Et

                def emit_ctx_qc(hp, qw, qc, Ets, psc):
                    """ctx for one q-chunk, both heads: psc[hd][q, 65*qc+d]
                    = sum_ki E^T chunk (stationary) x V_aug chunk (moving).
                    qc-contiguous so each PSUM bank has exactly one open
                    accumulation group at a time; fully-masked (ki, qc)
                    blocks are skipped."""
                    for ki in range(0, 4 * qw + qc + 1):
                        u, half = divmod(ki, 2)
                        Et = Ets[u]
                        for hd in range(2):
                            h = 2 * hp + hd
                            nc.tensor.matmul(
                                psc[hd][:, 65 * qc:65 * qc + 65],
                                Et[:, 1024 * hd + 512 * half + 128 * qc:
                                    1024 * hd + 512 * half + 128 * qc + 128],
                                V_sb[ki][:, 65 * h:65 * h + 65],
                                start=(ki == 0), stop=(ki == 4 * qw + qc),
                                skip_group_check=True)

                def emit_norm(hp, qw, psc):
                    """Per-partition softmax normalization (q is on
                    partitions), then PE-transpose ctx back to [d, q]. The
                    raw ctx is copied out of PSUM first so the psc slots
                    free fast (the next step's first ctx write reuses them);
                    the rest of the norm runs off the critical path."""
                    craws = []
                    for hd in range(2):
                        craw = NP.tile([128, 260], F32, name="craw_t")
                        nc.vector.tensor_copy(craw[:], psc[hd][:])
                        craws.append(craw)
                    # pt shares the out-proj pb tag (both are allocated
                    # right before their writers, keeping the slot ring in
                    # emission order)
                    pt = ps_tile([128, 512], "pb_t", 1, dtype=BF16)
                    ctxns = []
                    for hd in range(2):
                        craw = craws[hd]
                        cv = craw[:].rearrange("p (q x) -> p q x", x=65)
                        rinv = NP.tile([128, 4], F32, name="rinv_t")
                        nc.vector.reciprocal(
                            rinv[:].rearrange("p (q x) -> p q x", x=1),
                            cv[:, :, 64:65])
                        ctxn = NP.tile([128, 256], BF16, name="ctxn_t")
                        ctxns.append((craw, rinv, ctxn))
                    # qc-outer, with per-qc ctxT copy-out: each out-proj
                    # s-chunk only waits its own 128-column block
                    for qc in range(4):
                        for hd in range(2):
                            craw, rinv, ctxn = ctxns[hd]
                            nc.vector.tensor_scalar_mul(
                                ctxn[:, 64 * qc:64 * (qc + 1)],
                                craw[:, 65 * qc:65 * qc + 64],
                                rinv[:, qc:qc + 1])
                            nc.tensor.transpose(
                                pt[64 * hd:64 * (hd + 1),
                                   128 * qc:128 * (qc + 1)],
                                ctxn[:, 64 * qc:64 * (qc + 1)], id_sb[:])
                        nc.vector.tensor_copy(
                            ctxT_sb[hp][:, 512 * qw + 128 * qc:
                                         512 * qw + 128 * (qc + 1)],
                            pt[:, 128 * qc:128 * (qc + 1)])

                def emit_outproj_sc(sc):
                    osb = OP.tile([128, E], F32, name="osb_t")
                    # two f-window chains on two PSUM slots, c-outer so the
                    # ctxT stationary is loaded once per c; the two slots'
                    # copy-out rotations hide each other's latency
                    pos = {0: ps_tile([128, 512], "po_t", 1),
                           512: ps_tile([128, 256], "pb_t", 1)}
                    for c in range(3):
                        for f0, fn in ((0, 512), (512, 256)):
                            nc.tensor.matmul(
                                pos[f0][:, 0:fn],
                                ctxT_sb[c][:, 128 * sc:128 * (sc + 1)],
                                woT_sb[c][:, f0:f0 + fn],
                                start=(c == 0),
                                stop=(not with_bias and c == 2),
                                skip_group_check=True)
                    for f0, fn in ((0, 512), (512, 256)):
                        if with_bias:
                            nc.tensor.matmul(pos[f0][:, 0:fn],
                                             on_sb[:, 0:128],
                                             bo_sb[:, f0:f0 + fn],
                                             start=False, stop=True,
                                             skip_group_check=True)
                        nc.vector.tensor_copy(osb[:, f0:f0 + fn],
                                              pos[f0][:, 0:fn])
                    nc.sync.dma_start(y.ap()[128 * sc:128 * (sc + 1), :],
                                      osb[:])

                def emit_attention():
                    # software pipeline: ctx runs as per-q-chunk tasks (each
                    # a full contiguous PSUM accumulation group) queued when
                    # a step's scores complete; one task is popped per unit
                    # so ctx/norm/out-proj spread between later units while
                    # ACT chews on exps.
                    work = []      # deferred ctx/norm/outproj thunks

                    def flush_one():
                        if work:
                            work.pop(0)()
                        if len(work) > 2:   # backlog guard near the end
                            work.pop(0)()

                    def finish_step(hp, qw, Ets):
                        # psc is allocated lazily at the first ctx task so
                        # the PSUM slot ring advances in emission order
                        holder = {}

                        def get_psc():
                            if not holder:
                                holder[0] = {
                                    hd: ps_tile([128, 260], "psc_t", 2)
                                    for hd in range(2)}
                            return holder[0]

                        for qc in range(4):
                            work.append(lambda qc=qc: emit_ctx_qc(
                                hp, qw, qc, Ets, get_psc()))
                        work.append(lambda: emit_norm(hp, qw, get_psc()))
                        if hp == 2:
                            for sc in range(4 * qw, 4 * qw + 4):
                                work.append(
                                    lambda sc=sc: emit_outproj_sc(sc))

                    # phase-1 chains interleaved between attention units:
                    # (hp, qw, u) -> thunks emitted right after that unit's
                    # scores+flush (so the next exp is never delayed by a
                    # projection chain). Deadlines: qkT window-pair swp of
                    # f-chunks (hp)/(3+hp) is read by (hp, qw>=2*swp) scores;
                    # V[k] is read by the ctx of unit u=k//2, which flushes
                    # DEPTH units later. qk chains (2.6us) avoid the last
                    # unit of a step; V chains (1us) fit anywhere.
                    intra = {}

                    def add(hp, qw, u, fn):
                        intra.setdefault((hp, qw, u), []).append(fn)

                    def addv(hp, qw, u, sc):
                        add(hp, qw, u, lambda: emit_v_chunk(sc))

                    def addqk(hp, qw, u, fo, swp):
                        add(hp, qw, u, lambda: emit_qk_chain(fo, swp))

                    addv(0, 0, 0, 0)
                    addv(0, 0, 1, 1)
                    addv(0, 1, 0, 2)
                    addv(0, 1, 1, 3)
                    addqk(0, 1, 2, 0, 1)
                    addqk(0, 1, 3, 3, 1)
                    addv(0, 2, 0, 4)
                    addv(0, 2, 0, 5)
                    addv(0, 2, 1, 6)
                    addv(0, 2, 1, 7)
                    addv(0, 2, 2, 8)
                    addv(0, 2, 3, 9)
                    addv(0, 2, 4, 10)
                    addv(0, 2, 5, 11)
                    for i, sc in enumerate(range(12, 16)):
                        addv(0, 3, i, sc)
                    addqk(0, 3, 4, 1, 0)
                    addqk(0, 3, 5, 4, 0)
                    addqk(0, 3, 6, 1, 1)
                    addqk(1, 1, 0, 4, 1)
                    addqk(1, 2, 0, 2, 0)
                    addqk(1, 2, 2, 5, 0)
                    addqk(1, 3, 0, 2, 1)
                    addqk(1, 3, 2, 5, 1)

                    for hp in range(3):
                        for qw in range(QW):
                            nu = 2 * qw + 2
                            Ets = []
                            for u in range(nu):
                                Ets.append(emit_scores(hp, qw, u))
                                for fn in intra.get((hp, qw, u), ()):
                                    fn()
                                flush_one()
                            finish_step(hp, qw, Ets)
                    while work:
                        flush_one()

                # start-up: only what the first scores need, window at a
                # time (the (0,0) step reads just s-window 0 of fo 0/3)
                emit_qk_chain(0, 0, windows=(0,))
                emit_qk_chain(3, 0, windows=(0,))
                emit_qk_chain(0, 0, windows=(1,))
                emit_qk_chain(3, 0, windows=(1,))
                emit_attention()

    return _patch_multiwait(nc)


_NC = {}


def _get_nc(with_bias=True):
    if with_bias not in _NC:
        _NC[with_bias] = build_nc(with_bias=with_bias)
    return _NC[with_bias]


def _prep_core_inputs(x, in_proj_w, in_proj_b, out_w, out_b):
    """Build the 8 per-core input dicts (host-side shard + transpose)."""
    import ml_dtypes
    bf16 = ml_dtypes.bfloat16
    # 0/1 keep-mask for S^T[k, q] diagonal blocks: keep where k <= q
    tri_np = (np.arange(128)[:, None] <= np.arange(128)[None, :])
    tri_bf = tri_np.astype(bf16)
    id_bf = np.eye(128, dtype=np.float32).astype(bf16)
    ones_np = np.ones((1, 128), np.float32).astype(bf16)

    xT_by_b = [np.asarray(x[b]).T.astype(bf16) for b in range(B)]

    in_maps = []
    for c in range(8):
        b = c // 2
        g = c % 2
        f0 = FPC * g
        Wq = np.asarray(in_proj_w[f0:f0 + FPC])
        Wk = np.asarray(in_proj_w[E + f0:E + f0 + FPC])
        Wv = np.asarray(in_proj_w[2 * E + f0:2 * E + f0 + FPC])
        bq = np.asarray(in_proj_b[f0:f0 + FPC])
        bk = np.asarray(in_proj_b[E + f0:E + f0 + FPC])
        bvv = np.asarray(in_proj_b[2 * E + f0:2 * E + f0 + FPC])
        Wo = np.asarray(out_w[:, f0:f0 + FPC])
        bqk_np = np.concatenate([bq, bk]).astype(np.float32).reshape(6, 128).T
        in_maps.append({
            "xT": xT_by_b[b],
            "wqkT": np.ascontiguousarray(
                np.concatenate([Wq, Wk], axis=0).T).astype(bf16),
            "wvT": np.ascontiguousarray(Wv.T).astype(bf16),
            "woT": np.ascontiguousarray(Wo.T).astype(bf16),
            "bqk": np.ascontiguousarray(bqk_np),
            "bv": bvv.reshape(1, FPC).astype(bf16),
            # out bias only on even cores so the host-side pair-sum is exact
            "bo": np.asarray(out_b).reshape(1, E).astype(bf16) if g == 0
                  else np.zeros((1, E), bf16),
            "tri": tri_bf,
            "ident": id_bf,
            "ones": ones_np,
        })
    return in_maps


def kernel(x, in_proj_w, in_proj_b, out_w, out_b):
    zero_bias = (not np.any(np.asarray(in_proj_b))) and \
                (not np.any(np.asarray(out_b)))
    nc = _get_nc(with_bias=not zero_bias)
    in_maps = _prep_core_inputs(x, in_proj_w, in_proj_b, out_w, out_b)
    res = run_bass_kernel_spmd(nc, in_maps, core_ids=list(range(8)))
    out = np.empty((B, S, E), np.float32)
    for b in range(B):
        out[b] = res.results[2 * b]["y"] + res.results[2 * b + 1]["y"]
    return out


# revision 59
# speedup vs baseline: 1.2421x; 1.0624x over previous
"""Multi-head attention (B=4, S=2048, E=768, H=12, D=64, causal) on 8 trn2
NeuronCores.

Sharding: core c -> batch b = c//2, head-half g = c%2 (6 heads each).
Each core computes its 6 heads' attention plus the partial output
projection; the host sums the two half-head partials per batch.

On-device strategy (per core):
  - Host pre-transposes x[b] and the weight slices so every matmul
    contraction dim (e / d / k / e_h) lands on SBUF partitions; no
    on-device transposes of inputs. Everything ships bf16 (halves DMA).
  - QK projection emits qk^T [f, s]; V projection emits V [s, f] with a
    ones column packed per head (V_aug) so the PE computes softmax
    row-sums for free.
  - Scores are computed TRANSPOSED (S^T[k, q] = K^T Q) in bf16; the
    diagonal tiles shrink their moving window to skip fully-masked
    columns. Causal masking inside the diagonal 128x128 block is a DVE
    multiply of the exp'd tile by a 0/1 lower-triangle (no PE matmul).
  - ctx is computed with E^T chunks STATIONARY and V_aug [k, 65] MOVING:
    65-cycle matmuls, and fully-masked (q-chunk, k-chunk) blocks are
    skipped entirely. Each 65-col psc region is accumulated as ONE
    contiguous PSUM group (qc-outer over all k-chunks of the step) --
    interleaving several open accumulation groups inside one PSUM bank
    corrupts the early-stopping groups. The result lands [q, d+1] with
    q on partitions, so softmax normalization is a per-partition
    reciprocal + scalar multiply on DVE (no PE broadcast matmul), and a
    PE transpose brings ctx^T [d, q] back for the output projection.
  - All big matmuls run in bf16 at 1 cycle/row.
  - The attention phase is ACT(exp)-throughput-bound, so scores/exps
    stream per unit while ctx/norm/out-proj run as deferred tasks popped
    between later units, and the projection chains of phase 1 are
    interleaved into the attention loop with just-in-time deadlines.
"""
import sys, json, os

for _p in ("/opt/trn_rl_repo",):
    if _p not in sys.path and os.path.isdir(_p):
        sys.path.insert(0, _p)

import numpy as np
import concourse.bass as bass
import concourse.mybir as mybir
import concourse.tile as tile
from concourse.bass_utils import run_bass_kernel_spmd

B, S, E, H, D = 4, 2048, 768, 12, 64
HPC = H // 2          # heads per core = 6
FPC = HPC * D         # features per core per q/k/v = 384
EC = E // 128         # 6 contraction chunks for projections
SC = S // 128         # 16 s-chunks
QW = S // 512         # 4 q-windows
KC = S // 128         # 16 k-chunks
F32 = mybir.dt.float32
BF16 = mybir.dt.bfloat16
EXP = mybir.ActivationFunctionType.Exp


def _patch_multiwait(nc, max_waits=1):
    """This container's walrus rejects instructions with more than one sync
    wait. Split excess waits onto same-engine NOPs emitted immediately
    before the instruction (same-engine streams are order-preserving)."""
    raw = nc.to_json_bytes()
    m = json.loads(raw)
    for f in m["functions"]:
        for b in f["blocks"]:
            out = []
            for inst in b["instructions"]:
                si = inst.get("sync_info") or {}
                ws = si.get("on_wait") or []
                if len(ws) > max_waits:
                    eng = inst["engine"]
                    for i, w in enumerate(ws[:-max_waits]):
                        out.append({
                            "debug": inst.get("debug", 0), "engine": eng,
                            "ins": [], "name": inst["name"] + f"-mw{i}",
                            "opcode": "NoOp", "outs": [],
                            "sync_info": {"on_update": [], "on_wait": [w]},
                        })
                    si["on_wait"] = ws[-max_waits:]
                out.append(inst)
            b["instructions"] = out
    patched = json.dumps(m).encode()
    nc.to_json_bytes = lambda: patched
    return nc


def build_nc(with_bias=True):
    nc = bass.Bass()
    xT = nc.dram_tensor("xT", [E, S], BF16, kind="ExternalInput")
    wqkT = nc.dram_tensor("wqkT", [E, 2 * FPC], BF16, kind="ExternalInput")
    wvT = nc.dram_tensor("wvT", [E, FPC], BF16, kind="ExternalInput")
    woT = nc.dram_tensor("woT", [FPC, E], BF16, kind="ExternalInput")
    bqk = nc.dram_tensor("bqk", [128, 2 * FPC // 128], F32, kind="ExternalInput")
    bv = nc.dram_tensor("bv", [1, FPC], BF16, kind="ExternalInput")
    bo = nc.dram_tensor("bo", [1, E], BF16, kind="ExternalInput")
    tri = nc.dram_tensor("tri", [128, 128], BF16, kind="ExternalInput")
    ident = nc.dram_tensor("ident", [128, 128], BF16, kind="ExternalInput")
    ones = nc.dram_tensor("ones", [1, 128], BF16, kind="ExternalInput")
    y = nc.dram_tensor("y", [S, E], F32, kind="ExternalOutput")

    with tile.TileContext(nc) as tc, \
         nc.allow_low_precision(reason="bf16 matmul pipeline by design"):
        with tc.tile_pool(name="persist", bufs=1) as P, \
             tc.tile_pool(name="ps", bufs=1, space="PSUM") as PS:
            # --- persistent tiles (bottom-of-stack, live whole kernel)
            qkT_sb = [P.tile([128, S], BF16, name=f"qkT{i}") for i in range(6)]
            V_sb = [P.tile([128, 65 * HPC], BF16, name=f"V{i}") for i in range(KC)]
            ctxT_sb = [P.tile([128, S], BF16, name=f"ctxT{i}") for i in range(3)]
            woT_sb = [P.tile([128, E], BF16, name=f"woT{i}") for i in range(3)]
            bqk_sb = P.tile([128, 6], F32, name="bqk_sb")
            bv_sb = P.tile([1, FPC], BF16, name="bv_sb")
            bo_sb = P.tile([1, E], BF16, name="bo_sb")
            tri_sb = P.tile([128, 128], BF16, name="tri_sb")
            id_sb = P.tile([128, 128], BF16, name="id_sb")
            on_sb = P.tile([1, 128], BF16, name="on_sb")

            def ps_tile(shape, tag, bufs, dtype=F32):
                return PS.tile(shape, dtype, name=tag, tag=tag, bufs=bufs)

            # ============ phase 1 (projections) + attention, interleaved ====
            # The attention phase is ACT(exp)-throughput-bound, so the
            # projections are software-pipelined INTO the attention loop:
            # only the chains needed for the first scores run up front, and
            # the rest are emitted between attention units where the PE has
            # slack while ACT chews on exps.
            with tc.tile_pool(name="inp", bufs=1) as PI, \
                 tc.tile_pool(name="esb", bufs=14) as EP, \
                 tc.tile_pool(name="nrm", bufs=8) as NP, \
                 tc.tile_pool(name="osb", bufs=3) as OP:
                # consolidated phase-1 tiles: one DMA dispatch covers all six
                # e-chunks (the SP sequencer costs ~650ns per DMA, so fewer,
                # bigger strided DMAs win)
                xT_sb = PI.tile([128, EC * S], BF16, name="xT_all")
                wqkT_sb = PI.tile([128, EC * 2 * FPC], BF16, name="wqkT_all")
                wvT_sb = PI.tile([128, EC * FPC], BF16, name="wvT_all")
                xs = xT_sb[:].rearrange("p (e s) -> p e s", e=EC)
                xd = xT.ap().rearrange("(e p) s -> p e s", p=128)
                qs = wqkT_sb[:].rearrange("p (e f) -> p e f", e=EC)
                qd = wqkT.ap().rearrange("(e p) f -> p e f", p=128)
                # DMA order: first the tensors gating the two startup chains
                # (wqkT cols of fo=0/3, xT cols 0:1024), then wvT (V chunks),
                # tri (first diag mask), the rest of xT/wqkT, and the tail.
                # per-chunk pass-1 xT so the startup chains pipeline with the
                # DMA stream chunk by chunk
                nc.sync.dma_start(xs[:, 0, 0:1024], xd[:, 0, 0:1024])
                nc.sync.dma_start(qs[:, :, 0:128], qd[:, :, 0:128])
                nc.sync.dma_start(qs[:, :, 384:512], qd[:, :, 384:512])
                for i in range(1, EC):
                    nc.sync.dma_start(xs[:, i, 0:1024], xd[:, i, 0:1024])
                for i in range(EC):
                    nc.sync.dma_start(xs[:, i, 1024:S], xd[:, i, 1024:S])
                nc.sync.dma_start(
                    wvT_sb[:].rearrange("p (e f) -> p e f", e=EC),
                    wvT.ap().rearrange("(e p) f -> p e f", p=128))
                nc.sync.dma_start(tri_sb[:], tri.ap())
                nc.sync.dma_start(qs[:, :, 128:384], qd[:, :, 128:384])
                nc.sync.dma_start(qs[:, :, 512:768], qd[:, :, 512:768])
                nc.sync.dma_start(id_sb[:], ident.ap())
                for i in range(3):
                    nc.sync.dma_start(woT_sb[i][:],
                                      woT.ap()[128 * i:128 * (i + 1), :])
                nc.sync.dma_start(bqk_sb[:], bqk.ap())
                nc.sync.dma_start(bv_sb[:], bv.ap())
                nc.sync.dma_start(on_sb[:], ones.ap())
                nc.sync.dma_start(bo_sb[:], bo.ap())

                def emit_qk_chain(fo, swp, windows=(0, 1)):
                    """qk-proj for f-chunk fo, s-windows 2*swp+windows.
                    Concurrent window chains in one pss slot."""
                    pair = ps_tile([128, 1024], "pss_t", 3)
                    for ecc in range(EC):
                        for swl in windows:
                            sw = 2 * swp + swl
                            nc.tensor.matmul(
                                pair[:, 512 * swl:512 * (swl + 1)],
                                wqkT_sb[:, 768 * ecc + 128 * fo:
                                        768 * ecc + 128 * (fo + 1)],
                                xT_sb[:, S * ecc + 512 * sw:
                                      S * ecc + 512 * (sw + 1)],
                                start=(ecc == 0), stop=(ecc == EC - 1),
                                skip_group_check=True)
                    # per-window copy-out so the first window's consumers
                    # don't wait for the second's
                    for swl in windows:
                        dst = qkT_sb[fo][:, 1024 * swp + 512 * swl:
                                         1024 * swp + 512 * (swl + 1)]
                        src = pair[:, 512 * swl:512 * (swl + 1)]
                        if with_bias:
                            nc.vector.tensor_scalar_add(
                                dst, src, bqk_sb[:, fo:fo + 1])
                        else:
                            nc.vector.tensor_copy(dst, src)

                def emit_v_chunk(sc):
                    """V-proj for s-chunk sc (one k-chunk of V_aug)."""
                    psv = ps_tile([128, FPC], "pss_t", 3)
                    for ecc in range(EC):
                        nc.tensor.matmul(
                            psv[:],
                            xT_sb[:, S * ecc + 128 * sc:
                                  S * ecc + 128 * (sc + 1)],
                            wvT_sb[:, FPC * ecc:FPC * (ecc + 1)],
                            start=(ecc == 0),
                            stop=(not with_bias and ecc == EC - 1),
                            skip_group_check=True)
                    if with_bias:
                        nc.tensor.matmul(psv[:], on_sb[:, 0:128],
                                         bv_sb[:], start=False, stop=True,
                                         skip_group_check=True)
                    vv = V_sb[sc][:].rearrange("p (h x) -> p h x", x=65)
                    nc.vector.tensor_copy(
                        vv[:, :, 0:64],
                        psv[:].rearrange("p (h x) -> p h x", x=64))
                    nc.gpsimd.memset(vv[:, :, 64:65], 1.0)

                def emit_scores(hp, qw, u):
                    """Scores S^T[k, q] for a pair of k-chunks, both heads,
                    + exp, + DVE causal masks on diag blocks. Returns the
                    bf16 exp'd tile Et [128, 2048]
                    (cols 1024*hd + 512*half + qlocal)."""
                    qT, kT = qkT_sb[hp], qkT_sb[3 + hp]
                    pss = {hd: ps_tile([128, 1024], "pss_t", 3)
                           for hd in range(2)}
                    Et = EP.tile([128, 2048], BF16, name="E_t")
                    NOSHRINK = bool(int(os.environ.get("K_NOSHRINK", "0")))
                    for half in range(2):
                        ki = 2 * u + half
                        j = ki - 4 * qw
                        c = 128 * j if j > 0 and not NOSHRINK else 0
                        # strict row-group alternation (base 0,64,0,64) so the
                        # K=64 score matmul pairs run concurrently on the PE;
                        # diag tiles shrink the moving window to skip
                        # fully-masked columns
                        for hd in range(2):
                            base = 64 * hd
                            nc.tensor.matmul(
                                pss[hd][:, 512 * half + c:512 * (half + 1)],
                                kT[base:base + 64, 128 * ki:128 * (ki + 1)],
                                qT[base:base + 64,
                                   512 * qw + c:512 * (qw + 1)],
                                start=True, stop=True,
                                skip_group_check=True)
                    j0 = 2 * u - 4 * qw
                    j1 = j0 + 1
                    c0 = 128 * j0 if j0 > 0 else 0
                    # one exp per head spans both halves when contiguous;
                    # when the half-1 diag shrink leaves an unwritten PSUM
                    # gap, split the exp around it (reading the gap would
                    # race with the slot's previous occupant)
                    for hd in range(2):
                        if j1 > 0 and not NOSHRINK:
                            nc.scalar.activation(
                                Et[:, 1024 * hd + c0:1024 * hd + 512],
                                pss[hd][:, c0:512], EXP, scale=0.125)
                            c1 = 128 * j1
                            nc.scalar.activation(
                                Et[:, 1024 * hd + 512 + c1:1024 * (hd + 1)],
                                pss[hd][:, 512 + c1:1024], EXP, scale=0.125)
                        else:
                            nc.scalar.activation(
                                Et[:, 1024 * hd + c0:1024 * (hd + 1)],
                                pss[hd][:, c0:1024], EXP, scale=0.125)
                    # causal mask inside the diagonal 128x128 blocks:
                    # multiply by 0/1 upper-triangle (k<=q keeps)
                    for half in range(2):
                        j = 2 * u + half - 4 * qw
                        if j >= 0:
                            for hd in range(2):
                                off = 1024 * hd + 512 * half + 128 * j
                                nc.vector.tensor_mul(
                                    Et[:, off:off + 128],
                                    Et[:, off:off + 128], tri_sb[:])
                    return Et

                def emit_ctx_qc(hp, qw, qc, Ets, psc):
                    """ctx for one q-chunk, both heads: psc[hd][q, 65*qc+d]
                    = sum_ki E^T chunk (stationary) x V_aug chunk (moving).
                    qc-contiguous so each PSUM bank has exactly one open
                    accumulation group at a time; fully-masked (ki, qc)
                    blocks are skipped."""
                    for ki in range(0, 4 * qw + qc + 1):
                        u, half = divmod(ki, 2)
                        Et = Ets[u]
                        for hd in range(2):
                            h = 2 * hp + hd
                            nc.tensor.matmul(
                                psc[hd][:, 65 * qc:65 * qc + 65],
                                Et[:, 1024 * hd + 512 * half + 128 * qc:
                                    1024 * hd + 512 * half + 128 * qc + 128],
                                V_sb[ki][:, 65 * h:65 * h + 65],
                                start=(ki == 0), stop=(ki == 4 * qw + qc),
                                skip_group_check=True)

                def emit_norm(hp, qw, psc):
                    """Per-partition softmax normalization (q is on
                    partitions), then PE-transpose ctx back to [d, q]. The
                    raw ctx is copied out of PSUM first so the psc slots
                    free fast (the next step's first ctx write reuses them);
                    the rest of the norm runs off the critical path."""
                    craws = []
                    for hd in range(2):
                        craw = NP.tile([128, 260], F32, name="craw_t")
                        nc.vector.tensor_copy(craw[:], psc[hd][:])
                        craws.append(craw)
                    # pt shares the out-proj pb tag (both are allocated
                    # right before their writers, keeping the slot ring in
                    # emission order)
                    pt = ps_tile([128, 512], "psc_t", 2, dtype=BF16)
                    ctxns = []
                    for hd in range(2):
                        craw = craws[hd]
                        cv = craw[:].rearrange("p (q x) -> p q x", x=65)
                        rinv = NP.tile([128, 4], F32, name="rinv_t")
                        nc.vector.reciprocal(
                            rinv[:].rearrange("p (q x) -> p q x", x=1),
                            cv[:, :, 64:65])
                        ctxn = NP.tile([128, 256], BF16, name="ctxn_t")
                        ctxns.append((craw, rinv, ctxn))
                    # qc-outer, with per-qc ctxT copy-out: each out-proj
                    # s-chunk only waits its own 128-column block
                    for qc in range(4):
                        for hd in range(2):
                            craw, rinv, ctxn = ctxns[hd]
                            nc.vector.tensor_scalar_mul(
                                ctxn[:, 64 * qc:64 * (qc + 1)],
                                craw[:, 65 * qc:65 * qc + 64],
                                rinv[:, qc:qc + 1])
                            nc.tensor.transpose(
                                pt[64 * hd:64 * (hd + 1),
                                   128 * qc:128 * (qc + 1)],
                                ctxn[:, 64 * qc:64 * (qc + 1)], id_sb[:])
                        nc.vector.tensor_copy(
                            ctxT_sb[hp][:, 512 * qw + 128 * qc:
                                         512 * qw + 128 * (qc + 1)],
                            pt[:, 128 * qc:128 * (qc + 1)])

                def emit_outproj_sc(sc):
                    osb = OP.tile([128, E], F32, name="osb_t")
                    # two f-window chains on two PSUM slots, c-outer so the
                    # ctxT stationary is loaded once per c; the two slots'
                    # copy-out rotations hide each other's latency
                    pos = {0: ps_tile([128, 512], "psc_t", 2),
                           512: ps_tile([128, 256], "psc_t", 2)}
                    for c in range(3):
                        for f0, fn in ((0, 512), (512, 256)):
                            nc.tensor.matmul(
                                pos[f0][:, 0:fn],
                                ctxT_sb[c][:, 128 * sc:128 * (sc + 1)],
                                woT_sb[c][:, f0:f0 + fn],
                                start=(c == 0),
                                stop=(not with_bias and c == 2),
                                skip_group_check=True)
                    for f0, fn in ((0, 512), (512, 256)):
                        if with_bias:
                            nc.tensor.matmul(pos[f0][:, 0:fn],
                                             on_sb[:, 0:128],
                                             bo_sb[:, f0:f0 + fn],
                                             start=False, stop=True,
                                             skip_group_check=True)
                        nc.vector.tensor_copy(osb[:, f0:f0 + fn],
                                              pos[f0][:, 0:fn])
                    nc.sync.dma_start(y.ap()[128 * sc:128 * (sc + 1), :],
                                      osb[:])

                def emit_attention():
                    # software pipeline: ctx runs as per-q-chunk tasks (each
                    # a full contiguous PSUM accumulation group) queued when
                    # a step's scores complete; one task is popped per unit
                    # so ctx/norm/out-proj spread between later units while
                    # ACT chews on exps.
                    work = []      # deferred ctx/norm/outproj thunks

                    def flush_one():
                        if work:
                            work.pop(0)()
                        if len(work) > 2:   # backlog guard near the end
                            work.pop(0)()

                    def finish_step(hp, qw, Ets):
                        # psc is allocated lazily at the first ctx task so
                        # the PSUM slot ring advances in emission order
                        holder = {}

                        def get_psc():
                            if not holder:
                                holder[0] = {
                                    hd: ps_tile([128, 260], "psc_t", 2)
                                    for hd in range(2)}
                            return holder[0]

                        for qc in range(4):
                            work.append(lambda qc=qc: emit_ctx_qc(
                                hp, qw, qc, Ets, get_psc()))
                        work.append(lambda: emit_norm(hp, qw, get_psc()))
                        if hp == 2:
                            for sc in range(4 * qw, 4 * qw + 4):
                                work.append(
                                    lambda sc=sc: emit_outproj_sc(sc))

                    # phase-1 chains interleaved between attention units:
                    # (hp, qw, u) -> thunks emitted right after that unit's
                    # scores+flush (so the next exp is never delayed by a
                    # projection chain). Deadlines: qkT window-pair swp of
                    # f-chunks (hp)/(3+hp) is read by (hp, qw>=2*swp) scores;
                    # V[k] is read by the ctx of unit u=k//2, which flushes
                    # DEPTH units later. qk chains (2.6us) avoid the last
                    # unit of a step; V chains (1us) fit anywhere.
                    intra = {}

                    def add(hp, qw, u, fn):
                        intra.setdefault((hp, qw, u), []).append(fn)

                    def addv(hp, qw, u, sc):
                        add(hp, qw, u, lambda: emit_v_chunk(sc))

                    def addqk(hp, qw, u, fo, swp):
                        add(hp, qw, u, lambda: emit_qk_chain(fo, swp))

                    add(0, 0, 0, lambda: emit_qk_chain(0, 0, windows=(1,)))
                    add(0, 0, 1, lambda: emit_qk_chain(3, 0, windows=(1,)))
                    addv(0, 1, 0, 0)
                    addv(0, 1, 0, 1)
                    addv(0, 1, 1, 2)
                    addqk(0, 1, 1, 0, 1)
                    addv(0, 1, 2, 3)
                    addqk(0, 1, 2, 3, 1)
                    addv(0, 1, 3, 4)
                    addv(0, 2, 0, 5)
                    addv(0, 2, 1, 6)
                    addv(0, 2, 2, 7)
                    addv(0, 2, 3, 8)
                    addv(0, 2, 4, 9)
                    addv(0, 2, 5, 10)
                    addv(0, 3, 0, 11)
                    addv(0, 3, 1, 12)
                    addv(0, 3, 2, 13)
                    addv(0, 3, 3, 14)
                    addv(0, 3, 4, 15)
                    addqk(0, 3, 5, 1, 0)
                    addqk(0, 3, 6, 4, 0)
                    addqk(1, 0, 0, 1, 1)
                    addqk(1, 1, 0, 4, 1)
                    addqk(1, 2, 0, 2, 0)
                    addqk(1, 2, 2, 5, 0)
                    addqk(1, 3, 0, 2, 1)
                    addqk(1, 3, 2, 5, 1)

                    for hp in range(3):
                        for qw in range(QW):
                            nu = 2 * qw + 2
                            Ets = []
                            for u in range(nu):
                                Ets.append(emit_scores(hp, qw, u))
                                for fn in intra.get((hp, qw, u), ()):
                                    fn()
                                flush_one()
                            finish_step(hp, qw, Ets)
                    while work:
                        flush_one()

                # start-up: only what the first scores need (the (0,0)
                # step reads just s-window 0 of fo 0/3); window 1 and the
                # first V chunks follow as intra tasks behind the scores
                emit_qk_chain(0, 0, windows=(0,))
                emit_qk_chain(3, 0, windows=(0,))
                emit_attention()

    return _patch_multiwait(nc)


_NC = {}


def _get_nc(with_bias=True):
    if with_bias not in _NC:
        _NC[with_bias] = build_nc(with_bias=with_bias)
    return _NC[with_bias]


def _prep_core_inputs(x, in_proj_w, in_proj_b, out_w, out_b):
    """Build the 8 per-core input dicts (host-side shard + transpose)."""
    import ml_dtypes
    bf16 = ml_dtypes.bfloat16
    # 0/1 keep-mask for S^T[k, q] diagonal blocks: keep where k <= q
    tri_np = (np.arange(128)[:, None] <= np.arange(128)[None, :])
    tri_bf = tri_np.astype(bf16)
    id_bf = np.eye(128, dtype=np.float32).astype(bf16)
    ones_np = np.ones((1, 128), np.float32).astype(bf16)

    xT_by_b = [np.asarray(x[b]).T.astype(bf16) for b in range(B)]

    in_maps = []
    for c in range(8):
        b = c // 2
        g = c % 2
        f0 = FPC * g
        Wq = np.asarray(in_proj_w[f0:f0 + FPC])
        Wk = np.asarray(in_proj_w[E + f0:E + f0 + FPC])
        Wv = np.asarray(in_proj_w[2 * E + f0:2 * E + f0 + FPC])
        bq = np.asarray(in_proj_b[f0:f0 + FPC])
        bk = np.asarray(in_proj_b[E + f0:E + f0 + FPC])
        bvv = np.asarray(in_proj_b[2 * E + f0:2 * E + f0 + FPC])
        Wo = np.asarray(out_w[:, f0:f0 + FPC])
        bqk_np = np.concatenate([bq, bk]).astype(np.float32).reshape(6, 128).T
        in_maps.append({
            "xT": xT_by_b[b],
            "wqkT": np.ascontiguousarray(
                np.concatenate([Wq, Wk], axis=0).T).astype(bf16),
            "wvT": np.ascontiguousarray(Wv.T).astype(bf16),
            "woT": np.ascontiguousarray(Wo.T).astype(bf16),
            "bqk": np.ascontiguousarray(bqk_np),
            "bv": bvv.reshape(1, FPC).astype(bf16),
            # out bias only on even cores so the host-side pair-sum is exact
            "bo": np.asarray(out_b).reshape(1, E).astype(bf16) if g == 0
                  else np.zeros((1, E), bf16),
            "tri": tri_bf,
            "ident": id_bf,
            "ones": ones_np,
        })
    return in_maps


def kernel(x, in_proj_w, in_proj_b, out_w, out_b):
    zero_bias = (not np.any(np.asarray(in_proj_b))) and \
                (not np.any(np.asarray(out_b)))
    nc = _get_nc(with_bias=not zero_bias)
    in_maps = _prep_core_inputs(x, in_proj_w, in_proj_b, out_w, out_b)
    res = run_bass_kernel_spmd(nc, in_maps, core_ids=list(range(8)))
    out = np.empty((B, S, E), np.float32)
    for b in range(B):
        out[b] = res.results[2 * b]["y"] + res.results[2 * b + 1]["y"]
    return out


# revision 65
# speedup vs baseline: 1.3008x; 1.0472x over previous
"""Multi-head attention (B=4, S=2048, E=768, H=12, D=64, causal) on 8 trn2
NeuronCores.

Sharding: core c -> batch b = c//2, head-half g = c%2 (6 heads each).
Each core computes its 6 heads' attention plus the partial output
projection; the host sums the two half-head partials per batch.

On-device strategy (per core):
  - Host pre-transposes x[b] and the weight slices so every matmul
    contraction dim (e / d / k / e_h) lands on SBUF partitions; no
    on-device transposes of inputs. Everything ships bf16 (halves DMA).
  - QK projection emits qk^T [f, s]; V projection emits V [s, f] with a
    ones column packed per head (V_aug) so the PE computes softmax
    row-sums for free.
  - Scores are computed TRANSPOSED (S^T[k, q] = K^T Q) in bf16; the
    diagonal tiles shrink their moving window to skip fully-masked
    columns. Causal masking inside the diagonal 128x128 block is a DVE
    multiply of the exp'd tile by a 0/1 lower-triangle (no PE matmul).
  - ctx is computed with E^T chunks STATIONARY and V_aug [k, 65] MOVING:
    65-cycle matmuls, and fully-masked (q-chunk, k-chunk) blocks are
    skipped entirely. Each 65-col psc region is accumulated as ONE
    contiguous PSUM group (qc-outer over all k-chunks of the step) --
    interleaving several open accumulation groups inside one PSUM bank
    corrupts the early-stopping groups. The result lands [q, d+1] with
    q on partitions, so softmax normalization is a per-partition
    reciprocal + scalar multiply on DVE (no PE broadcast matmul), and a
    PE transpose brings ctx^T [d, q] back for the output projection.
  - All big matmuls run in bf16 at 1 cycle/row.
  - The attention phase is ACT(exp)-throughput-bound, so scores/exps
    stream per unit while ctx/norm/out-proj run as deferred tasks popped
    between later units, and the projection chains of phase 1 are
    interleaved into the attention loop with just-in-time deadlines.
"""
import sys, json, os

for _p in ("/opt/trn_rl_repo",):
    if _p not in sys.path and os.path.isdir(_p):
        sys.path.insert(0, _p)

import numpy as np
import concourse.bass as bass
import concourse.mybir as mybir
import concourse.tile as tile
from concourse.bass_utils import run_bass_kernel_spmd

B, S, E, H, D = 4, 2048, 768, 12, 64
HPC = H // 2          # heads per core = 6
FPC = HPC * D         # features per core per q/k/v = 384
EC = E // 128         # 6 contraction chunks for projections
SC = S // 128         # 16 s-chunks
QW = S // 512         # 4 q-windows
KC = S // 128         # 16 k-chunks
F32 = mybir.dt.float32
BF16 = mybir.dt.bfloat16
EXP = mybir.ActivationFunctionType.Exp


def _patch_multiwait(nc, max_waits=1):
    """This container's walrus rejects instructions with more than one sync
    wait. Split excess waits onto same-engine NOPs emitted immediately
    before the instruction (same-engine streams are order-preserving)."""
    raw = nc.to_json_bytes()
    m = json.loads(raw)
    for f in m["functions"]:
        for b in f["blocks"]:
            out = []
            for inst in b["instructions"]:
                si = inst.get("sync_info") or {}
                ws = si.get("on_wait") or []
                if len(ws) > max_waits:
                    eng = inst["engine"]
                    for i, w in enumerate(ws[:-max_waits]):
                        out.append({
                            "debug": inst.get("debug", 0), "engine": eng,
                            "ins": [], "name": inst["name"] + f"-mw{i}",
                            "opcode": "NoOp", "outs": [],
                            "sync_info": {"on_update": [], "on_wait": [w]},
                        })
                    si["on_wait"] = ws[-max_waits:]
                out.append(inst)
            b["instructions"] = out
    patched = json.dumps(m).encode()
    nc.to_json_bytes = lambda: patched
    return nc


def build_nc(with_bias=True):
    nc = bass.Bass()
    xT = nc.dram_tensor("xT", [E, S], BF16, kind="ExternalInput")
    wqkT = nc.dram_tensor("wqkT", [E, 2 * FPC], BF16, kind="ExternalInput")
    wvT = nc.dram_tensor("wvT", [E, FPC], BF16, kind="ExternalInput")
    woT = nc.dram_tensor("woT", [FPC, E], BF16, kind="ExternalInput")
    bqk = nc.dram_tensor("bqk", [128, 2 * FPC // 128], F32, kind="ExternalInput")
    bv = nc.dram_tensor("bv", [1, FPC], BF16, kind="ExternalInput")
    bo = nc.dram_tensor("bo", [1, E], BF16, kind="ExternalInput")
    tri = nc.dram_tensor("tri", [128, 128], BF16, kind="ExternalInput")
    ident = nc.dram_tensor("ident", [128, 128], BF16, kind="ExternalInput")
    ones = nc.dram_tensor("ones", [1, 128], BF16, kind="ExternalInput")
    y = nc.dram_tensor("y", [S, E], F32, kind="ExternalOutput")

    with tile.TileContext(nc) as tc, \
         nc.allow_low_precision(reason="bf16 matmul pipeline by design"):
        with tc.tile_pool(name="persist", bufs=1) as P, \
             tc.tile_pool(name="ps", bufs=1, space="PSUM") as PS:
            # --- persistent tiles (bottom-of-stack, live whole kernel)
            qkT_sb = [P.tile([128, S], BF16, name=f"qkT{i}") for i in range(6)]
            V_sb = [P.tile([128, 65 * HPC], BF16, name=f"V{i}") for i in range(KC)]
            ctxT_sb = [P.tile([128, S], BF16, name=f"ctxT{i}") for i in range(3)]
            woT_sb = [P.tile([128, E], BF16, name=f"woT{i}") for i in range(3)]
            bqk_sb = P.tile([128, 6], F32, name="bqk_sb")
            bv_sb = P.tile([1, FPC], BF16, name="bv_sb")
            bo_sb = P.tile([1, E], BF16, name="bo_sb")
            tri_sb = P.tile([128, 128], BF16, name="tri_sb")
            id_sb = P.tile([128, 128], BF16, name="id_sb")
            on_sb = P.tile([1, 128], BF16, name="on_sb")

            def ps_tile(shape, tag, bufs, dtype=F32):
                return PS.tile(shape, dtype, name=tag, tag=tag, bufs=bufs)

            # ============ phase 1 (projections) + attention, interleaved ====
            # The attention phase is ACT(exp)-throughput-bound, so the
            # projections are software-pipelined INTO the attention loop:
            # only the chains needed for the first scores run up front, and
            # the rest are emitted between attention units where the PE has
            # slack while ACT chews on exps.
            with tc.tile_pool(name="inp", bufs=1) as PI, \
                 tc.tile_pool(name="esb", bufs=14) as EP, \
                 tc.tile_pool(name="nrm", bufs=12) as NP, \
                 tc.tile_pool(name="osb", bufs=3) as OP:
                # consolidated phase-1 tiles: one DMA dispatch covers all six
                # e-chunks (the SP sequencer costs ~650ns per DMA, so fewer,
                # bigger strided DMAs win)
                xT_sb = PI.tile([128, EC * S], BF16, name="xT_all")
                wqkT_sb = PI.tile([128, EC * 2 * FPC], BF16, name="wqkT_all")
                wvT_sb = PI.tile([128, EC * FPC], BF16, name="wvT_all")
                xs = xT_sb[:].rearrange("p (e s) -> p e s", e=EC)
                xd = xT.ap().rearrange("(e p) s -> p e s", p=128)
                qs = wqkT_sb[:].rearrange("p (e f) -> p e f", e=EC)
                qd = wqkT.ap().rearrange("(e p) f -> p e f", p=128)
                # DMA order: first the tensors gating the two startup chains
                # (wqkT cols of fo=0/3, xT cols 0:1024), then wvT (V chunks),
                # tri (first diag mask), the rest of xT/wqkT, and the tail.
                # per-chunk pass-1 xT so the startup chains pipeline with the
                # DMA stream chunk by chunk
                nc.sync.dma_start(xs[:, 0, 0:1024], xd[:, 0, 0:1024])
                nc.sync.dma_start(qs[:, :, 0:128], qd[:, :, 0:128])
                nc.sync.dma_start(qs[:, :, 384:512], qd[:, :, 384:512])
                for i in range(1, EC):
                    nc.sync.dma_start(xs[:, i, 0:1024], xd[:, i, 0:1024])
                for i in range(EC):
                    nc.sync.dma_start(xs[:, i, 1024:S], xd[:, i, 1024:S])
                nc.sync.dma_start(
                    wvT_sb[:].rearrange("p (e f) -> p e f", e=EC),
                    wvT.ap().rearrange("(e p) f -> p e f", p=128))
                nc.sync.dma_start(tri_sb[:], tri.ap())
                nc.sync.dma_start(qs[:, :, 128:384], qd[:, :, 128:384])
                nc.sync.dma_start(qs[:, :, 512:768], qd[:, :, 512:768])
                nc.sync.dma_start(id_sb[:], ident.ap())
                for i in range(3):
                    nc.sync.dma_start(woT_sb[i][:],
                                      woT.ap()[128 * i:128 * (i + 1), :])
                nc.sync.dma_start(bqk_sb[:], bqk.ap())
                nc.sync.dma_start(bv_sb[:], bv.ap())
                nc.sync.dma_start(on_sb[:], ones.ap())
                nc.sync.dma_start(bo_sb[:], bo.ap())

                def emit_qk_chain(fo, swp, windows=(0, 1)):
                    """qk-proj for f-chunk fo, s-windows 2*swp+windows.
                    Concurrent window chains in one pss slot."""
                    pair = ps_tile([128, 1024], "pss_t", 3)
                    for ecc in range(EC):
                        for swl in windows:
                            sw = 2 * swp + swl
                            nc.tensor.matmul(
                                pair[:, 512 * swl:512 * (swl + 1)],
                                wqkT_sb[:, 768 * ecc + 128 * fo:
                                        768 * ecc + 128 * (fo + 1)],
                                xT_sb[:, S * ecc + 512 * sw:
                                      S * ecc + 512 * (sw + 1)],
                                start=(ecc == 0), stop=(ecc == EC - 1),
                                skip_group_check=True)
                    # per-window copy-out so the first window's consumers
                    # don't wait for the second's
                    for swl in windows:
                        dst = qkT_sb[fo][:, 1024 * swp + 512 * swl:
                                         1024 * swp + 512 * (swl + 1)]
                        src = pair[:, 512 * swl:512 * (swl + 1)]
                        if with_bias:
                            nc.vector.tensor_scalar_add(
                                dst, src, bqk_sb[:, fo:fo + 1])
                        else:
                            nc.vector.tensor_copy(dst, src)

                def emit_v_chunk(sc):
                    """V-proj for s-chunk sc (one k-chunk of V_aug)."""
                    psv = ps_tile([128, FPC], "pss_t", 3)
                    for ecc in range(EC):
                        nc.tensor.matmul(
                            psv[:],
                            xT_sb[:, S * ecc + 128 * sc:
                                  S * ecc + 128 * (sc + 1)],
                            wvT_sb[:, FPC * ecc:FPC * (ecc + 1)],
                            start=(ecc == 0),
                            stop=(not with_bias and ecc == EC - 1),
                            skip_group_check=True)
                    if with_bias:
                        nc.tensor.matmul(psv[:], on_sb[:, 0:128],
                                         bv_sb[:], start=False, stop=True,
                                         skip_group_check=True)
                    vv = V_sb[sc][:].rearrange("p (h x) -> p h x", x=65)
                    nc.vector.tensor_copy(
                        vv[:, :, 0:64],
                        psv[:].rearrange("p (h x) -> p h x", x=64))
                    nc.gpsimd.memset(vv[:, :, 64:65], 1.0)

                def emit_scores(hp, qw, u):
                    """Scores S^T[k, q] for a pair of k-chunks, both heads,
                    + exp, + DVE causal masks on diag blocks. Returns the
                    bf16 exp'd tile Et [128, 2048]
                    (cols 1024*hd + 512*half + qlocal)."""
                    qT, kT = qkT_sb[hp], qkT_sb[3 + hp]
                    pss = {hd: ps_tile([128, 1024], "pss_t", 3)
                           for hd in range(2)}
                    Et = EP.tile([128, 2048], BF16, name="E_t")
                    NOSHRINK = bool(int(os.environ.get("K_NOSHRINK", "0")))
                    for half in range(2):
                        ki = 2 * u + half
                        j = ki - 4 * qw
                        c = 128 * j if j > 0 and not NOSHRINK else 0
                        # strict row-group alternation (base 0,64,0,64) so the
                        # K=64 score matmul pairs run concurrently on the PE;
                        # diag tiles shrink the moving window to skip
                        # fully-masked columns
                        for hd in range(2):
                            base = 64 * hd
                            nc.tensor.matmul(
                                pss[hd][:, 512 * half + c:512 * (half + 1)],
                                kT[base:base + 64, 128 * ki:128 * (ki + 1)],
                                qT[base:base + 64,
                                   512 * qw + c:512 * (qw + 1)],
                                start=True, stop=True,
                                skip_group_check=True)
                    j0 = 2 * u - 4 * qw
                    j1 = j0 + 1
                    c0 = 128 * j0 if j0 > 0 else 0
                    # one exp per head spans both halves when contiguous;
                    # when the half-1 diag shrink leaves an unwritten PSUM
                    # gap, split the exp around it (reading the gap would
                    # race with the slot's previous occupant)
                    for hd in range(2):
                        if j1 > 0 and not NOSHRINK:
                            nc.scalar.activation(
                                Et[:, 1024 * hd + c0:1024 * hd + 512],
                                pss[hd][:, c0:512], EXP, scale=0.125)
                            c1 = 128 * j1
                            nc.scalar.activation(
                                Et[:, 1024 * hd + 512 + c1:1024 * (hd + 1)],
                                pss[hd][:, 512 + c1:1024], EXP, scale=0.125)
                        else:
                            nc.scalar.activation(
                                Et[:, 1024 * hd + c0:1024 * (hd + 1)],
                                pss[hd][:, c0:1024], EXP, scale=0.125)
                    # causal mask inside the diagonal 128x128 blocks:
                    # multiply by 0/1 upper-triangle (k<=q keeps)
                    for half in range(2):
                        j = 2 * u + half - 4 * qw
                        if j >= 0:
                            for hd in range(2):
                                off = 1024 * hd + 512 * half + 128 * j
                                nc.vector.tensor_mul(
                                    Et[:, off:off + 128],
                                    Et[:, off:off + 128], tri_sb[:])
                    return Et

                def emit_ctx_qc(hp, qw, qc, Ets, psc):
                    """ctx for one q-chunk, both heads: psc[hd][q, 65*qc+d]
                    = sum_ki E^T chunk (stationary) x V_aug chunk (moving).
                    qc-contiguous so each PSUM bank has exactly one open
                    accumulation group at a time; fully-masked (ki, qc)
                    blocks are skipped."""
                    for ki in range(0, 4 * qw + qc + 1):
                        u, half = divmod(ki, 2)
                        Et = Ets[u]
                        for hd in range(2):
                            h = 2 * hp + hd
                            nc.tensor.matmul(
                                psc[hd][:, 65 * qc:65 * qc + 65],
                                Et[:, 1024 * hd + 512 * half + 128 * qc:
                                    1024 * hd + 512 * half + 128 * qc + 128],
                                V_sb[ki][:, 65 * h:65 * h + 65],
                                start=(ki == 0), stop=(ki == 4 * qw + qc),
                                skip_group_check=True)

                def emit_norm_head(hp, qw, psc, st):
                    """Per-partition softmax normalization head: copy the
                    raw ctx out of PSUM (freeing the psc slots for the next
                    step's first ctx write) and compute the reciprocal
                    row-sums. The per-qc finish runs as separate tasks."""
                    craws = []
                    for hd in range(2):
                        craw = NP.tile([128, 260], F32, name="craw_t")
                        nc.vector.tensor_copy(craw[:], psc[hd][:])
                        craws.append(craw)
                    # pt is allocated right before its writers, keeping the
                    # slot ring in emission order
                    pt = ps_tile([128, 512], "psc_t", 2, dtype=BF16)
                    ctxns = []
                    for hd in range(2):
                        craw = craws[hd]
                        cv = craw[:].rearrange("p (q x) -> p q x", x=65)
                        rinv = NP.tile([128, 4], F32, name="rinv_t")
                        nc.vector.reciprocal(
                            rinv[:].rearrange("p (q x) -> p q x", x=1),
                            cv[:, :, 64:65])
                        ctxn = NP.tile([128, 256], BF16, name="ctxn_t")
                        ctxns.append((craw, rinv, ctxn))
                    st["pt"] = pt
                    st["ctxns"] = ctxns

                def emit_norm_qc(hp, qw, qc, st):
                    """Normalize + transpose + copy out one 128-column ctxT
                    block, so each out-proj s-chunk only waits its own."""
                    pt, ctxns = st["pt"], st["ctxns"]
                    for hd in range(2):
                        craw, rinv, ctxn = ctxns[hd]
                        nc.vector.tensor_scalar_mul(
                            ctxn[:, 64 * qc:64 * (qc + 1)],
                            craw[:, 65 * qc:65 * qc + 64],
                            rinv[:, qc:qc + 1])
                        nc.tensor.transpose(
                            pt[64 * hd:64 * (hd + 1),
                               128 * qc:128 * (qc + 1)],
                            ctxn[:, 64 * qc:64 * (qc + 1)], id_sb[:])
                    nc.vector.tensor_copy(
                        ctxT_sb[hp][:, 512 * qw + 128 * qc:
                                     512 * qw + 128 * (qc + 1)],
                        pt[:, 128 * qc:128 * (qc + 1)])

                def emit_outproj_sc(sc):
                    osb = OP.tile([128, E], F32, name="osb_t")
                    # two f-window chains on two PSUM slots, c-outer so the
                    # ctxT stationary is loaded once per c; the two slots'
                    # copy-out rotations hide each other's latency
                    # the final step's out-proj (sc>=12) runs at the
                    # drain when scores are done, so it can use the three
                    # idle pss slots and dodge the 2-slot rotation stalls
                    tg, nb = ("pss_t", 3) if sc >= 12 else ("psc_t", 2)
                    pos = {0: ps_tile([128, 512], tg, nb),
                           512: ps_tile([128, 256], tg, nb)}
                    for c in range(3):
                        for f0, fn in ((0, 512), (512, 256)):
                            nc.tensor.matmul(
                                pos[f0][:, 0:fn],
                                ctxT_sb[c][:, 128 * sc:128 * (sc + 1)],
                                woT_sb[c][:, f0:f0 + fn],
                                start=(c == 0),
                                stop=(not with_bias and c == 2),
                                skip_group_check=True)
                    for f0, fn in ((0, 512), (512, 256)):
                        if with_bias:
                            nc.tensor.matmul(pos[f0][:, 0:fn],
                                             on_sb[:, 0:128],
                                             bo_sb[:, f0:f0 + fn],
                                             start=False, stop=True,
                                             skip_group_check=True)
                        nc.vector.tensor_copy(osb[:, f0:f0 + fn],
                                              pos[f0][:, 0:fn])
                    nc.sync.dma_start(y.ap()[128 * sc:128 * (sc + 1), :],
                                      osb[:])

                def emit_attention():
                    # software pipeline: ctx runs as per-q-chunk tasks (each
                    # a full contiguous PSUM accumulation group) queued when
                    # a step's scores complete; one task is popped per unit
                    # so ctx/norm/out-proj spread between later units while
                    # ACT chews on exps.
                    work = []      # deferred ctx/norm/outproj thunks

                    def flush_one():
                        if work:
                            work.pop(0)()
                        if len(work) > 2:   # backlog guard near the end
                            work.pop(0)()

                    def finish_step(hp, qw, Ets):
                        # psc is allocated lazily at the first ctx task so
                        # the PSUM slot ring advances in emission order
                        holder = {}

                        def get_psc():
                            if not holder:
                                holder[0] = {
                                    hd: ps_tile([128, 260], "psc_t", 2)
                                    for hd in range(2)}
                            return holder[0]

                        st = {}
                        for qc in range(4):
                            work.append(lambda qc=qc: emit_ctx_qc(
                                hp, qw, qc, Ets, get_psc()))
                        if hp < 2:
                            def norm_all():
                                emit_norm_head(hp, qw, get_psc(), st)
                                for qc in range(4):
                                    emit_norm_qc(hp, qw, qc, st)
                            work.append(norm_all)
                        else:
                            # per-qc norm+out-proj tasks shorten the serial
                            # tail: each s-chunk starts once its own
                            # 128-column ctxT block lands
                            work.append(lambda: emit_norm_head(
                                hp, qw, get_psc(), st))

                            def norm_op(qc):
                                emit_norm_qc(hp, qw, qc, st)
                                emit_outproj_sc(4 * qw + qc)
                            for qc in range(4):
                                work.append(lambda qc=qc: norm_op(qc))

                    # phase-1 chains interleaved between attention units:
                    # (hp, qw, u) -> thunks emitted right after that unit's
                    # scores+flush (so the next exp is never delayed by a
                    # projection chain). Deadlines: qkT window-pair swp of
                    # f-chunks (hp)/(3+hp) is read by (hp, qw>=2*swp) scores;
                    # V[k] is read by the ctx of unit u=k//2, which flushes
                    # DEPTH units later. qk chains (2.6us) avoid the last
                    # unit of a step; V chains (1us) fit anywhere.
                    intra = {}

                    def add(hp, qw, u, fn):
                        intra.setdefault((hp, qw, u), []).append(fn)

                    def addv(hp, qw, u, sc):
                        add(hp, qw, u, lambda: emit_v_chunk(sc))

                    def addqk(hp, qw, u, fo, swp):
                        add(hp, qw, u, lambda: emit_qk_chain(fo, swp))

                    def addqkw(hp, qw, u, fo, swp, w):
                        add(hp, qw, u,
                            lambda: emit_qk_chain(fo, swp, windows=(w,)))

                    addqkw(0, 0, 0, 0, 0, 1)
                    addqkw(0, 0, 1, 3, 0, 1)
                    addv(0, 1, 0, 0)
                    addv(0, 1, 0, 1)
                    addv(0, 1, 1, 2)
                    addqkw(0, 1, 1, 0, 1, 0)
                    addv(0, 1, 2, 3)
                    addqkw(0, 1, 2, 3, 1, 0)
                    addv(0, 1, 3, 4)
                    addv(0, 2, 0, 5)
                    addv(0, 2, 1, 6)
                    addqkw(0, 2, 1, 0, 1, 1)
                    addv(0, 2, 2, 7)
                    addv(0, 2, 3, 8)
                    addqkw(0, 2, 3, 3, 1, 1)
                    addv(0, 2, 4, 9)
                    addv(0, 2, 5, 10)
                    addv(0, 3, 0, 11)
                    addv(0, 3, 1, 12)
                    addv(0, 3, 2, 13)
                    addv(0, 3, 3, 14)
                    addv(0, 3, 4, 15)
                    addqkw(0, 3, 5, 1, 0, 0)
                    addqkw(0, 3, 6, 1, 0, 1)
                    addqkw(0, 3, 7, 4, 0, 0)
                    addqkw(1, 0, 0, 4, 0, 1)
                    addqkw(1, 1, 0, 1, 1, 0)
                    addqkw(1, 1, 1, 4, 1, 0)
                    addqkw(1, 1, 2, 1, 1, 1)
                    addqkw(1, 1, 3, 4, 1, 1)
                    addqkw(1, 2, 0, 2, 0, 0)
                    addqkw(1, 2, 2, 5, 0, 0)
                    addqkw(1, 2, 4, 2, 0, 1)
                    addqkw(1, 3, 0, 5, 0, 1)
                    addqkw(1, 3, 2, 2, 1, 0)
                    addqkw(1, 3, 4, 5, 1, 0)
                    addqkw(1, 3, 6, 2, 1, 1)
                    addqkw(2, 0, 0, 5, 1, 1)

                    for hp in range(3):
                        for qw in range(QW):
                            nu = 2 * qw + 2
                            Ets = []
                            for u in range(nu):
                                Ets.append(emit_scores(hp, qw, u))
                                for fn in intra.get((hp, qw, u), ()):
                                    fn()
                                flush_one()
                            finish_step(hp, qw, Ets)
                    while work:
                        flush_one()

                # start-up: only what the first scores need (the (0,0)
                # step reads just s-window 0 of fo 0/3); window 1 and the
                # first V chunks follow as intra tasks behind the scores
                emit_qk_chain(0, 0, windows=(0,))
                emit_qk_chain(3, 0, windows=(0,))
                emit_attention()

    return _patch_multiwait(nc)


_NC = {}


def _get_nc(with_bias=True):
    if with_bias not in _NC:
        _NC[with_bias] = build_nc(with_bias=with_bias)
    return _NC[with_bias]


def _prep_core_inputs(x, in_proj_w, in_proj_b, out_w, out_b):
    """Build the 8 per-core input dicts (host-side shard + transpose)."""
    import ml_dtypes
    bf16 = ml_dtypes.bfloat16
    # 0/1 keep-mask for S^T[k, q] diagonal blocks: keep where k <= q
    tri_np = (np.arange(128)[:, None] <= np.arange(128)[None, :])
    tri_bf = tri_np.astype(bf16)
    id_bf = np.eye(128, dtype=np.float32).astype(bf16)
    ones_np = np.ones((1, 128), np.float32).astype(bf16)

    xT_by_b = [np.asarray(x[b]).T.astype(bf16) for b in range(B)]

    in_maps = []
    for c in range(8):
        b = c // 2
        g = c % 2
        f0 = FPC * g
        Wq = np.asarray(in_proj_w[f0:f0 + FPC])
        Wk = np.asarray(in_proj_w[E + f0:E + f0 + FPC])
        Wv = np.asarray(in_proj_w[2 * E + f0:2 * E + f0 + FPC])
        bq = np.asarray(in_proj_b[f0:f0 + FPC])
        bk = np.asarray(in_proj_b[E + f0:E + f0 + FPC])
        bvv = np.asarray(in_proj_b[2 * E + f0:2 * E + f0 + FPC])
        Wo = np.asarray(out_w[:, f0:f0 + FPC])
        bqk_np = np.concatenate([bq, bk]).astype(np.float32).reshape(6, 128).T
        in_maps.append({
            "xT": xT_by_b[b],
            "wqkT": np.ascontiguousarray(
                np.concatenate([Wq, Wk], axis=0).T).astype(bf16),
            "wvT": np.ascontiguousarray(Wv.T).astype(bf16),
            "woT": np.ascontiguousarray(Wo.T).astype(bf16),
            "bqk": np.ascontiguousarray(bqk_np),
            "bv": bvv.reshape(1, FPC).astype(bf16),
            # out bias only on even cores so the host-side pair-sum is exact
            "bo": np.asarray(out_b).reshape(1, E).astype(bf16) if g == 0
                  else np.zeros((1, E), bf16),
            "tri": tri_bf,
            "ident": id_bf,
            "ones": ones_np,
        })
    return in_maps


def kernel(x, in_proj_w, in_proj_b, out_w, out_b):
    zero_bias = (not np.any(np.asarray(in_proj_b))) and \
                (not np.any(np.asarray(out_b)))
    nc = _get_nc(with_bias=not zero_bias)
    in_maps = _prep_core_inputs(x, in_proj_w, in_proj_b, out_w, out_b)
    res = run_bass_kernel_spmd(nc, in_maps, core_ids=list(range(8)))
    out = np.empty((B, S, E), np.float32)
    for b in range(B):
        out[b] = res.results[2 * b]["y"] + res.results[2 * b + 1]["y"]
    return out


# revision 68
# speedup vs baseline: 1.3021x; 1.0010x over previous
"""Multi-head attention (B=4, S=2048, E=768, H=12, D=64, causal) on 8 trn2
NeuronCores.

Sharding: core c -> batch b = c//2, head-half g = c%2 (6 heads each).
Each core computes its 6 heads' attention plus the partial output
projection; the host sums the two half-head partials per batch.

On-device strategy (per core):
  - Host pre-transposes x[b] and the weight slices so every matmul
    contraction dim (e / d / k / e_h) lands on SBUF partitions; no
    on-device transposes of inputs. Everything ships bf16 (halves DMA).
  - QK projection emits qk^T [f, s]; V projection emits V [s, f] with a
    ones column packed per head (V_aug) so the PE computes softmax
    row-sums for free.
  - Scores are computed TRANSPOSED (S^T[k, q] = K^T Q) in bf16; the
    diagonal tiles shrink their moving window to skip fully-masked
    columns. Causal masking inside the diagonal 128x128 block is a DVE
    multiply of the exp'd tile by a 0/1 lower-triangle (no PE matmul).
  - ctx is computed with E^T chunks STATIONARY and V_aug [k, 65] MOVING:
    65-cycle matmuls, and fully-masked (q-chunk, k-chunk) blocks are
    skipped entirely. Each 65-col psc region is accumulated as ONE
    contiguous PSUM group (qc-outer over all k-chunks of the step) --
    interleaving several open accumulation groups inside one PSUM bank
    corrupts the early-stopping groups. The result lands [q, d+1] with
    q on partitions, so softmax normalization is a per-partition
    reciprocal + scalar multiply on DVE (no PE broadcast matmul), and a
    PE transpose brings ctx^T [d, q] back for the output projection.
  - All big matmuls run in bf16 at 1 cycle/row.
  - The attention phase is ACT(exp)-throughput-bound, so scores/exps
    stream per unit while ctx/norm/out-proj run as deferred tasks popped
    between later units, and the projection chains of phase 1 are
    interleaved into the attention loop with just-in-time deadlines.
"""
import sys, json, os

for _p in ("/opt/trn_rl_repo",):
    if _p not in sys.path and os.path.isdir(_p):
        sys.path.insert(0, _p)

import numpy as np
import concourse.bass as bass
import concourse.mybir as mybir
import concourse.tile as tile
from concourse.bass_utils import run_bass_kernel_spmd

B, S, E, H, D = 4, 2048, 768, 12, 64
HPC = H // 2          # heads per core = 6
FPC = HPC * D         # features per core per q/k/v = 384
EC = E // 128         # 6 contraction chunks for projections
SC = S // 128         # 16 s-chunks
QW = S // 512         # 4 q-windows
KC = S // 128         # 16 k-chunks
F32 = mybir.dt.float32
BF16 = mybir.dt.bfloat16
EXP = mybir.ActivationFunctionType.Exp


def _patch_multiwait(nc, max_waits=1):
    """This container's walrus rejects instructions with more than one sync
    wait. Split excess waits onto same-engine NOPs emitted immediately
    before the instruction (same-engine streams are order-preserving)."""
    raw = nc.to_json_bytes()
    m = json.loads(raw)
    for f in m["functions"]:
        for b in f["blocks"]:
            out = []
            for inst in b["instructions"]:
                si = inst.get("sync_info") or {}
                ws = si.get("on_wait") or []
                if len(ws) > max_waits:
                    eng = inst["engine"]
                    for i, w in enumerate(ws[:-max_waits]):
                        out.append({
                            "debug": inst.get("debug", 0), "engine": eng,
                            "ins": [], "name": inst["name"] + f"-mw{i}",
                            "opcode": "NoOp", "outs": [],
                            "sync_info": {"on_update": [], "on_wait": [w]},
                        })
                    si["on_wait"] = ws[-max_waits:]
                out.append(inst)
            b["instructions"] = out
    patched = json.dumps(m).encode()
    nc.to_json_bytes = lambda: patched
    return nc


def build_nc(with_bias=True):
    nc = bass.Bass()
    xT = nc.dram_tensor("xT", [E, S], BF16, kind="ExternalInput")
    wqkT = nc.dram_tensor("wqkT", [E, 2 * FPC], BF16, kind="ExternalInput")
    wvT = nc.dram_tensor("wvT", [E, FPC], BF16, kind="ExternalInput")
    woT = nc.dram_tensor("woT", [FPC, E], BF16, kind="ExternalInput")
    bqk = nc.dram_tensor("bqk", [128, 2 * FPC // 128], F32, kind="ExternalInput")
    bv = nc.dram_tensor("bv", [1, FPC], BF16, kind="ExternalInput")
    bo = nc.dram_tensor("bo", [1, E], BF16, kind="ExternalInput")
    tri = nc.dram_tensor("tri", [128, 128], BF16, kind="ExternalInput")
    ident = nc.dram_tensor("ident", [128, 128], BF16, kind="ExternalInput")
    ones = nc.dram_tensor("ones", [1, 128], BF16, kind="ExternalInput")
    y = nc.dram_tensor("y", [S, E], F32, kind="ExternalOutput")

    with tile.TileContext(nc) as tc, \
         nc.allow_low_precision(reason="bf16 matmul pipeline by design"):
        with tc.tile_pool(name="persist", bufs=1) as P, \
             tc.tile_pool(name="ps", bufs=1, space="PSUM") as PS:
            # --- persistent tiles (bottom-of-stack, live whole kernel)
            qkT_sb = [P.tile([128, S], BF16, name=f"qkT{i}") for i in range(6)]
            V_sb = [P.tile([128, 65 * HPC], BF16, name=f"V{i}") for i in range(KC)]
            ctxT_sb = [P.tile([128, S], BF16, name=f"ctxT{i}") for i in range(3)]
            woT_sb = [P.tile([128, E], BF16, name=f"woT{i}") for i in range(3)]
            bqk_sb = P.tile([128, 6], F32, name="bqk_sb")
            bv_sb = P.tile([1, FPC], BF16, name="bv_sb")
            bo_sb = P.tile([1, E], BF16, name="bo_sb")
            tri_sb = P.tile([128, 128], BF16, name="tri_sb")
            id_sb = P.tile([128, 128], BF16, name="id_sb")
            on_sb = P.tile([1, 128], BF16, name="on_sb")

            def ps_tile(shape, tag, bufs, dtype=F32):
                return PS.tile(shape, dtype, name=tag, tag=tag, bufs=bufs)

            # ============ phase 1 (projections) + attention, interleaved ====
            # The attention phase is ACT(exp)-throughput-bound, so the
            # projections are software-pipelined INTO the attention loop:
            # only the chains needed for the first scores run up front, and
            # the rest are emitted between attention units where the PE has
            # slack while ACT chews on exps.
            with tc.tile_pool(name="inp", bufs=1) as PI, \
                 tc.tile_pool(name="esb", bufs=14) as EP, \
                 tc.tile_pool(name="nrm", bufs=12) as NP, \
                 tc.tile_pool(name="osb", bufs=3) as OP:
                # consolidated phase-1 tiles: one DMA dispatch covers all six
                # e-chunks (the SP sequencer costs ~650ns per DMA, so fewer,
                # bigger strided DMAs win)
                xT_sb = PI.tile([128, EC * S], BF16, name="xT_all")
                wqkT_sb = PI.tile([128, EC * 2 * FPC], BF16, name="wqkT_all")
                wvT_sb = PI.tile([128, EC * FPC], BF16, name="wvT_all")
                xs = xT_sb[:].rearrange("p (e s) -> p e s", e=EC)
                xd = xT.ap().rearrange("(e p) s -> p e s", p=128)
                qs = wqkT_sb[:].rearrange("p (e f) -> p e f", e=EC)
                qd = wqkT.ap().rearrange("(e p) f -> p e f", p=128)
                # DMA order: first the tensors gating the two startup chains
                # (wqkT cols of fo=0/3, xT cols 0:1024), then wvT (V chunks),
                # tri (first diag mask), the rest of xT/wqkT, and the tail.
                # per-chunk pass-1 xT so the startup chains pipeline with the
                # DMA stream chunk by chunk
                nc.sync.dma_start(xs[:, 0, 0:1024], xd[:, 0, 0:1024])
                nc.sync.dma_start(qs[:, :, 0:128], qd[:, :, 0:128])
                nc.sync.dma_start(qs[:, :, 384:512], qd[:, :, 384:512])
                for i in range(1, EC):
                    nc.sync.dma_start(xs[:, i, 0:1024], xd[:, i, 0:1024])
                for i in range(EC):
                    nc.sync.dma_start(xs[:, i, 1024:S], xd[:, i, 1024:S])
                nc.sync.dma_start(
                    wvT_sb[:].rearrange("p (e f) -> p e f", e=EC),
                    wvT.ap().rearrange("(e p) f -> p e f", p=128))
                nc.sync.dma_start(tri_sb[:], tri.ap())
                nc.sync.dma_start(qs[:, :, 128:384], qd[:, :, 128:384])
                nc.sync.dma_start(qs[:, :, 512:768], qd[:, :, 512:768])
                nc.sync.dma_start(id_sb[:], ident.ap())
                for i in range(3):
                    nc.sync.dma_start(woT_sb[i][:],
                                      woT.ap()[128 * i:128 * (i + 1), :])
                nc.sync.dma_start(bqk_sb[:], bqk.ap())
                nc.sync.dma_start(bv_sb[:], bv.ap())
                nc.sync.dma_start(on_sb[:], ones.ap())
                nc.sync.dma_start(bo_sb[:], bo.ap())

                def emit_qk_chain(fo, swp, windows=(0, 1), on_act=False):
                    """qk-proj for f-chunk fo, s-windows 2*swp+windows.
                    Concurrent window chains in one pss slot. on_act routes
                    the copy-out through the (startup-idle) ACT engine so it
                    overlaps the other startup chain's DVE copy."""
                    pair = ps_tile([128, 1024], "pss_t", 3)
                    for ecc in range(EC):
                        for swl in windows:
                            sw = 2 * swp + swl
                            nc.tensor.matmul(
                                pair[:, 512 * swl:512 * (swl + 1)],
                                wqkT_sb[:, 768 * ecc + 128 * fo:
                                        768 * ecc + 128 * (fo + 1)],
                                xT_sb[:, S * ecc + 512 * sw:
                                      S * ecc + 512 * (sw + 1)],
                                start=(ecc == 0), stop=(ecc == EC - 1),
                                skip_group_check=True)
                    # per-window copy-out so the first window's consumers
                    # don't wait for the second's
                    for swl in windows:
                        dst = qkT_sb[fo][:, 1024 * swp + 512 * swl:
                                         1024 * swp + 512 * (swl + 1)]
                        src = pair[:, 512 * swl:512 * (swl + 1)]
                        if with_bias:
                            nc.vector.tensor_scalar_add(
                                dst, src, bqk_sb[:, fo:fo + 1])
                        elif on_act:
                            nc.scalar.copy(dst, src)
                        else:
                            nc.vector.tensor_copy(dst, src)

                def emit_v_chunk(sc):
                    """V-proj for s-chunk sc (one k-chunk of V_aug)."""
                    psv = ps_tile([128, FPC], "pss_t", 3)
                    for ecc in range(EC):
                        nc.tensor.matmul(
                            psv[:],
                            xT_sb[:, S * ecc + 128 * sc:
                                  S * ecc + 128 * (sc + 1)],
                            wvT_sb[:, FPC * ecc:FPC * (ecc + 1)],
                            start=(ecc == 0),
                            stop=(not with_bias and ecc == EC - 1),
                            skip_group_check=True)
                    if with_bias:
                        nc.tensor.matmul(psv[:], on_sb[:, 0:128],
                                         bv_sb[:], start=False, stop=True,
                                         skip_group_check=True)
                    vv = V_sb[sc][:].rearrange("p (h x) -> p h x", x=65)
                    nc.vector.tensor_copy(
                        vv[:, :, 0:64],
                        psv[:].rearrange("p (h x) -> p h x", x=64))
                    nc.gpsimd.memset(vv[:, :, 64:65], 1.0)

                def emit_scores(hp, qw, u):
                    """Scores S^T[k, q] for a pair of k-chunks, both heads,
                    + exp, + DVE causal masks on diag blocks. Returns the
                    bf16 exp'd tile Et [128, 2048]
                    (cols 1024*hd + 512*half + qlocal)."""
                    qT, kT = qkT_sb[hp], qkT_sb[3 + hp]
                    pss = {hd: ps_tile([128, 1024], "pss_t", 3)
                           for hd in range(2)}
                    Et = EP.tile([128, 2048], BF16, name="E_t")
                    NOSHRINK = bool(int(os.environ.get("K_NOSHRINK", "0")))
                    for half in range(2):
                        ki = 2 * u + half
                        j = ki - 4 * qw
                        c = 128 * j if j > 0 and not NOSHRINK else 0
                        # strict row-group alternation (base 0,64,0,64) so the
                        # K=64 score matmul pairs run concurrently on the PE;
                        # diag tiles shrink the moving window to skip
                        # fully-masked columns
                        for hd in range(2):
                            base = 64 * hd
                            nc.tensor.matmul(
                                pss[hd][:, 512 * half + c:512 * (half + 1)],
                                kT[base:base + 64, 128 * ki:128 * (ki + 1)],
                                qT[base:base + 64,
                                   512 * qw + c:512 * (qw + 1)],
                                start=True, stop=True,
                                skip_group_check=True)
                    j0 = 2 * u - 4 * qw
                    j1 = j0 + 1
                    c0 = 128 * j0 if j0 > 0 else 0
                    # one exp per head spans both halves when contiguous;
                    # when the half-1 diag shrink leaves an unwritten PSUM
                    # gap, split the exp around it (reading the gap would
                    # race with the slot's previous occupant)
                    for hd in range(2):
                        if j1 > 0 and not NOSHRINK:
                            nc.scalar.activation(
                                Et[:, 1024 * hd + c0:1024 * hd + 512],
                                pss[hd][:, c0:512], EXP, scale=0.125)
                            c1 = 128 * j1
                            nc.scalar.activation(
                                Et[:, 1024 * hd + 512 + c1:1024 * (hd + 1)],
                                pss[hd][:, 512 + c1:1024], EXP, scale=0.125)
                        else:
                            nc.scalar.activation(
                                Et[:, 1024 * hd + c0:1024 * (hd + 1)],
                                pss[hd][:, c0:1024], EXP, scale=0.125)
                    # causal mask inside the diagonal 128x128 blocks:
                    # multiply by 0/1 upper-triangle (k<=q keeps)
                    for half in range(2):
                        j = 2 * u + half - 4 * qw
                        if j >= 0:
                            for hd in range(2):
                                off = 1024 * hd + 512 * half + 128 * j
                                nc.vector.tensor_mul(
                                    Et[:, off:off + 128],
                                    Et[:, off:off + 128], tri_sb[:])
                    return Et

                def emit_ctx_qc(hp, qw, qc, Ets, psc):
                    """ctx for one q-chunk, both heads: psc[hd][q, 65*qc+d]
                    = sum_ki E^T chunk (stationary) x V_aug chunk (moving).
                    qc-contiguous so each PSUM bank has exactly one open
                    accumulation group at a time; fully-masked (ki, qc)
                    blocks are skipped."""
                    for ki in range(0, 4 * qw + qc + 1):
                        u, half = divmod(ki, 2)
                        Et = Ets[u]
                        for hd in range(2):
                            h = 2 * hp + hd
                            nc.tensor.matmul(
                                psc[hd][:, 65 * qc:65 * qc + 65],
                                Et[:, 1024 * hd + 512 * half + 128 * qc:
                                    1024 * hd + 512 * half + 128 * qc + 128],
                                V_sb[ki][:, 65 * h:65 * h + 65],
                                start=(ki == 0), stop=(ki == 4 * qw + qc),
                                skip_group_check=True)

                def emit_norm_head(hp, qw, psc, st):
                    """Per-partition softmax normalization head: copy the
                    raw ctx out of PSUM (freeing the psc slots for the next
                    step's first ctx write) and compute the reciprocal
                    row-sums. The per-qc finish runs as separate tasks."""
                    craws = []
                    for hd in range(2):
                        craw = NP.tile([128, 260], F32, name="craw_t")
                        nc.vector.tensor_copy(craw[:], psc[hd][:])
                        craws.append(craw)
                    # pt is allocated right before its writers, keeping the
                    # slot ring in emission order
                    pt = ps_tile([128, 512], "psc_t", 2, dtype=BF16)
                    ctxns = []
                    for hd in range(2):
                        craw = craws[hd]
                        cv = craw[:].rearrange("p (q x) -> p q x", x=65)
                        rinv = NP.tile([128, 4], F32, name="rinv_t")
                        nc.vector.reciprocal(
                            rinv[:].rearrange("p (q x) -> p q x", x=1),
                            cv[:, :, 64:65])
                        ctxn = NP.tile([128, 256], BF16, name="ctxn_t")
                        ctxns.append((craw, rinv, ctxn))
                    st["pt"] = pt
                    st["ctxns"] = ctxns

                def emit_norm_qc(hp, qw, qc, st):
                    """Normalize + transpose + copy out one 128-column ctxT
                    block, so each out-proj s-chunk only waits its own."""
                    pt, ctxns = st["pt"], st["ctxns"]
                    for hd in range(2):
                        craw, rinv, ctxn = ctxns[hd]
                        nc.vector.tensor_scalar_mul(
                            ctxn[:, 64 * qc:64 * (qc + 1)],
                            craw[:, 65 * qc:65 * qc + 64],
                            rinv[:, qc:qc + 1])
                        nc.tensor.transpose(
                            pt[64 * hd:64 * (hd + 1),
                               128 * qc:128 * (qc + 1)],
                            ctxn[:, 64 * qc:64 * (qc + 1)], id_sb[:])
                    nc.vector.tensor_copy(
                        ctxT_sb[hp][:, 512 * qw + 128 * qc:
                                     512 * qw + 128 * (qc + 1)],
                        pt[:, 128 * qc:128 * (qc + 1)])

                def emit_outproj_sc(sc):
                    osb = OP.tile([128, E], F32, name="osb_t")
                    # two f-window chains on two PSUM slots, c-outer so the
                    # ctxT stationary is loaded once per c; the two slots'
                    # copy-out rotations hide each other's latency
                    # the final step's out-proj (sc>=12) runs at the
                    # drain when scores are done, so it can use the three
                    # idle pss slots and dodge the 2-slot rotation stalls
                    tg, nb = ("pss_t", 3) if sc >= 12 else ("psc_t", 2)
                    pos = {0: ps_tile([128, 512], tg, nb),
                           512: ps_tile([128, 256], tg, nb)}
                    for c in range(3):
                        for f0, fn in ((0, 512), (512, 256)):
                            nc.tensor.matmul(
                                pos[f0][:, 0:fn],
                                ctxT_sb[c][:, 128 * sc:128 * (sc + 1)],
                                woT_sb[c][:, f0:f0 + fn],
                                start=(c == 0),
                                stop=(not with_bias and c == 2),
                                skip_group_check=True)
                    for f0, fn in ((0, 512), (512, 256)):
                        if with_bias:
                            nc.tensor.matmul(pos[f0][:, 0:fn],
                                             on_sb[:, 0:128],
                                             bo_sb[:, f0:f0 + fn],
                                             start=False, stop=True,
                                             skip_group_check=True)
                        nc.vector.tensor_copy(osb[:, f0:f0 + fn],
                                              pos[f0][:, 0:fn])
                    nc.sync.dma_start(y.ap()[128 * sc:128 * (sc + 1), :],
                                      osb[:])

                def emit_attention():
                    # software pipeline: ctx runs as per-q-chunk tasks (each
                    # a full contiguous PSUM accumulation group) queued when
                    # a step's scores complete; one task is popped per unit
                    # so ctx/norm/out-proj spread between later units while
                    # ACT chews on exps.
                    work = []      # deferred ctx/norm/outproj thunks

                    def flush_one():
                        if work:
                            work.pop(0)()
                        if len(work) > 2:   # backlog guard near the end
                            work.pop(0)()

                    def make_step(hp, qw, Ets):
                        # psc is allocated lazily at the first ctx task so
                        # the PSUM slot ring advances in emission order
                        holder = {}

                        def get_psc():
                            if not holder:
                                holder[0] = {
                                    hd: ps_tile([128, 260], "psc_t", 2)
                                    for hd in range(2)}
                            return holder[0]

                        def ctx_task(qc):
                            return lambda: emit_ctx_qc(
                                hp, qw, qc, Ets, get_psc())
                        return get_psc, ctx_task

                    def finish_step(hp, qw, get_psc):
                        st = {}
                        if hp < 2:
                            def norm_all():
                                emit_norm_head(hp, qw, get_psc(), st)
                                for qc in range(4):
                                    emit_norm_qc(hp, qw, qc, st)
                            work.append(norm_all)
                        else:
                            # per-qc norm+out-proj tasks shorten the serial
                            # tail: each s-chunk starts once its own
                            # 128-column ctxT block lands
                            work.append(lambda: emit_norm_head(
                                hp, qw, get_psc(), st))

                            def norm_op(qc):
                                emit_norm_qc(hp, qw, qc, st)
                                emit_outproj_sc(4 * qw + qc)
                            for qc in range(4):
                                work.append(lambda qc=qc: norm_op(qc))

                    # phase-1 chains interleaved between attention units:
                    # (hp, qw, u) -> thunks emitted right after that unit's
                    # scores+flush (so the next exp is never delayed by a
                    # projection chain). Deadlines: qkT window-pair swp of
                    # f-chunks (hp)/(3+hp) is read by (hp, qw>=2*swp) scores;
                    # V[k] is read by the ctx of unit u=k//2, which flushes
                    # DEPTH units later. qk chains (2.6us) avoid the last
                    # unit of a step; V chains (1us) fit anywhere.
                    intra = {}

                    def add(hp, qw, u, fn):
                        intra.setdefault((hp, qw, u), []).append(fn)

                    def addv(hp, qw, u, sc):
                        add(hp, qw, u, lambda: emit_v_chunk(sc))

                    def addqk(hp, qw, u, fo, swp):
                        add(hp, qw, u, lambda: emit_qk_chain(fo, swp))

                    def addqkw(hp, qw, u, fo, swp, w):
                        add(hp, qw, u,
                            lambda: emit_qk_chain(fo, swp, windows=(w,)))

                    addqkw(0, 0, 0, 0, 0, 1)
                    addqkw(0, 0, 1, 3, 0, 1)
                    addv(0, 1, 0, 0)
                    addv(0, 1, 0, 1)
                    addv(0, 1, 1, 2)
                    addqkw(0, 1, 1, 0, 1, 0)
                    addv(0, 1, 2, 3)
                    addqkw(0, 1, 2, 3, 1, 0)
                    addv(0, 1, 3, 4)
                    addv(0, 2, 0, 5)
                    addv(0, 2, 1, 6)
                    addqkw(0, 2, 1, 0, 1, 1)
                    addv(0, 2, 2, 7)
                    addv(0, 2, 3, 8)
                    addqkw(0, 2, 3, 3, 1, 1)
                    addv(0, 2, 4, 9)
                    addv(0, 2, 5, 10)
                    addv(0, 3, 0, 11)
                    addv(0, 3, 1, 12)
                    addv(0, 3, 2, 13)
                    addv(0, 3, 3, 14)
                    addv(0, 3, 4, 15)
                    addqkw(0, 3, 5, 1, 0, 0)
                    addqkw(0, 3, 6, 1, 0, 1)
                    addqkw(0, 3, 7, 4, 0, 0)
                    addqkw(1, 0, 0, 4, 0, 1)
                    addqkw(1, 1, 0, 1, 1, 0)
                    addqkw(1, 1, 1, 4, 1, 0)
                    addqkw(1, 1, 2, 1, 1, 1)
                    addqkw(1, 1, 3, 4, 1, 1)
                    addqkw(1, 2, 0, 2, 0, 0)
                    addqkw(1, 2, 2, 5, 0, 0)
                    addqkw(1, 2, 4, 2, 0, 1)
                    addqkw(1, 3, 0, 5, 0, 1)
                    addqkw(1, 3, 2, 2, 1, 0)
                    addqkw(1, 3, 4, 5, 1, 0)
                    addqkw(1, 3, 6, 2, 1, 1)
                    addqkw(2, 0, 0, 5, 1, 1)

                    for hp in range(3):
                        for qw in range(QW):
                            nu = 2 * qw + 2
                            Ets = []
                            get_psc, ctx_task = make_step(hp, qw, Ets)
                            for u in range(nu):
                                Ets.append(emit_scores(hp, qw, u))
                                for fn in intra.get((hp, qw, u), ()):
                                    fn()
                                flush_one()
                            # ctx tasks queue only at step end: the V-chunk
                            # intra schedule is calibrated to ctx popping
                            # during the NEXT step's units
                            for qc in range(4):
                                work.append(ctx_task(qc))
                            finish_step(hp, qw, get_psc)
                    while work:
                        flush_one()

                # start-up: only what the first scores need (the (0,0)
                # step reads just s-window 0 of fo 0/3); window 1 and the
                # first V chunks follow as intra tasks behind the scores
                emit_qk_chain(0, 0, windows=(0,))
                emit_qk_chain(3, 0, windows=(0,), on_act=True)
                emit_attention()

    return _patch_multiwait(nc)


_NC = {}


def _get_nc(with_bias=True):
    if with_bias not in _NC:
        _NC[with_bias] = build_nc(with_bias=with_bias)
    return _NC[with_bias]


def _prep_core_inputs(x, in_proj_w, in_proj_b, out_w, out_b):
    """Build the 8 per-core input dicts (host-side shard + transpose)."""
    import ml_dtypes
    bf16 = ml_dtypes.bfloat16
    # 0/1 keep-mask for S^T[k, q] diagonal blocks: keep where k <= q
    tri_np = (np.arange(128)[:, None] <= np.arange(128)[None, :])
    tri_bf = tri_np.astype(bf16)
    id_bf = np.eye(128, dtype=np.float32).astype(bf16)
    ones_np = np.ones((1, 128), np.float32).astype(bf16)

    xT_by_b = [np.asarray(x[b]).T.astype(bf16) for b in range(B)]

    in_maps = []
    for c in range(8):
        b = c // 2
        g = c % 2
        f0 = FPC * g
        Wq = np.asarray(in_proj_w[f0:f0 + FPC])
        Wk = np.asarray(in_proj_w[E + f0:E + f0 + FPC])
        Wv = np.asarray(in_proj_w[2 * E + f0:2 * E + f0 + FPC])
        bq = np.asarray(in_proj_b[f0:f0 + FPC])
        bk = np.asarray(in_proj_b[E + f0:E + f0 + FPC])
        bvv = np.asarray(in_proj_b[2 * E + f0:2 * E + f0 + FPC])
        Wo = np.asarray(out_w[:, f0:f0 + FPC])
        bqk_np = np.concatenate([bq, bk]).astype(np.float32).reshape(6, 128).T
        in_maps.append({
            "xT": xT_by_b[b],
            "wqkT": np.ascontiguousarray(
                np.concatenate([Wq, Wk], axis=0).T).astype(bf16),
            "wvT": np.ascontiguousarray(Wv.T).astype(bf16),
            "woT": np.ascontiguousarray(Wo.T).astype(bf16),
            "bqk": np.ascontiguousarray(bqk_np),
            "bv": bvv.reshape(1, FPC).astype(bf16),
            # out bias only on even cores so the host-side pair-sum is exact
            "bo": np.asarray(out_b).reshape(1, E).astype(bf16) if g == 0
                  else np.zeros((1, E), bf16),
            "tri": tri_bf,
            "ident": id_bf,
            "ones": ones_np,
        })
    return in_maps


def kernel(x, in_proj_w, in_proj_b, out_w, out_b):
    zero_bias = (not np.any(np.asarray(in_proj_b))) and \
                (not np.any(np.asarray(out_b)))
    nc = _get_nc(with_bias=not zero_bias)
    in_maps = _prep_core_inputs(x, in_proj_w, in_proj_b, out_w, out_b)
    res = run_bass_kernel_spmd(nc, in_maps, core_ids=list(range(8)))
    out = np.empty((B, S, E), np.float32)
    for b in range(B):
        out[b] = res.results[2 * b]["y"] + res.results[2 * b + 1]["y"]
    return out


# revision 70
# speedup vs baseline: 1.3103x; 1.0063x over previous
"""Multi-head attention (B=4, S=2048, E=768, H=12, D=64, causal) on 8 trn2
NeuronCores.

Sharding: core c -> batch b = c//2, head-half g = c%2 (6 heads each).
Each core computes its 6 heads' attention plus the partial output
projection; the host sums the two half-head partials per batch.

On-device strategy (per core):
  - Host pre-transposes x[b] and the weight slices so every matmul
    contraction dim (e / d / k / e_h) lands on SBUF partitions; no
    on-device transposes of inputs. Everything ships bf16 (halves DMA).
  - QK projection emits qk^T [f, s]; V projection emits V [s, f] with a
    ones column packed per head (V_aug) so the PE computes softmax
    row-sums for free.
  - Scores are computed TRANSPOSED (S^T[k, q] = K^T Q) in bf16; the
    diagonal tiles shrink their moving window to skip fully-masked
    columns. Causal masking inside the diagonal 128x128 block is a DVE
    multiply of the exp'd tile by a 0/1 lower-triangle (no PE matmul).
  - ctx is computed with E^T chunks STATIONARY and V_aug [k, 65] MOVING:
    65-cycle matmuls, and fully-masked (q-chunk, k-chunk) blocks are
    skipped entirely. Each 65-col psc region is accumulated as ONE
    contiguous PSUM group (qc-outer over all k-chunks of the step) --
    interleaving several open accumulation groups inside one PSUM bank
    corrupts the early-stopping groups. The result lands [q, d+1] with
    q on partitions, so softmax normalization is a per-partition
    reciprocal + scalar multiply on DVE (no PE broadcast matmul), and a
    PE transpose brings ctx^T [d, q] back for the output projection.
  - All big matmuls run in bf16 at 1 cycle/row.
  - The attention phase is ACT(exp)-throughput-bound, so scores/exps
    stream per unit while ctx/norm/out-proj run as deferred tasks popped
    between later units, and the projection chains of phase 1 are
    interleaved into the attention loop with just-in-time deadlines.
"""
import sys, json, os

for _p in ("/opt/trn_rl_repo",):
    if _p not in sys.path and os.path.isdir(_p):
        sys.path.insert(0, _p)

import numpy as np
import concourse.bass as bass
import concourse.mybir as mybir
import concourse.tile as tile
from concourse.bass_utils import run_bass_kernel_spmd

B, S, E, H, D = 4, 2048, 768, 12, 64
HPC = H // 2          # heads per core = 6
FPC = HPC * D         # features per core per q/k/v = 384
EC = E // 128         # 6 contraction chunks for projections
SC = S // 128         # 16 s-chunks
QW = S // 512         # 4 q-windows
KC = S // 128         # 16 k-chunks
F32 = mybir.dt.float32
BF16 = mybir.dt.bfloat16
EXP = mybir.ActivationFunctionType.Exp


def _patch_multiwait(nc, max_waits=1):
    """This container's walrus rejects instructions with more than one sync
    wait. Split excess waits onto same-engine NOPs emitted immediately
    before the instruction (same-engine streams are order-preserving)."""
    raw = nc.to_json_bytes()
    m = json.loads(raw)
    for f in m["functions"]:
        for b in f["blocks"]:
            out = []
            for inst in b["instructions"]:
                si = inst.get("sync_info") or {}
                ws = si.get("on_wait") or []
                if len(ws) > max_waits:
                    eng = inst["engine"]
                    for i, w in enumerate(ws[:-max_waits]):
                        out.append({
                            "debug": inst.get("debug", 0), "engine": eng,
                            "ins": [], "name": inst["name"] + f"-mw{i}",
                            "opcode": "NoOp", "outs": [],
                            "sync_info": {"on_update": [], "on_wait": [w]},
                        })
                    si["on_wait"] = ws[-max_waits:]
                out.append(inst)
            b["instructions"] = out
    patched = json.dumps(m).encode()
    nc.to_json_bytes = lambda: patched
    return nc


def build_nc(with_bias=True):
    nc = bass.Bass()
    xT = nc.dram_tensor("xT", [E, S], BF16, kind="ExternalInput")
    wqkT = nc.dram_tensor("wqkT", [E, 2 * FPC], BF16, kind="ExternalInput")
    wvT = nc.dram_tensor("wvT", [E, FPC], BF16, kind="ExternalInput")
    woT = nc.dram_tensor("woT", [FPC, E], BF16, kind="ExternalInput")
    bqk = nc.dram_tensor("bqk", [128, 2 * FPC // 128], F32, kind="ExternalInput")
    bv = nc.dram_tensor("bv", [1, FPC], BF16, kind="ExternalInput")
    bo = nc.dram_tensor("bo", [1, E], BF16, kind="ExternalInput")
    tri = nc.dram_tensor("tri", [128, 128], BF16, kind="ExternalInput")
    ident = nc.dram_tensor("ident", [128, 128], BF16, kind="ExternalInput")
    ones = nc.dram_tensor("ones", [1, 128], BF16, kind="ExternalInput")
    y = nc.dram_tensor("y", [S, E], F32, kind="ExternalOutput")

    with tile.TileContext(nc) as tc, \
         nc.allow_low_precision(reason="bf16 matmul pipeline by design"):
        with tc.tile_pool(name="persist", bufs=1) as P, \
             tc.tile_pool(name="ps", bufs=1, space="PSUM") as PS:
            # --- persistent tiles (bottom-of-stack, live whole kernel)
            qkT_sb = [P.tile([128, S], BF16, name=f"qkT{i}") for i in range(6)]
            V_sb = [P.tile([128, 65 * HPC], BF16, name=f"V{i}") for i in range(KC)]
            ctxT_sb = [P.tile([128, S], BF16, name=f"ctxT{i}") for i in range(3)]
            woT_sb = [P.tile([128, E], BF16, name=f"woT{i}") for i in range(3)]
            bqk_sb = P.tile([128, 6], F32, name="bqk_sb")
            bv_sb = P.tile([1, FPC], BF16, name="bv_sb")
            bo_sb = P.tile([1, E], BF16, name="bo_sb")
            tri_sb = P.tile([128, 128], BF16, name="tri_sb")
            id_sb = P.tile([128, 128], BF16, name="id_sb")
            on_sb = P.tile([1, 128], BF16, name="on_sb")

            def ps_tile(shape, tag, bufs, dtype=F32):
                return PS.tile(shape, dtype, name=tag, tag=tag, bufs=bufs)

            # ============ phase 1 (projections) + attention, interleaved ====
            # The attention phase is ACT(exp)-throughput-bound, so the
            # projections are software-pipelined INTO the attention loop:
            # only the chains needed for the first scores run up front, and
            # the rest are emitted between attention units where the PE has
            # slack while ACT chews on exps.
            with tc.tile_pool(name="inp", bufs=1) as PI, \
                 tc.tile_pool(name="esb", bufs=14) as EP, \
                 tc.tile_pool(name="nrm", bufs=12) as NP, \
                 tc.tile_pool(name="osb", bufs=3) as OP:
                # consolidated phase-1 tiles: one DMA dispatch covers all six
                # e-chunks (the SP sequencer costs ~650ns per DMA, so fewer,
                # bigger strided DMAs win)
                xT_sb = PI.tile([128, EC * S], BF16, name="xT_all")
                wqkT_sb = PI.tile([128, EC * 2 * FPC], BF16, name="wqkT_all")
                wvT_sb = PI.tile([128, EC * FPC], BF16, name="wvT_all")
                xs = xT_sb[:].rearrange("p (e s) -> p e s", e=EC)
                xd = xT.ap().rearrange("(e p) s -> p e s", p=128)
                qs = wqkT_sb[:].rearrange("p (e f) -> p e f", e=EC)
                qd = wqkT.ap().rearrange("(e p) f -> p e f", p=128)
                # DMA order: first the tensors gating the two startup chains
                # (wqkT cols of fo=0/3, xT cols 0:1024), then wvT (V chunks),
                # tri (first diag mask), the rest of xT/wqkT, and the tail.
                # per-chunk pass-1 xT so the startup chains pipeline with the
                # DMA stream chunk by chunk
                nc.sync.dma_start(xs[:, 0, 0:1024], xd[:, 0, 0:1024])
                nc.sync.dma_start(qs[:, :, 0:128], qd[:, :, 0:128])
                nc.sync.dma_start(qs[:, :, 384:512], qd[:, :, 384:512])
                for i in range(1, EC):
                    nc.sync.dma_start(xs[:, i, 0:1024], xd[:, i, 0:1024])
                nc.sync.dma_start(
                    wvT_sb[:].rearrange("p (e f) -> p e f", e=EC),
                    wvT.ap().rearrange("(e p) f -> p e f", p=128))
                for i in range(EC):
                    nc.sync.dma_start(xs[:, i, 1024:S], xd[:, i, 1024:S])
                nc.sync.dma_start(tri_sb[:], tri.ap())
                nc.sync.dma_start(qs[:, :, 128:384], qd[:, :, 128:384])
                nc.sync.dma_start(qs[:, :, 512:768], qd[:, :, 512:768])
                nc.sync.dma_start(id_sb[:], ident.ap())
                for i in range(3):
                    nc.sync.dma_start(woT_sb[i][:],
                                      woT.ap()[128 * i:128 * (i + 1), :])
                nc.sync.dma_start(bqk_sb[:], bqk.ap())
                nc.sync.dma_start(bv_sb[:], bv.ap())
                nc.sync.dma_start(on_sb[:], ones.ap())
                nc.sync.dma_start(bo_sb[:], bo.ap())

                def emit_qk_chain(fo, swp, windows=(0, 1), on_act=False):
                    """qk-proj for f-chunk fo, s-windows 2*swp+windows.
                    Concurrent window chains in one pss slot. on_act routes
                    the copy-out through the (startup-idle) ACT engine so it
                    overlaps the other startup chain's DVE copy."""
                    pair = ps_tile([128, 1024], "pss_t", 3)
                    for ecc in range(EC):
                        for swl in windows:
                            sw = 2 * swp + swl
                            nc.tensor.matmul(
                                pair[:, 512 * swl:512 * (swl + 1)],
                                wqkT_sb[:, 768 * ecc + 128 * fo:
                                        768 * ecc + 128 * (fo + 1)],
                                xT_sb[:, S * ecc + 512 * sw:
                                      S * ecc + 512 * (sw + 1)],
                                start=(ecc == 0), stop=(ecc == EC - 1),
                                skip_group_check=True)
                    # per-window copy-out so the first window's consumers
                    # don't wait for the second's
                    for swl in windows:
                        dst = qkT_sb[fo][:, 1024 * swp + 512 * swl:
                                         1024 * swp + 512 * (swl + 1)]
                        src = pair[:, 512 * swl:512 * (swl + 1)]
                        if with_bias:
                            nc.vector.tensor_scalar_add(
                                dst, src, bqk_sb[:, fo:fo + 1])
                        elif on_act:
                            nc.scalar.copy(dst, src)
                        else:
                            nc.vector.tensor_copy(dst, src)

                def emit_v_chunk(sc):
                    """V-proj for s-chunk sc (one k-chunk of V_aug)."""
                    psv = ps_tile([128, FPC], "pss_t", 3)
                    for ecc in range(EC):
                        nc.tensor.matmul(
                            psv[:],
                            xT_sb[:, S * ecc + 128 * sc:
                                  S * ecc + 128 * (sc + 1)],
                            wvT_sb[:, FPC * ecc:FPC * (ecc + 1)],
                            start=(ecc == 0),
                            stop=(not with_bias and ecc == EC - 1),
                            skip_group_check=True)
                    if with_bias:
                        nc.tensor.matmul(psv[:], on_sb[:, 0:128],
                                         bv_sb[:], start=False, stop=True,
                                         skip_group_check=True)
                    vv = V_sb[sc][:].rearrange("p (h x) -> p h x", x=65)
                    nc.vector.tensor_copy(
                        vv[:, :, 0:64],
                        psv[:].rearrange("p (h x) -> p h x", x=64))
                    nc.gpsimd.memset(vv[:, :, 64:65], 1.0)

                def emit_scores(hp, qw, u):
                    """Scores S^T[k, q] for a pair of k-chunks, both heads,
                    + exp, + DVE causal masks on diag blocks. Returns the
                    bf16 exp'd tile Et [128, 2048]
                    (cols 1024*hd + 512*half + qlocal)."""
                    qT, kT = qkT_sb[hp], qkT_sb[3 + hp]
                    pss = {hd: ps_tile([128, 1024], "pss_t", 3)
                           for hd in range(2)}
                    Et = EP.tile([128, 2048], BF16, name="E_t")
                    NOSHRINK = bool(int(os.environ.get("K_NOSHRINK", "0")))
                    for half in range(2):
                        ki = 2 * u + half
                        j = ki - 4 * qw
                        c = 128 * j if j > 0 and not NOSHRINK else 0
                        # strict row-group alternation (base 0,64,0,64) so the
                        # K=64 score matmul pairs run concurrently on the PE;
                        # diag tiles shrink the moving window to skip
                        # fully-masked columns
                        for hd in range(2):
                            base = 64 * hd
                            nc.tensor.matmul(
                                pss[hd][:, 512 * half + c:512 * (half + 1)],
                                kT[base:base + 64, 128 * ki:128 * (ki + 1)],
                                qT[base:base + 64,
                                   512 * qw + c:512 * (qw + 1)],
                                start=True, stop=True,
                                skip_group_check=True)
                    j0 = 2 * u - 4 * qw
                    j1 = j0 + 1
                    c0 = 128 * j0 if j0 > 0 else 0
                    # one exp per head spans both halves when contiguous;
                    # when the half-1 diag shrink leaves an unwritten PSUM
                    # gap, split the exp around it (reading the gap would
                    # race with the slot's previous occupant)
                    for hd in range(2):
                        if j1 > 0 and not NOSHRINK:
                            nc.scalar.activation(
                                Et[:, 1024 * hd + c0:1024 * hd + 512],
                                pss[hd][:, c0:512], EXP, scale=0.125)
                            c1 = 128 * j1
                            nc.scalar.activation(
                                Et[:, 1024 * hd + 512 + c1:1024 * (hd + 1)],
                                pss[hd][:, 512 + c1:1024], EXP, scale=0.125)
                        else:
                            nc.scalar.activation(
                                Et[:, 1024 * hd + c0:1024 * (hd + 1)],
                                pss[hd][:, c0:1024], EXP, scale=0.125)
                    # causal mask inside the diagonal 128x128 blocks:
                    # multiply by 0/1 upper-triangle (k<=q keeps)
                    for half in range(2):
                        j = 2 * u + half - 4 * qw
                        if j >= 0:
                            for hd in range(2):
                                off = 1024 * hd + 512 * half + 128 * j
                                nc.vector.tensor_mul(
                                    Et[:, off:off + 128],
                                    Et[:, off:off + 128], tri_sb[:])
                    return Et

                def emit_ctx_qc(hp, qw, qc, Ets, psc):
                    """ctx for one q-chunk, both heads: psc[hd][q, 65*qc+d]
                    = sum_ki E^T chunk (stationary) x V_aug chunk (moving).
                    qc-contiguous so each PSUM bank has exactly one open
                    accumulation group at a time; fully-masked (ki, qc)
                    blocks are skipped."""
                    for ki in range(0, 4 * qw + qc + 1):
                        u, half = divmod(ki, 2)
                        Et = Ets[u]
                        for hd in range(2):
                            h = 2 * hp + hd
                            nc.tensor.matmul(
                                psc[hd][:, 65 * qc:65 * qc + 65],
                                Et[:, 1024 * hd + 512 * half + 128 * qc:
                                    1024 * hd + 512 * half + 128 * qc + 128],
                                V_sb[ki][:, 65 * h:65 * h + 65],
                                start=(ki == 0), stop=(ki == 4 * qw + qc),
                                skip_group_check=True)

                def emit_norm_head(hp, qw, psc, st):
                    """Per-partition softmax normalization head: copy the
                    raw ctx out of PSUM (freeing the psc slots for the next
                    step's first ctx write) and compute the reciprocal
                    row-sums. The per-qc finish runs as separate tasks."""
                    craws = []
                    for hd in range(2):
                        craw = NP.tile([128, 260], F32, name="craw_t")
                        nc.vector.tensor_copy(craw[:], psc[hd][:])
                        craws.append(craw)
                    # pt is allocated right before its writers, keeping the
                    # slot ring in emission order
                    pt = ps_tile([128, 512], "psc_t", 2, dtype=BF16)
                    ctxns = []
                    for hd in range(2):
                        craw = craws[hd]
                        cv = craw[:].rearrange("p (q x) -> p q x", x=65)
                        rinv = NP.tile([128, 4], F32, name="rinv_t")
                        nc.vector.reciprocal(
                            rinv[:].rearrange("p (q x) -> p q x", x=1),
                            cv[:, :, 64:65])
                        ctxn = NP.tile([128, 256], BF16, name="ctxn_t")
                        ctxns.append((craw, rinv, ctxn))
                    st["pt"] = pt
                    st["ctxns"] = ctxns

                def emit_norm_qc(hp, qw, qc, st):
                    """Normalize + transpose + copy out one 128-column ctxT
                    block, so each out-proj s-chunk only waits its own."""
                    pt, ctxns = st["pt"], st["ctxns"]
                    for hd in range(2):
                        craw, rinv, ctxn = ctxns[hd]
                        nc.vector.tensor_scalar_mul(
                            ctxn[:, 64 * qc:64 * (qc + 1)],
                            craw[:, 65 * qc:65 * qc + 64],
                            rinv[:, qc:qc + 1])
                        nc.tensor.transpose(
                            pt[64 * hd:64 * (hd + 1),
                               128 * qc:128 * (qc + 1)],
                            ctxn[:, 64 * qc:64 * (qc + 1)], id_sb[:])
                    nc.vector.tensor_copy(
                        ctxT_sb[hp][:, 512 * qw + 128 * qc:
                                     512 * qw + 128 * (qc + 1)],
                        pt[:, 128 * qc:128 * (qc + 1)])

                def emit_outproj_sc(sc):
                    osb = OP.tile([128, E], F32, name="osb_t")
                    # two f-window chains on two PSUM slots, c-outer so the
                    # ctxT stationary is loaded once per c; the two slots'
                    # copy-out rotations hide each other's latency
                    # the final step's out-proj (sc>=12) runs at the
                    # drain when scores are done, so it can use the three
                    # idle pss slots and dodge the 2-slot rotation stalls
                    tg, nb = ("pss_t", 3) if sc >= 12 else ("psc_t", 2)
                    pos = {0: ps_tile([128, 512], tg, nb),
                           512: ps_tile([128, 256], tg, nb)}
                    for c in range(3):
                        for f0, fn in ((0, 512), (512, 256)):
                            nc.tensor.matmul(
                                pos[f0][:, 0:fn],
                                ctxT_sb[c][:, 128 * sc:128 * (sc + 1)],
                                woT_sb[c][:, f0:f0 + fn],
                                start=(c == 0),
                                stop=(not with_bias and c == 2),
                                skip_group_check=True)
                    for f0, fn in ((0, 512), (512, 256)):
                        if with_bias:
                            nc.tensor.matmul(pos[f0][:, 0:fn],
                                             on_sb[:, 0:128],
                                             bo_sb[:, f0:f0 + fn],
                                             start=False, stop=True,
                                             skip_group_check=True)
                        nc.vector.tensor_copy(osb[:, f0:f0 + fn],
                                              pos[f0][:, 0:fn])
                    nc.sync.dma_start(y.ap()[128 * sc:128 * (sc + 1), :],
                                      osb[:])

                def emit_attention():
                    # software pipeline: ctx runs as per-q-chunk tasks (each
                    # a full contiguous PSUM accumulation group) queued when
                    # a step's scores complete; one task is popped per unit
                    # so ctx/norm/out-proj spread between later units while
                    # ACT chews on exps.
                    work = []      # deferred ctx/norm/outproj thunks

                    def flush_one():
                        if work:
                            work.pop(0)()
                        if len(work) > 2:   # backlog guard near the end
                            work.pop(0)()

                    def make_step(hp, qw, Ets):
                        # psc is allocated lazily at the first ctx task so
                        # the PSUM slot ring advances in emission order
                        holder = {}

                        def get_psc():
                            if not holder:
                                holder[0] = {
                                    hd: ps_tile([128, 260], "psc_t", 2)
                                    for hd in range(2)}
                            return holder[0]

                        def ctx_task(qc):
                            return lambda: emit_ctx_qc(
                                hp, qw, qc, Ets, get_psc())
                        return get_psc, ctx_task

                    def finish_step(hp, qw, get_psc):
                        st = {}
                        if hp < 2:
                            def norm_all():
                                emit_norm_head(hp, qw, get_psc(), st)
                                for qc in range(4):
                                    emit_norm_qc(hp, qw, qc, st)
                            work.append(norm_all)
                        else:
                            # per-qc norm+out-proj tasks shorten the serial
                            # tail: each s-chunk starts once its own
                            # 128-column ctxT block lands
                            work.append(lambda: emit_norm_head(
                                hp, qw, get_psc(), st))

                            def norm_op(qc):
                                emit_norm_qc(hp, qw, qc, st)
                                emit_outproj_sc(4 * qw + qc)
                            for qc in range(4):
                                work.append(lambda qc=qc: norm_op(qc))

                    # phase-1 chains interleaved between attention units:
                    # (hp, qw, u) -> thunks emitted right after that unit's
                    # scores+flush (so the next exp is never delayed by a
                    # projection chain). Deadlines: qkT window-pair swp of
                    # f-chunks (hp)/(3+hp) is read by (hp, qw>=2*swp) scores;
                    # V[k] is read by the ctx of unit u=k//2, which flushes
                    # DEPTH units later. qk chains (2.6us) avoid the last
                    # unit of a step; V chains (1us) fit anywhere.
                    intra = {}

                    def add(hp, qw, u, fn):
                        intra.setdefault((hp, qw, u), []).append(fn)

                    def addv(hp, qw, u, sc):
                        add(hp, qw, u, lambda: emit_v_chunk(sc))

                    def addqk(hp, qw, u, fo, swp):
                        add(hp, qw, u, lambda: emit_qk_chain(fo, swp))

                    def addqkw(hp, qw, u, fo, swp, w):
                        add(hp, qw, u,
                            lambda: emit_qk_chain(fo, swp, windows=(w,)))

                    addqkw(0, 0, 0, 0, 0, 1)
                    addv(0, 0, 0, 0)
                    addqkw(0, 0, 1, 3, 0, 1)
                    addv(0, 0, 1, 1)
                    addv(0, 1, 0, 2)
                    addqkw(0, 1, 1, 0, 1, 0)
                    addv(0, 1, 1, 3)
                    addqkw(0, 1, 2, 3, 1, 0)
                    addv(0, 1, 2, 4)
                    addv(0, 1, 3, 5)
                    addv(0, 2, 0, 6)
                    addqkw(0, 2, 1, 0, 1, 1)
                    addv(0, 2, 1, 7)
                    addv(0, 2, 2, 8)
                    addqkw(0, 2, 3, 3, 1, 1)
                    addv(0, 2, 3, 9)
                    addv(0, 2, 4, 10)
                    addv(0, 3, 0, 11)
                    addv(0, 3, 1, 12)
                    addv(0, 3, 2, 13)
                    addv(0, 3, 3, 14)
                    addv(0, 3, 4, 15)
                    addqkw(0, 3, 5, 1, 0, 0)
                    addqkw(0, 3, 6, 1, 0, 1)
                    addqkw(0, 3, 7, 4, 0, 0)
                    addqkw(1, 0, 0, 4, 0, 1)
                    addqkw(1, 1, 0, 1, 1, 0)
                    addqkw(1, 1, 1, 4, 1, 0)
                    addqkw(1, 1, 2, 1, 1, 1)
                    addqkw(1, 1, 3, 4, 1, 1)
                    addqkw(1, 2, 0, 2, 0, 0)
                    addqkw(1, 2, 2, 5, 0, 0)
                    addqkw(1, 2, 4, 2, 0, 1)
                    addqkw(1, 3, 0, 5, 0, 1)
                    addqkw(1, 3, 2, 2, 1, 0)
                    addqkw(1, 3, 4, 5, 1, 0)
                    addqkw(1, 3, 6, 2, 1, 1)
                    addqkw(2, 0, 0, 5, 1, 1)

                    for hp in range(3):
                        for qw in range(QW):
                            nu = 2 * qw + 2
                            Ets = []
                            get_psc, ctx_task = make_step(hp, qw, Ets)
                            for u in range(nu):
                                Ets.append(emit_scores(hp, qw, u))
                                for fn in intra.get((hp, qw, u), ()):
                                    fn()
                                flush_one()
                            # ctx tasks queue only at step end: the V-chunk
                            # intra schedule is calibrated to ctx popping
                            # during the NEXT step's units
                            for qc in range(4):
                                work.append(ctx_task(qc))
                            finish_step(hp, qw, get_psc)
                    while work:
                        flush_one()

                # start-up: only what the first scores need (the (0,0)
                # step reads just s-window 0 of fo 0/3); window 1 and the
                # first V chunks follow as intra tasks behind the scores
                emit_qk_chain(0, 0, windows=(0,))
                emit_qk_chain(3, 0, windows=(0,), on_act=True)
                emit_attention()

    return _patch_multiwait(nc)


_NC = {}


def _get_nc(with_bias=True):
    if with_bias not in _NC:
        _NC[with_bias] = build_nc(with_bias=with_bias)
    return _NC[with_bias]


def _prep_core_inputs(x, in_proj_w, in_proj_b, out_w, out_b):
    """Build the 8 per-core input dicts (host-side shard + transpose)."""
    import ml_dtypes
    bf16 = ml_dtypes.bfloat16
    # 0/1 keep-mask for S^T[k, q] diagonal blocks: keep where k <= q
    tri_np = (np.arange(128)[:, None] <= np.arange(128)[None, :])
    tri_bf = tri_np.astype(bf16)
    id_bf = np.eye(128, dtype=np.float32).astype(bf16)
    ones_np = np.ones((1, 128), np.float32).astype(bf16)

    xT_by_b = [np.asarray(x[b]).T.astype(bf16) for b in range(B)]

    in_maps = []
    for c in range(8):
        b = c // 2
        g = c % 2
        f0 = FPC * g
        Wq = np.asarray(in_proj_w[f0:f0 + FPC])
        Wk = np.asarray(in_proj_w[E + f0:E + f0 + FPC])
        Wv = np.asarray(in_proj_w[2 * E + f0:2 * E + f0 + FPC])
        bq = np.asarray(in_proj_b[f0:f0 + FPC])
        bk = np.asarray(in_proj_b[E + f0:E + f0 + FPC])
        bvv = np.asarray(in_proj_b[2 * E + f0:2 * E + f0 + FPC])
        Wo = np.asarray(out_w[:, f0:f0 + FPC])
        bqk_np = np.concatenate([bq, bk]).astype(np.float32).reshape(6, 128).T
        in_maps.append({
            "xT": xT_by_b[b],
            "wqkT": np.ascontiguousarray(
                np.concatenate([Wq, Wk], axis=0).T).astype(bf16),
            "wvT": np.ascontiguousarray(Wv.T).astype(bf16),
            "woT": np.ascontiguousarray(Wo.T).astype(bf16),
            "bqk": np.ascontiguousarray(bqk_np),
            "bv": bvv.reshape(1, FPC).astype(bf16),
            # out bias only on even cores so the host-side pair-sum is exact
            "bo": np.asarray(out_b).reshape(1, E).astype(bf16) if g == 0
                  else np.zeros((1, E), bf16),
            "tri": tri_bf,
            "ident": id_bf,
            "ones": ones_np,
        })
    return in_maps


def kernel(x, in_proj_w, in_proj_b, out_w, out_b):
    zero_bias = (not np.any(np.asarray(in_proj_b))) and \
                (not np.any(np.asarray(out_b)))
    nc = _get_nc(with_bias=not zero_bias)
    in_maps = _prep_core_inputs(x, in_proj_w, in_proj_b, out_w, out_b)
    res = run_bass_kernel_spmd(nc, in_maps, core_ids=list(range(8)))
    out = np.empty((B, S, E), np.float32)
    for b in range(B):
        out[b] = res.results[2 * b]["y"] + res.results[2 * b + 1]["y"]
    return out


# revision 73
# speedup vs baseline: 1.3154x; 1.0039x over previous
"""Multi-head attention (B=4, S=2048, E=768, H=12, D=64, causal) on 8 trn2
NeuronCores.

Sharding: core c -> batch b = c//2, head-half g = c%2 (6 heads each).
Each core computes its 6 heads' attention plus the partial output
projection; the host sums the two half-head partials per batch.

On-device strategy (per core):
  - Host pre-transposes x[b] and the weight slices so every matmul
    contraction dim (e / d / k / e_h) lands on SBUF partitions; no
    on-device transposes of inputs. Everything ships bf16 (halves DMA).
  - QK projection emits qk^T [f, s]; V projection emits V [s, f] with a
    ones column packed per head (V_aug) so the PE computes softmax
    row-sums for free.
  - Scores are computed TRANSPOSED (S^T[k, q] = K^T Q) in bf16; the
    diagonal tiles shrink their moving window to skip fully-masked
    columns. Causal masking inside the diagonal 128x128 block is a DVE
    multiply of the exp'd tile by a 0/1 lower-triangle (no PE matmul).
  - ctx is computed with E^T chunks STATIONARY and V_aug [k, 65] MOVING:
    65-cycle matmuls, and fully-masked (q-chunk, k-chunk) blocks are
    skipped entirely. Each 65-col psc region is accumulated as ONE
    contiguous PSUM group (qc-outer over all k-chunks of the step) --
    interleaving several open accumulation groups inside one PSUM bank
    corrupts the early-stopping groups. The result lands [q, d+1] with
    q on partitions, so softmax normalization is a per-partition
    reciprocal + scalar multiply on DVE (no PE broadcast matmul), and a
    PE transpose brings ctx^T [d, q] back for the output projection.
  - All big matmuls run in bf16 at 1 cycle/row.
  - The attention phase is ACT(exp)-throughput-bound, so scores/exps
    stream per unit while ctx/norm/out-proj run as deferred tasks popped
    between later units, and the projection chains of phase 1 are
    interleaved into the attention loop with just-in-time deadlines.
"""
import sys, json, os

for _p in ("/opt/trn_rl_repo",):
    if _p not in sys.path and os.path.isdir(_p):
        sys.path.insert(0, _p)

import numpy as np
import concourse.bass as bass
import concourse.mybir as mybir
import concourse.tile as tile
from concourse.bass_utils import run_bass_kernel_spmd

B, S, E, H, D = 4, 2048, 768, 12, 64
HPC = H // 2          # heads per core = 6
FPC = HPC * D         # features per core per q/k/v = 384
EC = E // 128         # 6 contraction chunks for projections
SC = S // 128         # 16 s-chunks
QW = S // 512         # 4 q-windows
KC = S // 128         # 16 k-chunks
F32 = mybir.dt.float32
BF16 = mybir.dt.bfloat16
EXP = mybir.ActivationFunctionType.Exp


def _patch_multiwait(nc, max_waits=1):
    """This container's walrus rejects instructions with more than one sync
    wait. Split excess waits onto same-engine NOPs emitted immediately
    before the instruction (same-engine streams are order-preserving)."""
    raw = nc.to_json_bytes()
    m = json.loads(raw)
    for f in m["functions"]:
        for b in f["blocks"]:
            out = []
            for inst in b["instructions"]:
                si = inst.get("sync_info") or {}
                ws = si.get("on_wait") or []
                if len(ws) > max_waits:
                    eng = inst["engine"]
                    for i, w in enumerate(ws[:-max_waits]):
                        out.append({
                            "debug": inst.get("debug", 0), "engine": eng,
                            "ins": [], "name": inst["name"] + f"-mw{i}",
                            "opcode": "NoOp", "outs": [],
                            "sync_info": {"on_update": [], "on_wait": [w]},
                        })
                    si["on_wait"] = ws[-max_waits:]
                out.append(inst)
            b["instructions"] = out
    patched = json.dumps(m).encode()
    nc.to_json_bytes = lambda: patched
    return nc


def build_nc(with_bias=True):
    nc = bass.Bass()
    xT = nc.dram_tensor("xT", [E, S], BF16, kind="ExternalInput")
    wqkT = nc.dram_tensor("wqkT", [E, 2 * FPC], BF16, kind="ExternalInput")
    wvT = nc.dram_tensor("wvT", [E, FPC], BF16, kind="ExternalInput")
    woT = nc.dram_tensor("woT", [FPC, E], BF16, kind="ExternalInput")
    bqk = nc.dram_tensor("bqk", [128, 2 * FPC // 128], F32, kind="ExternalInput")
    bv = nc.dram_tensor("bv", [1, FPC], BF16, kind="ExternalInput")
    bo = nc.dram_tensor("bo", [1, E], BF16, kind="ExternalInput")
    tri = nc.dram_tensor("tri", [128, 128], BF16, kind="ExternalInput")
    ident = nc.dram_tensor("ident", [128, 128], BF16, kind="ExternalInput")
    ones = nc.dram_tensor("ones", [1, 128], BF16, kind="ExternalInput")
    y = nc.dram_tensor("y", [S, E], F32, kind="ExternalOutput")

    with tile.TileContext(nc) as tc, \
         nc.allow_low_precision(reason="bf16 matmul pipeline by design"):
        with tc.tile_pool(name="persist", bufs=1) as P, \
             tc.tile_pool(name="ps", bufs=1, space="PSUM") as PS:
            # --- persistent tiles (bottom-of-stack, live whole kernel)
            qkT_sb = [P.tile([128, S], BF16, name=f"qkT{i}") for i in range(6)]
            V_sb = [P.tile([128, 65 * HPC], BF16, name=f"V{i}") for i in range(KC)]
            ctxT_sb = [P.tile([128, S], BF16, name=f"ctxT{i}") for i in range(3)]
            woT_sb = [P.tile([128, E], BF16, name=f"woT{i}") for i in range(3)]
            bqk_sb = P.tile([128, 6], F32, name="bqk_sb")
            bv_sb = P.tile([1, FPC], BF16, name="bv_sb")
            bo_sb = P.tile([1, E], BF16, name="bo_sb")
            tri_sb = P.tile([128, 128], BF16, name="tri_sb")
            id_sb = P.tile([128, 128], BF16, name="id_sb")
            on_sb = P.tile([1, 128], BF16, name="on_sb")

            def ps_tile(shape, tag, bufs, dtype=F32):
                return PS.tile(shape, dtype, name=tag, tag=tag, bufs=bufs)

            # ============ phase 1 (projections) + attention, interleaved ====
            # The attention phase is ACT(exp)-throughput-bound, so the
            # projections are software-pipelined INTO the attention loop:
            # only the chains needed for the first scores run up front, and
            # the rest are emitted between attention units where the PE has
            # slack while ACT chews on exps.
            with tc.tile_pool(name="inp", bufs=1) as PI, \
                 tc.tile_pool(name="esb", bufs=14) as EP, \
                 tc.tile_pool(name="nrm", bufs=12) as NP, \
                 tc.tile_pool(name="osb", bufs=3) as OP:
                # consolidated phase-1 tiles: one DMA dispatch covers all six
                # e-chunks (the SP sequencer costs ~650ns per DMA, so fewer,
                # bigger strided DMAs win)
                xT_sb = PI.tile([128, EC * S], BF16, name="xT_all")
                wqkT_sb = PI.tile([128, EC * 2 * FPC], BF16, name="wqkT_all")
                wvT_sb = PI.tile([128, EC * FPC], BF16, name="wvT_all")
                xs = xT_sb[:].rearrange("p (e s) -> p e s", e=EC)
                xd = xT.ap().rearrange("(e p) s -> p e s", p=128)
                qs = wqkT_sb[:].rearrange("p (e f) -> p e f", e=EC)
                qd = wqkT.ap().rearrange("(e p) f -> p e f", p=128)
                # DMA order: first the tensors gating the two startup chains
                # (wqkT cols of fo=0/3, xT cols 0:1024), then wvT (V chunks),
                # tri (first diag mask), the rest of xT/wqkT, and the tail.
                # per-chunk pass-1 xT so the startup chains pipeline with the
                # DMA stream chunk by chunk
                nc.sync.dma_start(xs[:, 0, 0:1024], xd[:, 0, 0:1024])
                nc.sync.dma_start(qs[:, :, 0:128], qd[:, :, 0:128])
                nc.sync.dma_start(qs[:, :, 384:512], qd[:, :, 384:512])
                for i in range(1, EC):
                    nc.sync.dma_start(xs[:, i, 0:1024], xd[:, i, 0:1024])
                nc.sync.dma_start(
                    wvT_sb[:].rearrange("p (e f) -> p e f", e=EC),
                    wvT.ap().rearrange("(e p) f -> p e f", p=128))
                for i in range(EC):
                    nc.sync.dma_start(xs[:, i, 1024:S], xd[:, i, 1024:S])
                nc.sync.dma_start(tri_sb[:], tri.ap())
                nc.sync.dma_start(qs[:, :, 128:384], qd[:, :, 128:384])
                nc.sync.dma_start(qs[:, :, 512:768], qd[:, :, 512:768])
                nc.sync.dma_start(id_sb[:], ident.ap())
                for i in range(3):
                    nc.sync.dma_start(woT_sb[i][:],
                                      woT.ap()[128 * i:128 * (i + 1), :])
                nc.sync.dma_start(bqk_sb[:], bqk.ap())
                nc.sync.dma_start(bv_sb[:], bv.ap())
                nc.sync.dma_start(on_sb[:], ones.ap())
                nc.sync.dma_start(bo_sb[:], bo.ap())

                def emit_qk_chain(fo, swp, windows=(0, 1), on_act=False):
                    """qk-proj for f-chunk fo, s-windows 2*swp+windows.
                    Concurrent window chains in one pss slot. on_act routes
                    the copy-out through the (startup-idle) ACT engine so it
                    overlaps the other startup chain's DVE copy."""
                    pair = ps_tile([128, 1024], "pss_t", 3)
                    for ecc in range(EC):
                        for swl in windows:
                            sw = 2 * swp + swl
                            nc.tensor.matmul(
                                pair[:, 512 * swl:512 * (swl + 1)],
                                wqkT_sb[:, 768 * ecc + 128 * fo:
                                        768 * ecc + 128 * (fo + 1)],
                                xT_sb[:, S * ecc + 512 * sw:
                                      S * ecc + 512 * (sw + 1)],
                                start=(ecc == 0), stop=(ecc == EC - 1),
                                skip_group_check=True)
                    # per-window copy-out so the first window's consumers
                    # don't wait for the second's
                    for swl in windows:
                        dst = qkT_sb[fo][:, 1024 * swp + 512 * swl:
                                         1024 * swp + 512 * (swl + 1)]
                        src = pair[:, 512 * swl:512 * (swl + 1)]
                        if with_bias:
                            nc.vector.tensor_scalar_add(
                                dst, src, bqk_sb[:, fo:fo + 1])
                        elif on_act:
                            nc.scalar.copy(dst, src)
                        else:
                            nc.vector.tensor_copy(dst, src)

                def emit_v_chunk(sc):
                    """V-proj for s-chunk sc (one k-chunk of V_aug)."""
                    psv = ps_tile([128, FPC], "pss_t", 3)
                    for ecc in range(EC):
                        nc.tensor.matmul(
                            psv[:],
                            xT_sb[:, S * ecc + 128 * sc:
                                  S * ecc + 128 * (sc + 1)],
                            wvT_sb[:, FPC * ecc:FPC * (ecc + 1)],
                            start=(ecc == 0),
                            stop=(not with_bias and ecc == EC - 1),
                            skip_group_check=True)
                    if with_bias:
                        nc.tensor.matmul(psv[:], on_sb[:, 0:128],
                                         bv_sb[:], start=False, stop=True,
                                         skip_group_check=True)
                    vv = V_sb[sc][:].rearrange("p (h x) -> p h x", x=65)
                    nc.vector.tensor_copy(
                        vv[:, :, 0:64],
                        psv[:].rearrange("p (h x) -> p h x", x=64))
                    nc.gpsimd.memset(vv[:, :, 64:65], 1.0)

                def emit_scores(hp, qw, u):
                    """Scores S^T[k, q] for a pair of k-chunks, both heads,
                    + exp, + DVE causal masks on diag blocks. Returns the
                    bf16 exp'd tile Et [128, 2048]
                    (cols 1024*hd + 512*half + qlocal)."""
                    qT, kT = qkT_sb[hp], qkT_sb[3 + hp]
                    pss = {hd: ps_tile([128, 1024], "pss_t", 3)
                           for hd in range(2)}
                    Et = EP.tile([128, 2048], BF16, name="E_t")
                    NOSHRINK = bool(int(os.environ.get("K_NOSHRINK", "0")))
                    for half in range(2):
                        ki = 2 * u + half
                        j = ki - 4 * qw
                        c = 128 * j if j > 0 and not NOSHRINK else 0
                        # strict row-group alternation (base 0,64,0,64) so the
                        # K=64 score matmul pairs run concurrently on the PE;
                        # diag tiles shrink the moving window to skip
                        # fully-masked columns
                        for hd in range(2):
                            base = 64 * hd
                            nc.tensor.matmul(
                                pss[hd][:, 512 * half + c:512 * (half + 1)],
                                kT[base:base + 64, 128 * ki:128 * (ki + 1)],
                                qT[base:base + 64,
                                   512 * qw + c:512 * (qw + 1)],
                                start=True, stop=True,
                                skip_group_check=True)
                    j0 = 2 * u - 4 * qw
                    j1 = j0 + 1
                    c0 = 128 * j0 if j0 > 0 else 0
                    # one exp per head spans both halves when contiguous;
                    # when the half-1 diag shrink leaves an unwritten PSUM
                    # gap, split the exp around it (reading the gap would
                    # race with the slot's previous occupant)
                    for hd in range(2):
                        if j1 > 0 and not NOSHRINK:
                            nc.scalar.activation(
                                Et[:, 1024 * hd + c0:1024 * hd + 512],
                                pss[hd][:, c0:512], EXP, scale=0.125)
                            c1 = 128 * j1
                            nc.scalar.activation(
                                Et[:, 1024 * hd + 512 + c1:1024 * (hd + 1)],
                                pss[hd][:, 512 + c1:1024], EXP, scale=0.125)
                        else:
                            nc.scalar.activation(
                                Et[:, 1024 * hd + c0:1024 * (hd + 1)],
                                pss[hd][:, c0:1024], EXP, scale=0.125)
                    # causal mask inside the diagonal 128x128 blocks:
                    # multiply by 0/1 upper-triangle (k<=q keeps)
                    for half in range(2):
                        j = 2 * u + half - 4 * qw
                        if j >= 0:
                            for hd in range(2):
                                off = 1024 * hd + 512 * half + 128 * j
                                nc.vector.tensor_mul(
                                    Et[:, off:off + 128],
                                    Et[:, off:off + 128], tri_sb[:])
                    return Et

                def emit_ctx_qc(hp, qw, qc, Ets, psc):
                    """ctx for one q-chunk, both heads: psc[hd][q, 65*qc+d]
                    = sum_ki E^T chunk (stationary) x V_aug chunk (moving).
                    qc-contiguous so each PSUM bank has exactly one open
                    accumulation group at a time; fully-masked (ki, qc)
                    blocks are skipped."""
                    for ki in range(0, 4 * qw + qc + 1):
                        u, half = divmod(ki, 2)
                        Et = Ets[u]
                        for hd in range(2):
                            h = 2 * hp + hd
                            nc.tensor.matmul(
                                psc[hd][:, 65 * qc:65 * qc + 65],
                                Et[:, 1024 * hd + 512 * half + 128 * qc:
                                    1024 * hd + 512 * half + 128 * qc + 128],
                                V_sb[ki][:, 65 * h:65 * h + 65],
                                start=(ki == 0), stop=(ki == 4 * qw + qc),
                                skip_group_check=True)

                def emit_norm_head(hp, qw, psc, st):
                    """Per-partition softmax normalization head: copy the
                    raw ctx out of PSUM (freeing the psc slots for the next
                    step's first ctx write) and compute the reciprocal
                    row-sums. The per-qc finish runs as separate tasks."""
                    craws = []
                    for hd in range(2):
                        craw = NP.tile([128, 260], F32, name="craw_t")
                        nc.vector.tensor_copy(craw[:], psc[hd][:])
                        craws.append(craw)
                    # pt is allocated right before its writers, keeping the
                    # slot ring in emission order
                    pt = ps_tile([128, 512], "psc_t", 2, dtype=BF16)
                    ctxns = []
                    for hd in range(2):
                        craw = craws[hd]
                        cv = craw[:].rearrange("p (q x) -> p q x", x=65)
                        rinv = NP.tile([128, 4], F32, name="rinv_t")
                        nc.vector.reciprocal(
                            rinv[:].rearrange("p (q x) -> p q x", x=1),
                            cv[:, :, 64:65])
                        ctxn = NP.tile([128, 256], BF16, name="ctxn_t")
                        ctxns.append((craw, rinv, ctxn))
                    st["pt"] = pt
                    st["ctxns"] = ctxns

                def emit_norm_qc(hp, qw, qc, st):
                    """Normalize + transpose + copy out one 128-column ctxT
                    block, so each out-proj s-chunk only waits its own."""
                    pt, ctxns = st["pt"], st["ctxns"]
                    for hd in range(2):
                        craw, rinv, ctxn = ctxns[hd]
                        nc.vector.tensor_scalar_mul(
                            ctxn[:, 64 * qc:64 * (qc + 1)],
                            craw[:, 65 * qc:65 * qc + 64],
                            rinv[:, qc:qc + 1])
                        nc.tensor.transpose(
                            pt[64 * hd:64 * (hd + 1),
                               128 * qc:128 * (qc + 1)],
                            ctxn[:, 64 * qc:64 * (qc + 1)], id_sb[:])
                    nc.vector.tensor_copy(
                        ctxT_sb[hp][:, 512 * qw + 128 * qc:
                                     512 * qw + 128 * (qc + 1)],
                        pt[:, 128 * qc:128 * (qc + 1)])

                def emit_outproj_sc(sc):
                    osb = OP.tile([128, E], F32, name="osb_t")
                    # two f-window chains on two PSUM slots, c-outer so the
                    # ctxT stationary is loaded once per c; the two slots'
                    # copy-out rotations hide each other's latency
                    # the final step's out-proj (sc>=12) runs at the
                    # drain when scores are done, so it can use the three
                    # idle pss slots and dodge the 2-slot rotation stalls
                    tg, nb = ("pss_t", 3) if sc >= 12 else ("psc_t", 2)
                    pos = {0: ps_tile([128, 512], tg, nb),
                           512: ps_tile([128, 256], tg, nb)}
                    for c in range(3):
                        for f0, fn in ((0, 512), (512, 256)):
                            nc.tensor.matmul(
                                pos[f0][:, 0:fn],
                                ctxT_sb[c][:, 128 * sc:128 * (sc + 1)],
                                woT_sb[c][:, f0:f0 + fn],
                                start=(c == 0),
                                stop=(not with_bias and c == 2),
                                skip_group_check=True)
                    for f0, fn in ((0, 512), (512, 256)):
                        if with_bias:
                            nc.tensor.matmul(pos[f0][:, 0:fn],
                                             on_sb[:, 0:128],
                                             bo_sb[:, f0:f0 + fn],
                                             start=False, stop=True,
                                             skip_group_check=True)
                        nc.vector.tensor_copy(osb[:, f0:f0 + fn],
                                              pos[f0][:, 0:fn])
                    nc.sync.dma_start(y.ap()[128 * sc:128 * (sc + 1), :],
                                      osb[:])

                def emit_attention():
                    # software pipeline: ctx runs as per-q-chunk tasks (each
                    # a full contiguous PSUM accumulation group) queued when
                    # a step's scores complete; one task is popped per unit
                    # so ctx/norm/out-proj spread between later units while
                    # ACT chews on exps.
                    work = []      # deferred ctx/norm/outproj thunks

                    def flush_one():
                        if work:
                            work.pop(0)()
                        if len(work) > 3:   # backlog guard near the end
                            work.pop(0)()

                    def make_step(hp, qw, Ets):
                        # psc is allocated lazily at the first ctx task so
                        # the PSUM slot ring advances in emission order
                        holder = {}

                        def get_psc():
                            if not holder:
                                holder[0] = {
                                    hd: ps_tile([128, 260], "psc_t", 2)
                                    for hd in range(2)}
                            return holder[0]

                        def ctx_task(qc):
                            return lambda: emit_ctx_qc(
                                hp, qw, qc, Ets, get_psc())
                        return get_psc, ctx_task

                    def finish_step(hp, qw, get_psc):
                        st = {}
                        if hp < 2:
                            def norm_all():
                                emit_norm_head(hp, qw, get_psc(), st)
                                for qc in range(4):
                                    emit_norm_qc(hp, qw, qc, st)
                            work.append(norm_all)
                        else:
                            # per-qc norm+out-proj tasks shorten the serial
                            # tail: each s-chunk starts once its own
                            # 128-column ctxT block lands
                            work.append(lambda: emit_norm_head(
                                hp, qw, get_psc(), st))

                            def norm_op(qc):
                                emit_norm_qc(hp, qw, qc, st)
                                emit_outproj_sc(4 * qw + qc)
                            for qc in range(4):
                                work.append(lambda qc=qc: norm_op(qc))

                    # phase-1 chains interleaved between attention units:
                    # (hp, qw, u) -> thunks emitted right after that unit's
                    # scores+flush (so the next exp is never delayed by a
                    # projection chain). Deadlines: qkT window-pair swp of
                    # f-chunks (hp)/(3+hp) is read by (hp, qw>=2*swp) scores;
                    # V[k] is read by the ctx of unit u=k//2, which flushes
                    # DEPTH units later. qk chains (2.6us) avoid the last
                    # unit of a step; V chains (1us) fit anywhere.
                    intra = {}

                    def add(hp, qw, u, fn):
                        intra.setdefault((hp, qw, u), []).append(fn)

                    def addv(hp, qw, u, sc):
                        add(hp, qw, u, lambda: emit_v_chunk(sc))

                    def addqk(hp, qw, u, fo, swp):
                        add(hp, qw, u, lambda: emit_qk_chain(fo, swp))

                    def addqkw(hp, qw, u, fo, swp, w):
                        add(hp, qw, u,
                            lambda: emit_qk_chain(fo, swp, windows=(w,)))

                    addqkw(0, 0, 0, 0, 0, 1)
                    addv(0, 0, 0, 0)
                    addqkw(0, 0, 1, 3, 0, 1)
                    addv(0, 0, 1, 1)
                    addv(0, 1, 0, 2)
                    addqkw(0, 1, 1, 0, 1, 0)
                    addv(0, 1, 1, 3)
                    addqkw(0, 1, 2, 3, 1, 0)
                    addv(0, 1, 2, 4)
                    addv(0, 1, 3, 5)
                    addv(0, 2, 0, 6)
                    addqkw(0, 2, 1, 0, 1, 1)
                    addv(0, 2, 1, 7)
                    addv(0, 2, 2, 8)
                    addqkw(0, 2, 3, 3, 1, 1)
                    addv(0, 2, 3, 9)
                    addv(0, 2, 4, 10)
                    addv(0, 3, 0, 11)
                    addv(0, 3, 1, 12)
                    addv(0, 3, 2, 13)
                    addv(0, 3, 3, 14)
                    addv(0, 3, 4, 15)
                    addqkw(0, 3, 5, 1, 0, 0)
                    addqkw(0, 3, 6, 1, 0, 1)
                    addqkw(0, 3, 7, 4, 0, 0)
                    addqkw(1, 0, 0, 4, 0, 1)
                    addqkw(1, 1, 0, 1, 1, 0)
                    addqkw(1, 1, 1, 4, 1, 0)
                    addqkw(1, 1, 2, 1, 1, 1)
                    addqkw(1, 1, 3, 4, 1, 1)
                    addqkw(1, 2, 0, 2, 0, 0)
                    addqkw(1, 2, 2, 5, 0, 0)
                    addqkw(1, 2, 4, 2, 0, 1)
                    addqkw(1, 3, 0, 5, 0, 1)
                    addqkw(1, 3, 2, 2, 1, 0)
                    addqkw(1, 3, 4, 5, 1, 0)
                    addqkw(1, 3, 6, 2, 1, 1)
                    addqkw(2, 0, 0, 5, 1, 1)

                    for hp in range(3):
                        for qw in range(QW):
                            nu = 2 * qw + 2
                            Ets = []
                            get_psc, ctx_task = make_step(hp, qw, Ets)
                            for u in range(nu):
                                Ets.append(emit_scores(hp, qw, u))
                                for fn in intra.get((hp, qw, u), ()):
                                    fn()
                                flush_one()
                            # ctx tasks queue only at step end: the V-chunk
                            # intra schedule is calibrated to ctx popping
                            # during the NEXT step's units
                            for qc in range(4):
                                work.append(ctx_task(qc))
                            finish_step(hp, qw, get_psc)
                    while work:
                        flush_one()

                # start-up: only what the first scores need (the (0,0)
                # step reads just s-window 0 of fo 0/3); window 1 and the
                # first V chunks follow as intra tasks behind the scores
                emit_qk_chain(0, 0, windows=(0,))
                emit_qk_chain(3, 0, windows=(0,), on_act=True)
                emit_attention()

    return _patch_multiwait(nc)


_NC = {}


def _get_nc(with_bias=True):
    if with_bias not in _NC:
        _NC[with_bias] = build_nc(with_bias=with_bias)
    return _NC[with_bias]


def _prep_core_inputs(x, in_proj_w, in_proj_b, out_w, out_b):
    """Build the 8 per-core input dicts (host-side shard + transpose)."""
    import ml_dtypes
    bf16 = ml_dtypes.bfloat16
    # 0/1 keep-mask for S^T[k, q] diagonal blocks: keep where k <= q
    tri_np = (np.arange(128)[:, None] <= np.arange(128)[None, :])
    tri_bf = tri_np.astype(bf16)
    id_bf = np.eye(128, dtype=np.float32).astype(bf16)
    ones_np = np.ones((1, 128), np.float32).astype(bf16)

    xT_by_b = [np.asarray(x[b]).T.astype(bf16) for b in range(B)]

    in_maps = []
    for c in range(8):
        b = c // 2
        g = c % 2
        f0 = FPC * g
        Wq = np.asarray(in_proj_w[f0:f0 + FPC])
        Wk = np.asarray(in_proj_w[E + f0:E + f0 + FPC])
        Wv = np.asarray(in_proj_w[2 * E + f0:2 * E + f0 + FPC])
        bq = np.asarray(in_proj_b[f0:f0 + FPC])
        bk = np.asarray(in_proj_b[E + f0:E + f0 + FPC])
        bvv = np.asarray(in_proj_b[2 * E + f0:2 * E + f0 + FPC])
        Wo = np.asarray(out_w[:, f0:f0 + FPC])
        bqk_np = np.concatenate([bq, bk]).astype(np.float32).reshape(6, 128).T
        in_maps.append({
            "xT": xT_by_b[b],
            "wqkT": np.ascontiguousarray(
                np.concatenate([Wq, Wk], axis=0).T).astype(bf16),
            "wvT": np.ascontiguousarray(Wv.T).astype(bf16),
            "woT": np.ascontiguousarray(Wo.T).astype(bf16),
            "bqk": np.ascontiguousarray(bqk_np),
            "bv": bvv.reshape(1, FPC).astype(bf16),
            # out bias only on even cores so the host-side pair-sum is exact
            "bo": np.asarray(out_b).reshape(1, E).astype(bf16) if g == 0
                  else np.zeros((1, E), bf16),
            "tri": tri_bf,
            "ident": id_bf,
            "ones": ones_np,
        })
    return in_maps


def kernel(x, in_proj_w, in_proj_b, out_w, out_b):
    zero_bias = (not np.any(np.asarray(in_proj_b))) and \
                (not np.any(np.asarray(out_b)))
    nc = _get_nc(with_bias=not zero_bias)
    in_maps = _prep_core_inputs(x, in_proj_w, in_proj_b, out_w, out_b)
    res = run_bass_kernel_spmd(nc, in_maps, core_ids=list(range(8)))
    out = np.empty((B, S, E), np.float32)
    for b in range(B):
        out[b] = res.results[2 * b]["y"] + res.results[2 * b + 1]["y"]
    return out
